# revision 1
# baseline (speedup 1.0000x reference)
"""Trainium2 Bass kernel for nn_Attention_LR_65249143160949 (cross-attention block).

Sharding: 8 cores = 4 batches x 2 token-halves (1152 tokens each). Each core
computes k/v for its whole batch (cheap MQA single head, duplicated within the
pair) and q/attention/output for its own tokens. The host permutes tokens so
each core's own rows come first -> identical SPMD program, no collectives.

On-chip layout: features on partitions, tokens on the free axis (matches the
channels-first HBM layout; no input transpose). LayerNorm is folded into the
projections: q = rs_i * (x @ Wq' - mu_i * colsum(Wq')), with Wq' pre-scaled on
the host; k/v analogous (rank-1 -colsum*mu matmul accumulated into the same
psum group). Attention runs in sim^T layout (keys on partitions, query tokens
on the free axis): kT is pre-scaled by rs_j so softmax is a plain exp; the
denominator comes free as a ones-column appended to v (row 64 of the out
psum); out^T columns are normalized by a PE-broadcast reciprocal row.
Per-token LN stats come from PE matmuls (ones as one operand), never from
cross-partition vector ops.

Precision: fp32 end-to-end math except the five big matmul groups
(q/kv/sim/attn.v/out-proj), whose operands are bf16 with fp32 PSUM
accumulation. LN statistics, softmax normalization, and the residual path
stay fp32.

Two walrus quirks are handled: every TPB instruction holds at most ONE sync
wait (extras are split onto same-engine NoOps by _split_multi_waits), and
custom DVE ops are unavailable (exact reciprocal is used).
"""

import sys

import numpy as np

if "/opt/trn_rl_repo" not in sys.path:
    sys.path.insert(0, "/opt/trn_rl_repo")

C = 512          # channels
N = 2304         # tokens per batch (48*48)
NH = 1152        # tokens per core
HEADS = 8
DH = 64
CTXL = 77
CTXD = 768
JT = 19          # j tiles of 128: 18 img + 1 (ctx 0:77 | null 77 | pad)
JP = JT * 128
CHUNKS = [(0, 512), (512, 512), (1024, 128)]  # (start, len) token chunks
NCH = len(CHUNKS)
KT = 4           # C / 128
EPS = 1e-5

PROFILE = False
PROFILE_DIR = None

_cached = {}


def _split_multi_waits(nc):
    """Walrus codegen supports one sync-wait per TPB instruction (the EVENTS
    struct has a single wait slot). Tile attaches several. Split the extras
    onto same-engine NoOps inserted just before each instruction."""
    import concourse.mybir as mybir

    n = 0
    for fn in nc.m.functions:
        for bb in fn.blocks:
            insts = bb.instructions
            i = 0
            while i < len(insts):
                ins = insts[i]
                si = getattr(ins, "sync_info", None)
                if si is not None and si.on_wait and len(si.on_wait) > 1:
                    waits = list(si.on_wait)
                    for w in waits[:-1]:
                        n += 1
                        nop = mybir.InstNoOp(name=f"WSPLIT-{n}", engine=ins.engine)
                        nop.sync_info = mybir.SyncInfo(on_wait=[w], on_update=[])
                        insts.insert(i, nop)
                        i += 1
                    ins.sync_info = mybir.SyncInfo(
                        on_wait=[waits[-1]], on_update=si.on_update)
                i += 1
    return n


def _build_bass():
    import concourse.bass as bass
    import concourse.mybir as mybir
    import concourse.tile as tile
    from concourse.masks import make_identity
    from contextlib import ExitStack

    F32 = mybir.dt.float32
    BF = mybir.dt.bfloat16
    AF = mybir.ActivationFunctionType
    ALU = mybir.AluOpType

    nc = bass.Bass()
    x_own = nc.declare_dram_parameter("x_own", [C, NH], F32, isOutput=False)
    x_oth = nc.declare_dram_parameter("x_oth", [C, NH], F32, isOutput=False)
    ctxt = nc.declare_dram_parameter("ctxt", [CTXL, CTXD], F32, isOutput=False)
    wq = nc.declare_dram_parameter("wq", [C, C], BF, isOutput=False)
    negcq = nc.declare_dram_parameter("negcq", [1, C], BF, isOutput=False)
    wkv = nc.declare_dram_parameter("wkv", [C, 2 * DH], BF, isOutput=False)
    ncsk = nc.declare_dram_parameter("ncsk", [1, DH], BF, isOutput=False)
    ncsv = nc.declare_dram_parameter("ncsv", [1, DH], BF, isOutput=False)
    wctx = nc.declare_dram_parameter("wctx", [CTXD, 2 * DH], F32, isOutput=False)
    bctxk = nc.declare_dram_parameter("bctxk", [DH, 1], F32, isOutput=False)
    bctxv = nc.declare_dram_parameter("bctxv", [DH, 1], F32, isOutput=False)
    nullkt = nc.declare_dram_parameter("nullkt", [DH, 1], F32, isOutput=False)
    nullv = nc.declare_dram_parameter("nullv", [DH, 1], F32, isOutput=False)
    wout = nc.declare_dram_parameter("wout", [DH, HEADS * C], BF, isOutput=False)
    outg = nc.declare_dram_parameter("outg", [128, KT], F32, isOutput=False)
    y = nc.declare_dram_parameter("y", [C, NH], F32, isOutput=True)

    with tile.TileContext(nc) as tc, ExitStack() as ctx:
        pconst = ctx.enter_context(tc.tile_pool(name="const", bufs=1))
        pbig = ctx.enter_context(tc.tile_pool(name="big", bufs=1))

        ident = pconst.tile([128, 128], F32)
        make_identity(nc, ident[:])
        ident_bf = pconst.tile([128, 128], BF)
        make_identity(nc, ident_bf[:])
        ones_col = pconst.tile([128, 1], F32)
        nc.vector.memset(ones_col[:], 1.0)
        ones_blk = pconst.tile([128, 128], F32)
        nc.vector.memset(ones_blk[:], 1.0)
        eps_col = pconst.tile([128, 1], F32)
        nc.vector.memset(eps_col[:], EPS)

        x_sb = pbig.tile([128, KT * N], F32)         # kt-major; own rows first
        x_bf = pbig.tile([128, KT * N], BF)
        qT = pbig.tile([128, (HEADS // 2) * NH], BF)  # head-pair blocks
        kT2 = pbig.tile([128, JP], BF)               # rs-scaled keys, both halves
        v_sb = pbig.tile([128, JT * (DH + 1)], BF)   # per j-tile [128, 64+ones]
        projT = pbig.tile([128, KT * NH], F32)
        stats = pbig.tile([128, 40], F32)            # col jt: rs_j (v scaling)
        wout_sb = pbig.tile([64, HEADS * C], BF)
        outg_sb = pbig.tile([128, KT], F32)
        # per-token stat rows on partition 0: mu 0:N | rs N:2N
        # (LN2 reuses per cc: mu2 at cc*CH, rs2 at N+cc*CH, ex2 at 2N+cc*CH)
        rows = pbig.tile([1, 2 * N + NH], F32)
        rows_bf = pbig.tile([1, N], BF)
        R_RS, R_SC = N, 2 * N

        nc.sync.dma_start(wout_sb[:], wout[:, :])
        nc.sync.dma_start(outg_sb[:], outg[:, :])

        with tc.tile_pool(name="load", bufs=1) as pload, \
             tc.tile_pool(name="x2p", bufs=2) as px2, \
             tc.tile_pool(name="pss", bufs=2, space="PSUM") as pss:
            # setup psum tags: b1 [<=64,384]x2, bS [128,<=512]x4, bT [128,128]x2
            wq_sb = pload.tile([128, KT * C], BF)
            wkv_sb = pload.tile([128, KT * 2 * DH], BF)
            wctx_sb = pload.tile([128, CTXD], F32)
            negcq_sb = pload.tile([1, C], BF)
            ncsk_sb = pload.tile([1, DH], BF)
            ncsv_sb = pload.tile([1, DH], BF)
            bctxk_sb = pload.tile([DH, 1], F32)
            bctxv_sb = pload.tile([DH, 1], F32)
            vT = pload.tile([64, N], BF)
            ck_sb = pload.tile([64, CTXL], F32)
            cv_sb = pload.tile([64, CTXL + 1], F32)
            nullk_st = pload.tile([DH, 1], F32)
            nullv_st = pload.tile([DH, 1], F32)
            ctx_sb = pload.tile([CTXL, CTXD], F32)
            ctxnT = pload.tile([128, 6 * CTXL], F32)
            ex2 = pload.tile([1, N], F32)

            x_v = x_sb[:].rearrange("p (k n) -> p k n", k=KT)
            nc.sync.dma_start(x_v[:, :, 0:NH],
                              x_own[:].rearrange("(k p) n -> p k n", p=128))
            nc.sync.dma_start(x_v[:, :, NH:N],
                              x_oth[:].rearrange("(k p) n -> p k n", p=128))
            nc.sync.dma_start(wq_sb[:].rearrange("p (k n) -> p k n", k=KT),
                              wq[:].rearrange("(k p) n -> p k n", p=128))
            nc.sync.dma_start(wkv_sb[:].rearrange("p (k n) -> p k n", k=KT),
                              wkv[:].rearrange("(k p) n -> p k n", p=128))
            nc.sync.dma_start(wctx_sb[:].rearrange("p (k n) -> p k n", k=6),
                              wctx[:].rearrange("(k p) n -> p k n", p=128))
            nc.sync.dma_start(negcq_sb[:], negcq[:, :])
            nc.sync.dma_start(ncsk_sb[:], ncsk[:, :])
            nc.sync.dma_start(ncsv_sb[:], ncsv[:, :])
            nc.sync.dma_start(bctxk_sb[:], bctxk[:, :])
            nc.sync.dma_start(bctxv_sb[:], bctxv[:, :])
            nc.sync.dma_start(ctx_sb[:], ctxt[:, :])
            nc.sync.dma_start(nullk_st[:], nullkt[:, :])
            nc.sync.dma_start(nullv_st[:], nullv[:, :])

            for kt in range(KT):
                nc.vector.tensor_copy(x_bf[:, kt * N : (kt + 1) * N],
                                      x_sb[:, kt * N : (kt + 1) * N])

            # ---- context: LN (layout A, bn_stats) + k/v projection ----
            cstat = pload.tile([CTXL, 3, 6], F32)
            for sg in range(3):
                nc.vector.bn_stats(cstat[:, sg, :],
                                   ctx_sb[:, sg * 256 : (sg + 1) * 256])
            cmv = pload.tile([CTXL, 2], F32)
            nc.vector.bn_aggr(cmv[:], cstat[:])
            nc.scalar.activation(cmv[:, 1:2], cmv[:, 1:2], AF.Ln,
                                 bias=eps_col[0:CTXL, :])
            nc.scalar.activation(cmv[:, 1:2], cmv[:, 1:2], AF.Exp, scale=-0.5)
            nc.vector.tensor_scalar(
                out=ctx_sb[:], in0=ctx_sb[:],
                scalar1=cmv[:, 0:1], scalar2=cmv[:, 1:2],
                op0=ALU.subtract, op1=ALU.mult)
            for kt in range(6):
                ps_ct = pss.tile([128, 128], F32, tag="bT")
                nc.tensor.transpose(ps_ct[:, 0:CTXL],
                                    ctx_sb[:, kt * 128 : (kt + 1) * 128],
                                    ident[:CTXL, :CTXL])
                nc.vector.tensor_copy(ctxnT[:, kt * CTXL : (kt + 1) * CTXL],
                                      ps_ct[:, 0:CTXL])
            ps_ck = pss.tile([64, 384], F32, tag="b1")
            ps_cv = pss.tile([64, 384], F32, tag="b1")
            for kt in range(6):
                nc.tensor.matmul(ps_ck[:, 0:CTXL],
                                 wctx_sb[:, kt * 128 : kt * 128 + DH],
                                 ctxnT[:, kt * CTXL : (kt + 1) * CTXL],
                                 start=(kt == 0), stop=(kt == 5))
                nc.tensor.matmul(ps_cv[:, 0:CTXL],
                                 wctx_sb[:, kt * 128 + DH : (kt + 1) * 128],
                                 ctxnT[:, kt * CTXL : (kt + 1) * CTXL],
                                 start=(kt == 0), stop=(kt == 5))
            nc.vector.tensor_scalar_add(ck_sb[:], ps_ck[:, 0:CTXL], bctxk_sb[:])
            nc.vector.tensor_scalar_add(cv_sb[:, 0:CTXL], ps_cv[:, 0:CTXL],
                                        bctxv_sb[:])
            nc.vector.tensor_copy(cv_sb[:, CTXL : CTXL + 1], nullv_st[:])

            # ---- LN1 stats (row form): mu, then rs = exp(-0.5 ln(var+eps)) ----
            ones_col_bf = pconst.tile([128, 1], BF)
            nc.vector.memset(ones_col_bf[:], 1.0)
            for ch in range(6):
                sl = slice(ch * 384, (ch + 1) * 384)
                ps_r1 = pss.tile([64, 384], F32, tag="b1")
                for kt in range(KT):
                    nc.tensor.matmul(
                        ps_r1[0:1, :], ones_col_bf[:],
                        x_bf[:, kt * N + ch * 384 : kt * N + (ch + 1) * 384],
                        start=(kt == 0), stop=(kt == KT - 1))
                nc.scalar.mul(rows[0:1, sl], ps_r1[0:1, :], 1.0 / C)
                nc.vector.tensor_copy(rows_bf[0:1, sl], rows[0:1, sl])
            for ch in range(6):
                x2 = px2.tile([128, KT * 384], BF, tag="x2")
                ps_r2 = pss.tile([64, 384], F32, tag="b1")
                for kt in range(KT):
                    xs = x_bf[:, kt * N + ch * 384 : kt * N + (ch + 1) * 384]
                    nc.vector.tensor_mul(x2[:, kt * 384 : (kt + 1) * 384], xs, xs)
                    nc.tensor.matmul(
                        ps_r2[0:1, :], ones_col_bf[:],
                        x2[:, kt * 384 : (kt + 1) * 384],
                        start=(kt == 0), stop=(kt == KT - 1))
                nc.scalar.mul(ex2[0:1, ch * 384 : (ch + 1) * 384],
                              ps_r2[0:1, :], 1.0 / C)
            for ch in range(6):
                a, b = R_RS + ch * 384, R_RS + (ch + 1) * 384
                mu = rows[0:1, ch * 384 : (ch + 1) * 384]
                nc.vector.tensor_mul(rows[0:1, a:b], mu, mu)
                nc.vector.tensor_sub(rows[0:1, a:b],
                                     ex2[0:1, ch * 384 : (ch + 1) * 384],
                                     rows[0:1, a:b])
                nc.scalar.activation(rows[0:1, a:b], rows[0:1, a:b], AF.Ln,
                                     bias=eps_col[0:1, :])
                nc.scalar.activation(rows[0:1, a:b], rows[0:1, a:b], AF.Exp,
                                     scale=-0.5)
            # rs as per-partition columns (v scaling)
            for jt in range(18):
                ps_c = pss.tile([128, 128], F32, tag="bT")
                nc.tensor.matmul(ps_c[:, 0:1],
                                 rows[0:1, R_RS + jt * 128 : R_RS + (jt + 1) * 128],
                                 ones_col[0:1, :])
                nc.vector.tensor_copy(stats[:, jt : jt + 1], ps_c[:, 0:1])

            # ---- j-tile 18: [ctx 0:77 | null 77 | pad 78:128] ----
            VB = 18 * (DH + 1)
            nc.vector.memset(kT2[0:64, 18 * 128 : JP], 0.0)
            nc.vector.tensor_copy(kT2[0:64, 18 * 128 : 18 * 128 + CTXL], ck_sb[:])
            nc.vector.tensor_copy(kT2[0:64, 18 * 128 + CTXL : 18 * 128 + CTXL + 1],
                                  nullk_st[:])
            nc.vector.memset(v_sb[:, VB : VB + DH + 1], 0.0)
            ps_cvt = pss.tile([128, 128], F32, tag="bT")
            nc.tensor.transpose(ps_cvt[0 : CTXL + 1, 0:64], cv_sb[:],
                                ident[:64, :64])
            nc.vector.tensor_copy(v_sb[0 : CTXL + 1, VB : VB + DH],
                                  ps_cvt[0 : CTXL + 1, 0:64])
            nc.vector.memset(v_sb[0 : CTXL + 1, VB + DH : VB + DH + 1], 1.0)

            # ---- kv projection (all tokens; LN folded; kT rs-scaled) ----
            KV_CHUNKS = [(0, 512), (512, 512), (1024, 512), (1536, 512), (2048, 256)]
            for ch, (kva, kvl) in enumerate(KV_CHUNKS):
                sl = slice(kva, kva + kvl)
                ps_k = pss.tile([128, 512], F32, tag="bS")
                ps_v = pss.tile([128, 512], F32, tag="bS")
                for kt in range(KT):
                    xs = x_bf[:, kt * N + kva : kt * N + kva + kvl]
                    nc.tensor.matmul(ps_k[0:64, 0:kvl],
                                     wkv_sb[:, kt * 2 * DH : kt * 2 * DH + DH],
                                     xs, start=(kt == 0), stop=False)
                    nc.tensor.matmul(ps_v[0:64, 0:kvl],
                                     wkv_sb[:, kt * 2 * DH + DH : (kt + 1) * 2 * DH],
                                     xs, start=(kt == 0), stop=False)
                nc.tensor.matmul(ps_k[0:64, 0:kvl], ncsk_sb[:], rows_bf[0:1, sl],
                                 start=False, stop=True)
                nc.tensor.matmul(ps_v[0:64, 0:kvl], ncsv_sb[:], rows_bf[0:1, sl],
                                 start=False, stop=True)
                ps_bc = pss.tile([128, 512], F32, tag="bS")
                nc.tensor.matmul(ps_bc[0:64, 0:kvl], ones_blk[0:1, 0:64],
                                 rows[0:1, R_RS + kva : R_RS + kva + kvl])
                kk = px2.tile([64, 512], F32, tag="kk")
                nc.vector.tensor_copy(kk[:, 0:kvl], ps_k[0:64, 0:kvl])
                nc.vector.tensor_mul(kT2[0:64, sl], kk[:, 0:kvl], ps_bc[0:64, 0:kvl])
                nc.vector.tensor_copy(vT[:, sl], ps_v[0:64, 0:kvl])

            # ---- v tiles: transpose + rs scale + ones col ----
            for jt in range(18):
                ps_vt = pss.tile([128, 128], BF, tag="bT")
                nc.tensor.transpose(ps_vt[:, 0:64], vT[:, jt * 128 : (jt + 1) * 128],
                                    ident_bf[:64, :64])
                vb = jt * (DH + 1)
                nc.vector.tensor_scalar_mul(v_sb[:, vb : vb + DH], ps_vt[:, 0:64],
                                            stats[:, jt : jt + 1])
                nc.vector.memset(v_sb[:, vb + DH : vb + DH + 1], 1.0)

            # ---- duplicate kT to partitions 64:128 (sbuf->sbuf DMA) ----
            nc.sync.dma_start(kT2[64:128, :], kT2[0:64, :])

            # ---- q projection (head pairs; LN + 1/sqrt(dh) folded) ----
            for a0, ln in CHUNKS:
                sl = slice(a0, a0 + ln)
                ps_rs = pss.tile([128, 512], F32, tag="bS")
                nc.tensor.matmul(ps_rs[:, 0:ln], ones_blk[0:1, :],
                                 rows[0:1, R_RS + a0 : R_RS + a0 + ln])
                rs_b = px2.tile([128, 512], F32, tag="rsb")
                nc.vector.tensor_copy(rs_b[:, 0:ln], ps_rs[:, 0:ln])
                for hg in range(HEADS // 2):
                    ps_q = pss.tile([128, 512], F32, tag="bS")
                    for kt in range(KT):
                        nc.tensor.matmul(
                            ps_q[:, 0:ln],
                            wq_sb[:, kt * C + hg * 128 : kt * C + (hg + 1) * 128],
                            x_bf[:, kt * N + a0 : kt * N + a0 + ln],
                            start=(kt == 0), stop=False)
                    nc.tensor.matmul(ps_q[:, 0:ln],
                                     negcq_sb[0:1, hg * 128 : (hg + 1) * 128],
                                     rows_bf[0:1, sl], start=False, stop=True)
                    nc.vector.tensor_mul(
                        qT[:, hg * NH + a0 : hg * NH + a0 + ln],
                        ps_q[:, 0:ln], rs_b[:, 0:ln])

        # ========= attention + output + LN2 + residual, per chunk =========
        # Per (chunk, head-pair): row-packed sims -> one exp -> attn.v pair
        # lagging one j-tile. Softmax normalization and the whole output tail
        # (out-proj, LN2, y) are DEFERRED one stage so slow reciprocals and
        # tail matmuls never head-of-line-block the in-order PE queue.
        with tc.tile_pool(name="attn", bufs=3) as pattn, \
             tc.tile_pool(name="outp", bufs=9) as pout, \
             tc.tile_pool(name="pocp", bufs=10) as ppoc, \
             tc.tile_pool(name="recp", bufs=10) as prec, \
             tc.tile_pool(name="rbsp", bufs=2) as prbs, \
             tc.tile_pool(name="p2p", bufs=2) as pp2, \
             tc.tile_pool(name="yp", bufs=2) as pyt, \
             tc.tile_pool(name="psatt", bufs=2, space="PSUM") as psA, \
             tc.tile_pool(name="psacc", bufs=4, space="PSUM") as psB:
            pending_tail = [None]

            def run_hg(hg, a0, ln, recs, pocs):
                po0 = psB.tile([128, 512], F32, tag="po")
                po1 = psB.tile([128, 512], F32, tag="po")
                po = [po0, po1]
                q0 = qT[0:64, hg * NH + a0 : hg * NH + a0 + ln]
                q1 = qT[64:128, hg * NH + a0 : hg * NH + a0 + ln]
                off1 = 512
                ats = [None] * JT
                for jt in range(JT):
                    ps_s = psA.tile([128, 1024], F32, tag="sim")
                    nc.tensor.matmul(ps_s[:, 0:ln],
                                     kT2[0:64, jt * 128 : (jt + 1) * 128],
                                     q0, start=True, stop=True)
                    nc.tensor.matmul(ps_s[:, off1 : off1 + ln],
                                     kT2[64:128, jt * 128 : (jt + 1) * 128],
                                     q1, start=True, stop=True)
                    at = pattn.tile([128, 1024], BF, tag="at")
                    if ln == 512:
                        nc.scalar.activation(at[:], ps_s[:], AF.Exp)
                    else:
                        nc.scalar.activation(at[:, 0:ln], ps_s[:, 0:ln], AF.Exp)
                        nc.scalar.activation(at[:, 512 : 512 + ln],
                                             ps_s[:, 512 : 512 + ln], AF.Exp)
                    ats[jt] = at
                    if jt > 0:
                        j0 = jt - 1
                        vs = v_sb[:, j0 * (DH + 1) : (j0 + 1) * (DH + 1)]
                        nc.tensor.matmul(po[0][0:65, 0:ln], vs, ats[j0][:, 0:ln],
                                         start=(j0 == 0), stop=False)
                        nc.tensor.matmul(po[1][0:65, 0:ln], vs,
                                         ats[j0][:, off1 : off1 + ln],
                                         start=(j0 == 0), stop=False)
                        ats[j0] = None
                j0 = JT - 1
                vs = v_sb[:, j0 * (DH + 1) : (j0 + 1) * (DH + 1)]
                nc.tensor.matmul(po[0][0:65, 0:ln], vs, ats[j0][:, 0:ln],
                                 start=False, stop=True)
                nc.tensor.matmul(po[1][0:65, 0:ln], vs,
                                 ats[j0][:, off1 : off1 + ln],
                                 start=False, stop=True)
                for i in range(2):
                    rec = prec.tile([65, 512], F32, tag="rec")
                    nc.vector.reciprocal(rec[64:65, 0:ln], po[i][64:65, 0:ln])
                    poc = ppoc.tile([64, 512], F32, tag="poc")
                    nc.vector.tensor_copy(poc[:, 0:ln], po[i][0:64, 0:ln])
                    recs.append(rec)
                    pocs.append(poc)

            def tail_phases(cc, a0, ln, recs, pocs):
                ots = []

                def ph_norm():
                    for h in range(HEADS):
                        ps_rb = psB.tile([128, 512], F32, tag="po")
                        nc.tensor.matmul(ps_rb[0:64, 0:ln], ones_blk[64:65, 0:64],
                                         recs[h][64:65, 0:ln])
                        rb_sb = prbs.tile([64, 512], F32, tag="rbs")
                        nc.vector.tensor_copy(rb_sb[:, 0:ln], ps_rb[0:64, 0:ln])
                        ot = pout.tile([64, 512], BF, tag="ot")
                        nc.vector.tensor_mul(ot[:, 0:ln], pocs[h][:, 0:ln],
                                             rb_sb[:, 0:ln])
                        ots.append(ot)

                def ph_proj():
                    for ct in range(KT):
                        ps_p = psB.tile([128, 512], F32, tag="po")
                        for h in range(HEADS):
                            nc.tensor.matmul(
                                ps_p[:, 0:ln],
                                wout_sb[:, h * C + ct * 128 : h * C + (ct + 1) * 128],
                                ots[h][:, 0:ln],
                                start=(h == 0), stop=(h == HEADS - 1))
                        nc.vector.tensor_copy(
                            projT[:, ct * NH + a0 : ct * NH + a0 + ln],
                            ps_p[:, 0:ln])

                def ph_ln2():
                    ra, rb2 = R_RS + a0, R_RS + a0 + ln
                    sca, scb = R_SC + a0, R_SC + a0 + ln
                    ps_m2 = psB.tile([128, 512], F32, tag="po")
                    for ct in range(KT):
                        nc.tensor.matmul(
                            ps_m2[0:1, 0:ln], ones_col[:],
                            projT[:, ct * NH + a0 : ct * NH + a0 + ln],
                            start=(ct == 0), stop=(ct == KT - 1))
                    nc.scalar.mul(rows[0:1, a0 : a0 + ln], ps_m2[0:1, 0:ln], 1.0 / C)
                    p2 = pp2.tile([128, KT * 512], F32, tag="p2")
                    ps_q2 = psB.tile([128, 512], F32, tag="po")
                    for ct in range(KT):
                        pslc = projT[:, ct * NH + a0 : ct * NH + a0 + ln]
                        nc.vector.tensor_mul(p2[:, ct * 512 : ct * 512 + ln],
                                             pslc, pslc)
                        nc.tensor.matmul(ps_q2[0:1, 0:ln], ones_col[:],
                                         p2[:, ct * 512 : ct * 512 + ln],
                                         start=(ct == 0), stop=(ct == KT - 1))
                    nc.scalar.mul(rows[0:1, sca:scb], ps_q2[0:1, 0:ln], 1.0 / C)
                    nc.vector.tensor_mul(rows[0:1, ra:rb2], rows[0:1, a0 : a0 + ln],
                                         rows[0:1, a0 : a0 + ln])
                    nc.vector.tensor_sub(rows[0:1, ra:rb2], rows[0:1, sca:scb],
                                         rows[0:1, ra:rb2])
                    nc.scalar.activation(rows[0:1, ra:rb2], rows[0:1, ra:rb2],
                                         AF.Ln, bias=eps_col[0:1, :])
                    nc.scalar.activation(rows[0:1, ra:rb2], rows[0:1, ra:rb2],
                                         AF.Exp, scale=-0.5)

                def ph_y():
                    ra, rb2 = R_RS + a0, R_RS + a0 + ln
                    ps_bm = psB.tile([128, 512], F32, tag="po")
                    nc.tensor.matmul(ps_bm[:, 0:ln], ones_blk[0:1, :],
                                     rows[0:1, a0 : a0 + ln])
                    ps_br = psB.tile([128, 512], F32, tag="po")
                    nc.tensor.matmul(ps_br[:, 0:ln], ones_blk[0:1, :],
                                     rows[0:1, ra:rb2])
                    for ct in range(KT):
                        yt = pyt.tile([128, 512], F32, tag="yt")
                        pslice = projT[:, ct * NH + a0 : ct * NH + a0 + ln]
                        nc.vector.tensor_sub(yt[:, 0:ln], pslice, ps_bm[:, 0:ln])
                        nc.vector.tensor_mul(yt[:, 0:ln], yt[:, 0:ln],
                                             ps_br[:, 0:ln])
                        nc.vector.tensor_scalar_mul(yt[:, 0:ln], yt[:, 0:ln],
                                                    outg_sb[:, ct : ct + 1])
                        nc.vector.tensor_add(
                            yt[:, 0:ln], yt[:, 0:ln],
                            x_sb[:, ct * N + a0 : ct * N + a0 + ln])
                        nc.sync.dma_start(
                            y[ct * 128 : (ct + 1) * 128, a0 : a0 + ln],
                            yt[:, 0:ln])

                return [ph_norm, ph_proj, ph_ln2, ph_y]

            phases = []
            for cc, (a0, ln) in enumerate(CHUNKS):
                recs, pocs = [], []
                for hg in range(HEADS // 2):
                    run_hg(hg, a0, ln, recs, pocs)
                    if phases:
                        phases.pop(0)()
                phases = tail_phases(cc, a0, ln, recs, pocs)
            for ph in phases:
                ph()
    _split_multi_waits(nc)
    return nc


def _prep_inputs(x, context, norm_gamma, null_kv, Wq, Wkv, ctx_ln_g, ctx_ln_b,
                 Wctx, bctx, Wout, out_ln_g):
    import ml_dtypes
    bf = ml_dtypes.bfloat16
    f = np.float32
    x = np.asarray(x, f).reshape(4, C, N)
    context = np.asarray(context, f)
    g = np.asarray(norm_gamma, f)
    scale = 1.0 / np.sqrt(DH)
    wq_h = (g[:, None] * np.asarray(Wq, f)) * scale
    negcq_h = -wq_h.sum(0, dtype=np.float64).astype(f)[None, :]
    wkv_h = g[:, None] * np.asarray(Wkv, f)
    ncsk_h = -wkv_h[:, :DH].sum(0, dtype=np.float64).astype(f)[None, :]
    ncsv_h = -wkv_h[:, DH:].sum(0, dtype=np.float64).astype(f)[None, :]
    wctx_h = np.asarray(ctx_ln_g, f)[:, None] * np.asarray(Wctx, f)
    bctx_h = (np.asarray(bctx, f) + np.asarray(ctx_ln_b, f) @ np.asarray(Wctx, f))
    null = np.asarray(null_kv, f)
    wout_b = np.concatenate(
        [np.asarray(Wout, f)[h * DH:(h + 1) * DH, :] for h in range(HEADS)], axis=1)
    outg_h = np.ascontiguousarray(np.asarray(out_ln_g, f).reshape(KT, 128).T)

    shared = {
        "wq": np.ascontiguousarray(wq_h).astype(bf),
        "negcq": negcq_h.astype(bf),
        "wkv": np.ascontiguousarray(wkv_h).astype(bf),
        "ncsk": ncsk_h.astype(bf), "ncsv": ncsv_h.astype(bf),
        "wctx": np.ascontiguousarray(wctx_h),
        "bctxk": np.ascontiguousarray(bctx_h[:DH, None]),
        "bctxv": np.ascontiguousarray(bctx_h[DH:, None]),
        "nullkt": np.ascontiguousarray(null[0][:, None]),
        "nullv": np.ascontiguousarray(null[1][:, None]),
        "wout": np.ascontiguousarray(wout_b).astype(bf),
        "outg": outg_h,
    }
    in_maps = []
    for core in range(8):
        b, half = core // 2, core % 2
        m = dict(shared)
        m["x_own"] = np.ascontiguousarray(x[b][:, half * NH : (half + 1) * NH])
        m["x_oth"] = np.ascontiguousarray(x[b][:, (1 - half) * NH : (2 - half) * NH])
        m["ctxt"] = np.ascontiguousarray(context[b])
        in_maps.append(m)
    return in_maps


_LDW_OPT = [False]


def _patch_ldw_opt():
    import concourse.bass_utils as bu
    if getattr(bu, "_ldwopt_patched", False):
        return
    orig = bu.run_command

    def run2(cmd, **kw):
        if _LDW_OPT[0]:
            cmd = [c.replace("--enable-ldw-opt=false", "--enable-ldw-opt=true")
                   for c in cmd]
        return orig(cmd, **kw)

    bu.run_command = run2
    bu._ldwopt_patched = True


def kernel(**inputs):
    from concourse.bass_utils import run_bass_kernel_spmd
    _patch_ldw_opt()

    if "nc" not in _cached:
        _cached["nc"] = _build_bass()
    nc = _cached["nc"]
    in_maps = _prep_inputs(**inputs)
    kw = {}
    if PROFILE:
        import importlib.util

        if "antenv.axon_hooks" not in sys.modules:
            spec = importlib.util.spec_from_file_location(
                "antenv.axon_hooks", "/opt/trn_rl_repo/antenv/axon_hooks.py")
            m = importlib.util.module_from_spec(spec)
            spec.loader.exec_module(m)
            sys.modules["antenv.axon_hooks"] = m
            import antenv

            antenv.axon_hooks = m
        kw = dict(trace=True, tmpdir=PROFILE_DIR)
    res = run_bass_kernel_spmd(nc, in_maps, list(range(8)), **kw)
    _cached["last"] = res
    out = np.empty((4, C, N), np.float32)
    for core in range(8):
        b, half = core // 2, core % 2
        out[b][:, half * NH : (half + 1) * NH] = res.results[core]["y"]
    return out.reshape(4, C, 48, 48)



# revision 34
# speedup vs baseline: 1.3419x; 1.3419x over previous
"""Trainium2 Bass kernel for nn_Attention_LR_65249143160949 (cross-attention block).

Sharding: 8 cores = 4 batches x 2 token-halves (1152 tokens each). Each core
computes k/v for its whole batch (cheap MQA single head, duplicated within the
pair) and q/attention/output for its own tokens. The host permutes tokens so
each core's own rows come first -> identical SPMD program, no collectives.

On-chip layout: features on partitions, tokens on the free axis. LayerNorm is
folded into the projections (pre-scaled weights + rank-1 -colsum*mu term).
Attention runs in sim^T layout (keys on partitions, query tokens free): kT is
rs-scaled so softmax is a plain exp; the denominator comes free as a ones
column (col 64) of the 128-col-padded v stationary (row 64 of the out psum).

Engine balance (the v1 kernel was ACT+PE serialized at ~460us):
- exp is split ACT (exact, bf16 out) / DVE (Schraudolph bit-trick: i16 =
  round(sim*184.665 + 16250.4) bitcast bf16, ~3% max err on weights).
- all fp32 broadcast/stat matmuls use float32r (1 cyc/row vs 4) or bf16.
- the 24 per-head reciprocals are batched: den rows DMA-gathered to one
  [8, 512] tile, ONE reciprocal per chunk, then per-head selector matmuls
  (K=8 one-hot-row-of-ones, f32r) broadcast 1/den to 64 partitions.
- GPSIMD (cannot touch PSUM) takes SBUF-only elementwise work: LN1 x^2,
  LN2 bf16 casts + squares, the residual add.
- v/wkv stationaries are 128-col padded/fused so FWL weight loads stay fast.
- x arrives bf16 from the host in token chunks (LN1 starts ~3us in); the
  fp32 x needed only for the residual is DMA'd last.

Walrus quirks handled: one sync-wait per TPB instruction (_split_multi_waits),
no custom DVE ops, engine ops must start at partition 0/32/64/96, GPSIMD has
no PSUM access, f32r tiles must be produced by a rounding op (DVE copy).
"""

import sys

import numpy as np

if "/opt/trn_rl_repo" not in sys.path:
    sys.path.insert(0, "/opt/trn_rl_repo")

C = 512          # channels
N = 2304         # tokens per batch (48*48)
NH = 1152        # tokens per core
HEADS = 8
DH = 64
CTXL = 77
CTXD = 768
JT = 19          # j tiles of 128: 18 img + 1 (ctx 0:77 | null 77 | pad)
JP = JT * 128
CHUNKS = [(0, 512), (512, 512), (1024, 128)]  # (start, len) token chunks
NCH = len(CHUNKS)
KT = 4           # C / 128
EPS = 1e-5

FEXP_S = 184.6649186888274   # 128 / ln(2)
FEXP_C = 16250.4             # 127*128 - 5.6 (minimax-tuned, round-to-nearest)

PROFILE = False
PROFILE_DIR = None

_cached = {}


USE_DVE_EXP = True
USE_GPS_TAIL = True
USE_F32R = True
USE_DENT_DMA = True


def _exp_engine(cc, hg, jt):
    """Engine for the softmax exp of (chunk cc, head-pair hg, j-tile jt).
    512-token chunks: every 4th j-tile on DVE (bit-trick exp). Returns
    'act' or 'dve'."""
    if not USE_DVE_EXP:
        return "act"
    return "dve" if (jt % 4) == 3 else "act"


def _exp_engine_128(hg, grp):
    """Engine for the packed 4-j-tile exp groups of the 128-token chunk."""
    if not USE_DVE_EXP:
        return "act"
    return "dve" if grp in (1, 3) else "act"


def _split_multi_waits(nc):
    """Walrus codegen supports one sync-wait per TPB instruction (the EVENTS
    struct has a single wait slot). Tile attaches several. Split the extras
    onto same-engine NoOps inserted just before each instruction."""
    import concourse.mybir as mybir

    n = 0
    for fn in nc.m.functions:
        for bb in fn.blocks:
            insts = bb.instructions
            i = 0
            while i < len(insts):
                ins = insts[i]
                si = getattr(ins, "sync_info", None)
                if si is not None and si.on_wait and len(si.on_wait) > 1:
                    waits = list(si.on_wait)
                    for w in waits[:-1]:
                        n += 1
                        nop = mybir.InstNoOp(name=f"WSPLIT-{n}", engine=ins.engine)
                        nop.sync_info = mybir.SyncInfo(on_wait=[w], on_update=[])
                        insts.insert(i, nop)
                        i += 1
                    ins.sync_info = mybir.SyncInfo(
                        on_wait=[waits[-1]], on_update=si.on_update)
                i += 1
    return n


def _build_bass():
    import concourse.bass as bass
    import concourse.mybir as mybir
    import concourse.tile as tile
    from concourse.masks import make_identity
    from contextlib import ExitStack

    F32 = mybir.dt.float32
    F32R = mybir.dt.float32r if USE_F32R else mybir.dt.float32
    BF = mybir.dt.bfloat16
    I16 = mybir.dt.int16
    AF = mybir.ActivationFunctionType
    ALU = mybir.AluOpType

    nc = bass.Bass()
    xbf = nc.declare_dram_parameter("xbf", [C, N], BF, isOutput=False)
    x_own = nc.declare_dram_parameter("x_own", [C, NH], F32, isOutput=False)
    ctxt = nc.declare_dram_parameter("ctxt", [CTXL, CTXD], F32, isOutput=False)
    wq = nc.declare_dram_parameter("wq", [C, C], BF, isOutput=False)
    negcq = nc.declare_dram_parameter("negcq", [1, C], BF, isOutput=False)
    wkvc = nc.declare_dram_parameter("wkvc", [C, 128], BF, isOutput=False)
    ncskv = nc.declare_dram_parameter("ncskv", [1, 128], BF, isOutput=False)
    wctx = nc.declare_dram_parameter("wctx", [CTXD, 2 * DH], F32, isOutput=False)
    bctxk = nc.declare_dram_parameter("bctxk", [DH, 1], F32, isOutput=False)
    bctxv = nc.declare_dram_parameter("bctxv", [DH, 1], F32, isOutput=False)
    nullkt = nc.declare_dram_parameter("nullkt", [DH, 1], F32, isOutput=False)
    nullv = nc.declare_dram_parameter("nullv", [DH, 1], F32, isOutput=False)
    wout = nc.declare_dram_parameter("wout", [DH, HEADS * C], BF, isOutput=False)
    selin = nc.declare_dram_parameter("selin", [8, 8 * DH], F32, isOutput=False)
    outgr = nc.declare_dram_parameter("outgr", [1, C], F32, isOutput=False)
    y = nc.declare_dram_parameter("y", [C, NH], F32, isOutput=True)

    with tile.TileContext(nc) as tc, ExitStack() as ctx:
        pconst = ctx.enter_context(tc.tile_pool(name="const", bufs=1))
        pbig = ctx.enter_context(tc.tile_pool(name="big", bufs=1))

        ident = pconst.tile([128, 128], F32)
        make_identity(nc, ident[:])
        ident_bf = pconst.tile([128, 128], BF)
        make_identity(nc, ident_bf[:])
        ones_col = pconst.tile([128, 1], F32)
        nc.vector.memset(ones_col[:], 1.0)
        ones_col_bf = pconst.tile([128, 1], BF)
        nc.vector.memset(ones_col_bf[:], 1.0)
        ones_f = pconst.tile([1, 128], F32)
        nc.vector.memset(ones_f[:], 1.0)
        ones_r = pconst.tile([1, 128], F32R)
        nc.vector.tensor_copy(ones_r[:], ones_f[:])
        eps_col = pconst.tile([128, 1], F32)
        nc.vector.memset(eps_col[:], EPS)
        ones_blk = pconst.tile([128, 64], F32)
        nc.vector.memset(ones_blk[:], 1.0)
        sel_f = pconst.tile([8, 8 * DH], F32)
        sel = pconst.tile([8, 8 * DH], F32R)
        outgr_f = pconst.tile([1, C], F32)
        outgr_r = pconst.tile([1, C], F32R)

        x_sb = pbig.tile([128, KT * NH], F32)        # kt-major; OWN half only
        x_bf = pbig.tile([128, KT * N], BF)
        qT = pbig.tile([128, (HEADS // 2) * NH], BF)  # head-pair blocks
        kT2 = pbig.tile([128, JP], BF)               # rs-scaled keys, both halves
        v_sb = pbig.tile([128, JT * 128], BF)        # per j-tile [v 0:64|ones 64|pad]
        projBF = pbig.tile([128, KT * 512], BF)      # bf16 proj (per chunk)
        stats = pbig.tile([128, 40], F32)            # col jt: rs_j (v scaling)
        wout_sb = pbig.tile([64, HEADS * C], BF)
        # per-token stat rows on partition 0: mu 0:N | rs N:2N
        # (LN2 reuses per cc: mu2 at cc*CH, rs2 at N+cc*CH, ex2 at 2N+cc*CH)
        rows = pbig.tile([1, 2 * N + NH], F32)
        rows_bf = pbig.tile([1, N], BF)
        rows_r = pbig.tile([1, N], F32R)             # f32r copy of rs row
        R_RS, R_SC = N, 2 * N

        nc.sync.dma_start(sel_f[:], selin[:, :])
        nc.sync.dma_start(outgr_f[:], outgr[:, :])
        nc.vector.tensor_copy(sel[:], sel_f[:])
        nc.vector.tensor_copy(outgr_r[:], outgr_f[:])
        nc.sync.dma_start(wout_sb[:], wout[:, :])

        with tc.tile_pool(name="load", bufs=1) as pload, \
             tc.tile_pool(name="x2p", bufs=2) as px2, \
             tc.tile_pool(name="pss", bufs=2, space="PSUM") as pss:
            # psum tags: b1 [<=64,384]x2, bS [128,<=512]x4, bT [128,128]x2
            wq_sb = pload.tile([128, KT * C], BF)
            wkv_sb = pload.tile([128, KT * 128], BF)
            wctx_sb = pload.tile([128, CTXD], F32)
            negcq_sb = pload.tile([1, C], BF)
            ncskv_sb = pload.tile([1, 128], BF)
            bctxk_sb = pload.tile([DH, 1], F32)
            bctxv_sb = pload.tile([DH, 1], F32)
            vT = pload.tile([64, N], BF)
            ck_sb = pload.tile([64, CTXL], F32)
            cv_sb = pload.tile([64, CTXL + 1], F32)
            nullk_st = pload.tile([DH, 1], F32)
            nullv_st = pload.tile([DH, 1], F32)
            ctx_sb = pload.tile([CTXL, CTXD], F32)
            ctxnT = pload.tile([128, 6 * CTXL], F32)
            ex2 = pload.tile([1, N], F32)
            kk = pload.tile([128, 512], F32)

            # DMA order = need order: ctx path, x_bf chunks, weights, x f32.
            nc.sync.dma_start(ctx_sb[:], ctxt[:, :])
            nc.sync.dma_start(wctx_sb[:].rearrange("p (k n) -> p k n", k=6),
                              wctx[:].rearrange("(k p) n -> p k n", p=128))
            nc.sync.dma_start(bctxk_sb[:], bctxk[:, :])
            nc.sync.dma_start(bctxv_sb[:], bctxv[:, :])
            nc.sync.dma_start(nullk_st[:], nullkt[:, :])
            nc.sync.dma_start(nullv_st[:], nullv[:, :])
            xbf_v = x_bf[:].rearrange("p (k n) -> p k n", k=KT)
            xbf_d = xbf[:].rearrange("(k p) n -> p k n", p=128)
            for ch in range(6):
                a, b = ch * 384, (ch + 1) * 384
                nc.sync.dma_start(xbf_v[:, :, a:b], xbf_d[:, :, a:b])
            nc.sync.dma_start(wkv_sb[:].rearrange("p (k n) -> p k n", k=KT),
                              wkvc[:].rearrange("(k p) n -> p k n", p=128))
            nc.sync.dma_start(ncskv_sb[:], ncskv[:, :])
            nc.sync.dma_start(wq_sb[:].rearrange("p (k n) -> p k n", k=KT),
                              wq[:].rearrange("(k p) n -> p k n", p=128))
            nc.sync.dma_start(negcq_sb[:], negcq[:, :])
            x_v = x_sb[:].rearrange("p (k n) -> p k n", k=KT)
            nc.sync.dma_start(x_v[:, :, 0:NH],
                              x_own[:].rearrange("(k p) n -> p k n", p=128))

            # ---- context: LN (layout A, bn_stats) + k/v projection ----
            cstat = pload.tile([CTXL, 3, 6], F32)
            for sg in range(3):
                nc.vector.bn_stats(cstat[:, sg, :],
                                   ctx_sb[:, sg * 256 : (sg + 1) * 256])
            cmv = pload.tile([CTXL, 2], F32)
            nc.vector.bn_aggr(cmv[:], cstat[:])
            nc.scalar.activation(cmv[:, 1:2], cmv[:, 1:2], AF.Ln,
                                 bias=eps_col[0:CTXL, :])
            nc.scalar.activation(cmv[:, 1:2], cmv[:, 1:2], AF.Exp, scale=-0.5)
            nc.vector.tensor_scalar(
                out=ctx_sb[:], in0=ctx_sb[:],
                scalar1=cmv[:, 0:1], scalar2=cmv[:, 1:2],
                op0=ALU.subtract, op1=ALU.mult)
            for kt in range(6):
                ps_ct = pss.tile([128, 128], F32, tag="bT")
                nc.tensor.transpose(ps_ct[:, 0:CTXL],
                                    ctx_sb[:, kt * 128 : (kt + 1) * 128],
                                    ident[:CTXL, :CTXL])
                nc.vector.tensor_copy(ctxnT[:, kt * CTXL : (kt + 1) * CTXL],
                                      ps_ct[:, 0:CTXL])
            ps_ck = pss.tile([64, 384], F32, tag="b1")
            ps_cv = pss.tile([64, 384], F32, tag="b1")
            for kt in range(6):
                nc.tensor.matmul(ps_ck[:, 0:CTXL],
                                 wctx_sb[:, kt * 128 : kt * 128 + DH],
                                 ctxnT[:, kt * CTXL : (kt + 1) * CTXL],
                                 start=(kt == 0), stop=(kt == 5))
                nc.tensor.matmul(ps_cv[:, 0:CTXL],
                                 wctx_sb[:, kt * 128 + DH : (kt + 1) * 128],
                                 ctxnT[:, kt * CTXL : (kt + 1) * CTXL],
                                 start=(kt == 0), stop=(kt == 5))
            nc.vector.tensor_scalar_add(ck_sb[:], ps_ck[:, 0:CTXL], bctxk_sb[:])
            nc.vector.tensor_scalar_add(cv_sb[:, 0:CTXL], ps_cv[:, 0:CTXL],
                                        bctxv_sb[:])
            nc.vector.tensor_copy(cv_sb[:, CTXL : CTXL + 1], nullv_st[:])

            # ---- LN1 stats (row form): mu, then rs = exp(-0.5 ln(var+eps)) ----
            for ch in range(6):
                sl = slice(ch * 384, (ch + 1) * 384)
                ps_r1 = pss.tile([64, 384], F32, tag="b1")
                for kt in range(KT):
                    nc.tensor.matmul(
                        ps_r1[0:1, :], ones_col_bf[:],
                        x_bf[:, kt * N + ch * 384 : kt * N + (ch + 1) * 384],
                        start=(kt == 0), stop=(kt == KT - 1))
                nc.scalar.mul(rows[0:1, sl], ps_r1[0:1, :], 1.0 / C)
                nc.vector.tensor_copy(rows_bf[0:1, sl], rows[0:1, sl])
            for ch in range(6):
                x2 = px2.tile([128, KT * 384], BF, tag="x2")
                ps_r2 = pss.tile([64, 384], F32, tag="b1")
                for kt in range(KT):
                    xs = x_bf[:, kt * N + ch * 384 : kt * N + (ch + 1) * 384]
                    nc.vector.tensor_mul(x2[:, kt * 384 : (kt + 1) * 384], xs, xs)
                    nc.tensor.matmul(
                        ps_r2[0:1, :], ones_col_bf[:],
                        x2[:, kt * 384 : (kt + 1) * 384],
                        start=(kt == 0), stop=(kt == KT - 1))
                nc.scalar.mul(ex2[0:1, ch * 384 : (ch + 1) * 384],
                              ps_r2[0:1, :], 1.0 / C)
            for ch in range(6):
                a, b = R_RS + ch * 384, R_RS + (ch + 1) * 384
                mu = rows[0:1, ch * 384 : (ch + 1) * 384]
                nc.vector.tensor_mul(rows[0:1, a:b], mu, mu)
                nc.vector.tensor_sub(rows[0:1, a:b],
                                     ex2[0:1, ch * 384 : (ch + 1) * 384],
                                     rows[0:1, a:b])
                nc.scalar.activation(rows[0:1, a:b], rows[0:1, a:b], AF.Ln,
                                     bias=eps_col[0:1, :])
                nc.scalar.activation(rows[0:1, a:b], rows[0:1, a:b], AF.Exp,
                                     scale=-0.5)
            nc.vector.tensor_copy(rows_r[0:1, :], rows[0:1, R_RS : R_RS + N])
            # rs as per-partition columns (v scaling)
            for jt in range(18):
                ps_c = pss.tile([128, 128], F32, tag="bT")
                nc.tensor.matmul(ps_c[:, 0:1],
                                 rows[0:1, R_RS + jt * 128 : R_RS + (jt + 1) * 128],
                                 ones_col[0:1, :])
                nc.vector.tensor_copy(stats[:, jt : jt + 1], ps_c[:, 0:1])

            # ---- j-tile 18: [ctx 0:77 | null 77 | pad 78:128] ----
            nc.vector.memset(v_sb[:], 0.0)
            nc.vector.memset(kT2[0:64, 18 * 128 : JP], 0.0)
            nc.vector.tensor_copy(kT2[0:64, 18 * 128 : 18 * 128 + CTXL], ck_sb[:])
            nc.vector.tensor_copy(kT2[0:64, 18 * 128 + CTXL : 18 * 128 + CTXL + 1],
                                  nullk_st[:])
            VB = 18 * 128
            ps_cvt = pss.tile([128, 128], F32, tag="bT")
            nc.tensor.transpose(ps_cvt[0 : CTXL + 1, 0:64], cv_sb[:],
                                ident[:64, :64])
            nc.vector.tensor_copy(v_sb[0 : CTXL + 1, VB : VB + DH],
                                  ps_cvt[0 : CTXL + 1, 0:64])
            nc.vector.memset(v_sb[0 : CTXL + 1, VB + DH : VB + DH + 1], 1.0)

            # ---- kv projection (all tokens; LN folded; k into kT2 top half) ----
            KV_CHUNKS = [(0, 512), (512, 512), (1024, 512), (1536, 512), (2048, 256)]
            for kva, kvl in KV_CHUNKS:
                sl = slice(kva, kva + kvl)
                ps_kv = pss.tile([128, 512], F32, tag="bS")
                for kt in range(KT):
                    xs = x_bf[:, kt * N + kva : kt * N + kva + kvl]
                    nc.tensor.matmul(ps_kv[:, 0:kvl],
                                     wkv_sb[:, kt * 128 : (kt + 1) * 128],
                                     xs, start=(kt == 0), stop=False)
                nc.tensor.matmul(ps_kv[:, 0:kvl], ncskv_sb[:], rows_bf[0:1, sl],
                                 start=False, stop=True)
                ps_bc = pss.tile([128, 512], F32, tag="bS")
                nc.tensor.matmul(ps_bc[:, 0:kvl], ones_r[0:1, 0:128],
                                 rows_r[0:1, sl])
                nc.vector.tensor_copy(kk[64:128, 0:kvl], ps_kv[64:128, 0:kvl])
                nc.vector.tensor_mul(kT2[64:128, sl], kk[64:128, 0:kvl],
                                     ps_bc[64:128, 0:kvl])
                nc.vector.tensor_copy(vT[:, sl], ps_kv[0:64, 0:kvl])

            # ---- v tiles: transpose + rs scale + ones col ----
            for jt in range(18):
                ps_vt = pss.tile([128, 128], BF, tag="bT")
                nc.tensor.transpose(ps_vt[:, 0:64], vT[:, jt * 128 : (jt + 1) * 128],
                                    ident_bf[:64, :64])
                vb = jt * 128
                nc.vector.tensor_scalar_mul(v_sb[:, vb : vb + DH], ps_vt[:, 0:64],
                                            stats[:, jt : jt + 1])
                nc.vector.memset(v_sb[:, vb + DH : vb + DH + 1], 1.0)

            # ---- duplicate kT to partitions 0:64 (sbuf->sbuf DMA) ----
            nc.sync.dma_start(kT2[0:64, 0 : 18 * 128], kT2[64:128, 0 : 18 * 128])
            nc.sync.dma_start(kT2[64:128, 18 * 128 : JP],
                              kT2[0:64, 18 * 128 : JP])

            # ---- q projection (head pairs; LN + 1/sqrt(dh) folded) ----
            for a0, ln in CHUNKS:
                sl = slice(a0, a0 + ln)
                ps_rs = pss.tile([128, 512], F32, tag="bS")
                nc.tensor.matmul(ps_rs[:, 0:ln], ones_r[0:1, :],
                                 rows_r[0:1, sl])
                rs_b = px2.tile([128, 512], F32, tag="rsb")
                nc.vector.tensor_copy(rs_b[:, 0:ln], ps_rs[:, 0:ln])
                for hg in range(HEADS // 2):
                    ps_q = pss.tile([128, 512], F32, tag="bS")
                    for kt in range(KT):
                        nc.tensor.matmul(
                            ps_q[:, 0:ln],
                            wq_sb[:, kt * C + hg * 128 : kt * C + (hg + 1) * 128],
                            x_bf[:, kt * N + a0 : kt * N + a0 + ln],
                            start=(kt == 0), stop=False)
                    nc.tensor.matmul(ps_q[:, 0:ln],
                                     negcq_sb[0:1, hg * 128 : (hg + 1) * 128],
                                     rows_bf[0:1, sl], start=False, stop=True)
                    nc.vector.tensor_mul(
                        qT[:, hg * NH + a0 : hg * NH + a0 + ln],
                        ps_q[:, 0:ln], rs_b[:, 0:ln])

        # ========= attention + output + LN2 + residual, per chunk =========
        # Per (chunk, head-pair): sim pair (row-tiled concurrent) -> exp on
        # ACT or DVE (bit-trick) -> attn.v pair lagging one j-tile. The tail
        # (recip batch, norm, out-proj, LN2, y) is deferred one stage so it
        # never head-of-line-blocks the PE queue.
        with tc.tile_pool(name="attb", bufs=3) as patb, \
             tc.tile_pool(name="atti", bufs=3) as pati, \
             tc.tile_pool(name="outp", bufs=9) as pout, \
             tc.tile_pool(name="pocp", bufs=12) as ppoc, \
             tc.tile_pool(name="denp", bufs=2) as pden, \
             tc.tile_pool(name="recp", bufs=2) as prec, \
             tc.tile_pool(name="rowp", bufs=1) as prow, \
             tc.tile_pool(name="p2p", bufs=1) as pp2, \
             tc.tile_pool(name="yp", bufs=2) as pyt, \
             tc.tile_pool(name="psatt", bufs=2, space="PSUM") as psA, \
             tc.tile_pool(name="psacc", bufs=4, space="PSUM") as psB:

            def at_tile(engine):
                if engine == "act":
                    return patb.tile([128, 1024], BF, tag="atb", name="atb")
                return pati.tile([128, 1024], I16, tag="ati", name="ati")

            def do_exp(engine, at, ps_s, lo, hi):
                if engine == "act":
                    nc.scalar.activation(at[:, lo:hi], ps_s[:, lo:hi], AF.Exp)
                else:
                    nc.vector.tensor_scalar(
                        out=at[:, lo:hi], in0=ps_s[:, lo:hi],
                        scalar1=FEXP_S, scalar2=FEXP_C,
                        op0=ALU.mult, op1=ALU.add)

            def at_slice(at_eng, lo, hi):
                at, eng = at_eng
                ap = at[:, lo:hi]
                return ap if eng == "act" else ap.bitcast(BF)

            def run_hg_512(cc, hg, a0, ln, dent, pocs):
                po0 = psB.tile([128, 512], F32, tag="po")
                po1 = psB.tile([128, 512], F32, tag="po")
                po = [po0, po1]
                q0 = qT[0:64, hg * NH + a0 : hg * NH + a0 + ln]
                q1 = qT[64:128, hg * NH + a0 : hg * NH + a0 + ln]
                ats = [None] * JT
                for jt in range(JT):
                    ps_s = psA.tile([128, 1024], F32, tag="sim")
                    nc.tensor.matmul(ps_s[:, 0:ln],
                                     kT2[0:64, jt * 128 : (jt + 1) * 128],
                                     q0, start=True, stop=True)
                    nc.tensor.matmul(ps_s[:, 512 : 512 + ln],
                                     kT2[64:128, jt * 128 : (jt + 1) * 128],
                                     q1, start=True, stop=True)
                    eng = _exp_engine(cc, hg, jt)
                    at = at_tile(eng)
                    do_exp(eng, at, ps_s, 0, 1024)
                    ats[jt] = (at, eng)
                    if jt > 0:
                        j0 = jt - 1
                        vs = v_sb[:, j0 * 128 : (j0 + 1) * 128]
                        nc.tensor.matmul(po[0][:, 0:ln], vs,
                                         at_slice(ats[j0], 0, ln),
                                         start=(j0 == 0), stop=False)
                        nc.tensor.matmul(po[1][:, 0:ln], vs,
                                         at_slice(ats[j0], 512, 512 + ln),
                                         start=(j0 == 0), stop=False)
                        ats[j0] = None
                j0 = JT - 1
                vs = v_sb[:, j0 * 128 : (j0 + 1) * 128]
                nc.tensor.matmul(po[0][:, 0:ln], vs, at_slice(ats[j0], 0, ln),
                                 start=False, stop=True)
                nc.tensor.matmul(po[1][:, 0:ln], vs,
                                 at_slice(ats[j0], 512, 512 + ln),
                                 start=False, stop=True)
                _extract(hg, ln, po, dent, pocs)

            def run_hg_128(cc, hg, a0, ln, dent, pocs):
                # 19 j-tiles packed 4-per-psum-tile; slot g: q0 at col 128g
                # (bank A), q1 at 512+128g (bank B) so the row-tiled
                # concurrent sim pair never co-writes one psum bank.
                po0 = psB.tile([128, 512], F32, tag="po")
                po1 = psB.tile([128, 512], F32, tag="po")
                po = [po0, po1]
                q0 = qT[0:64, hg * NH + a0 : hg * NH + a0 + ln]
                q1 = qT[64:128, hg * NH + a0 : hg * NH + a0 + ln]
                GRP = [(0, 4), (4, 4), (8, 4), (12, 4), (16, 3)]
                ats = [None] * len(GRP)
                for gi, (jt0, ng) in enumerate(GRP):
                    ps_s = psA.tile([128, 1024], F32, tag="sim")
                    for g in range(ng):
                        jt = jt0 + g
                        nc.tensor.matmul(
                            ps_s[:, 128 * g : 128 * g + ln],
                            kT2[0:64, jt * 128 : (jt + 1) * 128],
                            q0, start=True, stop=True)
                        nc.tensor.matmul(
                            ps_s[:, 512 + 128 * g : 512 + 128 * g + ln],
                            kT2[64:128, jt * 128 : (jt + 1) * 128],
                            q1, start=True, stop=True)
                    eng = _exp_engine_128(hg, gi)
                    at = at_tile(eng)
                    if ng == 4:
                        do_exp(eng, at, ps_s, 0, 1024)
                    else:
                        do_exp(eng, at, ps_s, 0, 128 * ng)
                        do_exp(eng, at, ps_s, 512, 512 + 128 * ng)
                    ats[gi] = (at, eng)
                    if gi > 0:
                        _attnv_128(po, ats[gi - 1], GRP[gi - 1], ln,
                                   start=(gi == 1), stop=False)
                        ats[gi - 1] = None
                _attnv_128(po, ats[-1], GRP[-1], ln, start=False, stop=True)
                _extract(hg, ln, po, dent, pocs)

            def _attnv_128(po, at_eng, grp, ln, start, stop):
                jt0, ng = grp
                for g in range(ng):
                    jt = jt0 + g
                    vs = v_sb[:, jt * 128 : (jt + 1) * 128]
                    last = stop and (g == ng - 1)
                    nc.tensor.matmul(po[0][:, 0:ln], vs,
                                     at_slice(at_eng, 128 * g, 128 * g + ln),
                                     start=(start and g == 0), stop=last)
                    nc.tensor.matmul(po[1][:, 0:ln], vs,
                                     at_slice(at_eng, 512 + 128 * g,
                                              512 + 128 * g + ln),
                                     start=(start and g == 0), stop=last)

            def _extract(hg, ln, po, dent, pocs):
                for i in range(2):
                    poc = ppoc.tile([65, 512], BF, tag="poc")
                    nc.vector.tensor_copy(poc[:, 0:ln], po[i][0:65, 0:ln])
                    h = 2 * hg + i
                    if USE_DENT_DMA:
                        nc.sync.dma_start(dent[h : h + 1, 0:ln],
                                          poc[64:65, 0:ln])
                        pocs.append(poc)
                    else:
                        rec = ppoc.tile([65, 512], F32, tag="rech")
                        nc.vector.reciprocal(rec[64:65, 0:ln],
                                             po[i][64:65, 0:ln])
                        pocs.append((poc, rec))

            def tail_phases(cc, a0, ln, dent, pocs):
                ots = []
                rec8r = prec.tile([8, 512], F32R, tag="rec")

                def norm_head(h):
                    ps_rb = psB.tile([128, 512], F32, tag="po")
                    if USE_DENT_DMA:
                        nc.tensor.matmul(ps_rb[0:64, 0:ln],
                                         sel[:, h * DH : (h + 1) * DH],
                                         rec8r[:, 0:ln])
                        poc = pocs[h]
                    else:
                        poc, rec = pocs[h]
                        nc.tensor.matmul(ps_rb[0:64, 0:ln],
                                         ones_blk[64:65, 0:64],
                                         rec[64:65, 0:ln])
                    ot = pout.tile([64, 512], BF, tag="ot")
                    nc.vector.tensor_mul(ot[:, 0:ln], poc[:64, 0:ln],
                                         ps_rb[0:64, 0:ln])
                    ots.append(ot)

                def ph_norm_a():
                    if USE_DENT_DMA:
                        dentf = prec.tile([8, 512], F32, tag="dentf")
                        nc.vector.tensor_copy(dentf[:, 0:ln], dent[:, 0:ln])
                        rec8 = prec.tile([8, 512], F32, tag="rec8")
                        nc.vector.reciprocal(rec8[:, 0:ln], dentf[:, 0:ln])
                        nc.vector.tensor_copy(rec8r[:, 0:ln], rec8[:, 0:ln])
                    for h in range(4):
                        norm_head(h)

                def ph_norm_b():
                    for h in range(4, 8):
                        norm_head(h)

                def ph_proj():
                    for ct in range(KT):
                        ps_p = psB.tile([128, 512], F32, tag="po")
                        for h in range(HEADS):
                            nc.tensor.matmul(
                                ps_p[:, 0:ln],
                                wout_sb[:, h * C + ct * 128 : h * C + (ct + 1) * 128],
                                ots[h][:, 0:ln],
                                start=(h == 0), stop=(h == HEADS - 1))
                        nc.vector.tensor_copy(
                            projBF[:, ct * 512 : ct * 512 + ln],
                            ps_p[:, 0:ln])

                def ph_ln2y():
                    ra, rb2 = R_RS + a0, R_RS + a0 + ln
                    sca, scb = R_SC + a0, R_SC + a0 + ln
                    ps_m2 = psB.tile([128, 512], F32, tag="po")
                    for ct in range(KT):
                        nc.tensor.matmul(ps_m2[0:1, 0:ln], ones_col_bf[:],
                                         projBF[:, ct * 512 : ct * 512 + ln],
                                         start=(ct == 0), stop=(ct == KT - 1))
                    nc.scalar.mul(rows[0:1, a0 : a0 + ln], ps_m2[0:1, 0:ln], 1.0 / C)
                    p2 = pp2.tile([128, KT * 512], BF, tag="p2")
                    ps_q2 = psB.tile([128, 512], F32, tag="po")
                    for ct in range(KT):
                        pslc = projBF[:, ct * 512 : ct * 512 + ln]
                        eng2 = nc.gpsimd if USE_GPS_TAIL else nc.vector
                        eng2.tensor_mul(p2[:, ct * 512 : ct * 512 + ln],
                                        pslc, pslc)
                        nc.tensor.matmul(ps_q2[0:1, 0:ln], ones_col_bf[:],
                                         p2[:, ct * 512 : ct * 512 + ln],
                                         start=(ct == 0), stop=(ct == KT - 1))
                    nc.scalar.mul(rows[0:1, sca:scb], ps_q2[0:1, 0:ln], 1.0 / C)
                    nc.vector.tensor_mul(rows[0:1, ra:rb2], rows[0:1, a0 : a0 + ln],
                                         rows[0:1, a0 : a0 + ln])
                    nc.vector.tensor_sub(rows[0:1, ra:rb2], rows[0:1, sca:scb],
                                         rows[0:1, ra:rb2])
                    nc.scalar.activation(rows[0:1, ra:rb2], rows[0:1, ra:rb2],
                                         AF.Ln, bias=eps_col[0:1, :])
                    nc.scalar.activation(rows[0:1, ra:rb2], rows[0:1, ra:rb2],
                                         AF.Exp, scale=-0.5)
                    # mrs = mu * rs ; f32r rows for the g*rs / g*mu*rs bcasts
                    rs2r = prow.tile([1, 2 * 512], F32R, tag="r2")
                    nc.vector.tensor_mul(rows[0:1, sca:scb], rows[0:1, a0 : a0 + ln],
                                         rows[0:1, ra:rb2])
                    nc.vector.tensor_copy(rs2r[0:1, 0:ln], rows[0:1, ra:rb2])
                    nc.vector.tensor_copy(rs2r[0:1, 512 : 512 + ln],
                                          rows[0:1, sca:scb])
                    ps_gb = psB.tile([128, 512], F32, tag="po")
                    ps_gm = psB.tile([128, 512], F32, tag="po")
                    for ct in range(KT):
                        gsl = outgr_r[0:1, ct * 128 : (ct + 1) * 128]
                        nc.tensor.matmul(ps_gb[:, 0:ln], gsl, rs2r[0:1, 0:ln])
                        nc.tensor.matmul(ps_gm[:, 0:ln], gsl,
                                         rs2r[0:1, 512 : 512 + ln])
                        yt = pyt.tile([128, 512], F32, tag="yt")
                        pslice = projBF[:, ct * 512 : ct * 512 + ln]
                        nc.vector.tensor_mul(yt[:, 0:ln], pslice, ps_gb[:, 0:ln])
                        nc.vector.tensor_sub(yt[:, 0:ln], yt[:, 0:ln],
                                             ps_gm[:, 0:ln])
                        eng3 = nc.gpsimd if USE_GPS_TAIL else nc.vector
                        eng3.tensor_add(
                            yt[:, 0:ln], yt[:, 0:ln],
                            x_sb[:, ct * NH + a0 : ct * NH + a0 + ln])
                        nc.sync.dma_start(
                            y[ct * 128 : (ct + 1) * 128, a0 : a0 + ln],
                            yt[:, 0:ln])

                return [ph_norm_a, ph_norm_b, ph_proj, ph_ln2y]

            phases = []
            for cc, (a0, ln) in enumerate(CHUNKS):
                pocs = []
                dent = pden.tile([8, 512], BF, tag="dent")
                for hg in range(HEADS // 2):
                    if ln == 512:
                        run_hg_512(cc, hg, a0, ln, dent, pocs)
                    else:
                        run_hg_128(cc, hg, a0, ln, dent, pocs)
                    if phases:
                        phases.pop(0)()
                phases = tail_phases(cc, a0, ln, dent, pocs)
            for ph in phases:
                ph()
    _split_multi_waits(nc)
    return nc


def _prep_inputs(x, context, norm_gamma, null_kv, Wq, Wkv, ctx_ln_g, ctx_ln_b,
                 Wctx, bctx, Wout, out_ln_g):
    import ml_dtypes
    bf = ml_dtypes.bfloat16
    f = np.float32
    x = np.asarray(x, f).reshape(4, C, N)
    context = np.asarray(context, f)
    g = np.asarray(norm_gamma, f)
    scale = 1.0 / np.sqrt(DH)
    wq_h = (g[:, None] * np.asarray(Wq, f)) * scale
    negcq_h = -wq_h.sum(0, dtype=np.float64).astype(f)[None, :]
    wkv_h = g[:, None] * np.asarray(Wkv, f)
    # combined stationary: [v | k] so k lands on psum rows 64:128
    wkvc_h = np.concatenate([wkv_h[:, DH:], wkv_h[:, :DH]], axis=1)
    ncs = -wkv_h.sum(0, dtype=np.float64).astype(f)
    ncskv_h = np.concatenate([ncs[DH:], ncs[:DH]])[None, :]
    wctx_h = np.asarray(ctx_ln_g, f)[:, None] * np.asarray(Wctx, f)
    bctx_h = (np.asarray(bctx, f) + np.asarray(ctx_ln_b, f) @ np.asarray(Wctx, f))
    null = np.asarray(null_kv, f)
    wout_b = np.concatenate(
        [np.asarray(Wout, f)[h * DH:(h + 1) * DH, :] for h in range(HEADS)], axis=1)
    sel_h = np.kron(np.eye(8, dtype=f), np.ones((1, DH), f))

    shared = {
        "wq": np.ascontiguousarray(wq_h).astype(bf),
        "negcq": negcq_h.astype(bf),
        "wkvc": np.ascontiguousarray(wkvc_h).astype(bf),
        "ncskv": np.ascontiguousarray(ncskv_h).astype(bf),
        "wctx": np.ascontiguousarray(wctx_h),
        "bctxk": np.ascontiguousarray(bctx_h[:DH, None]),
        "bctxv": np.ascontiguousarray(bctx_h[DH:, None]),
        "nullkt": np.ascontiguousarray(null[0][:, None]),
        "nullv": np.ascontiguousarray(null[1][:, None]),
        "wout": np.ascontiguousarray(wout_b).astype(bf),
        "selin": sel_h,
        "outgr": np.ascontiguousarray(np.asarray(out_ln_g, f)[None, :]),
    }
    in_maps = []
    for core in range(8):
        b, half = core // 2, core % 2
        m = dict(shared)
        xo = x[b][:, half * NH : (half + 1) * NH]
        xt = x[b][:, (1 - half) * NH : (2 - half) * NH]
        m["x_own"] = np.ascontiguousarray(xo)
        m["xbf"] = np.ascontiguousarray(
            np.concatenate([xo, xt], axis=1)).astype(bf)
        m["ctxt"] = np.ascontiguousarray(context[b])
        in_maps.append(m)
    return in_maps


_LDW_OPT = [False]


def _patch_ldw_opt():
    import concourse.bass_utils as bu
    if getattr(bu, "_ldwopt_patched", False):
        return
    orig = bu.run_command

    def run2(cmd, **kw):
        if _LDW_OPT[0]:
            cmd = [c.replace("--enable-ldw-opt=false", "--enable-ldw-opt=true")
                   for c in cmd]
        return orig(cmd, **kw)

    bu.run_command = run2
    bu._ldwopt_patched = True


def kernel(**inputs):
    from concourse.bass_utils import run_bass_kernel_spmd
    _patch_ldw_opt()

    if "nc" not in _cached:
        _cached["nc"] = _build_bass()
    nc = _cached["nc"]
    in_maps = _prep_inputs(**inputs)
    kw = {}
    if PROFILE:
        import importlib.util

        if "antenv.axon_hooks" not in sys.modules:
            spec = importlib.util.spec_from_file_location(
                "antenv.axon_hooks", "/opt/trn_rl_repo/antenv/axon_hooks.py")
            m = importlib.util.module_from_spec(spec)
            spec.loader.exec_module(m)
            sys.modules["antenv.axon_hooks"] = m
            import antenv

            antenv.axon_hooks = m
        kw = dict(trace=True, tmpdir=PROFILE_DIR)
    res = run_bass_kernel_spmd(nc, in_maps, list(range(8)), **kw)
    _cached["last"] = res
    out = np.empty((4, C, N), np.float32)
    for core in range(8):
        b, half = core // 2, core % 2
        out[b][:, half * NH : (half + 1) * NH] = res.results[core]["y"]
    return out.reshape(4, C, 48, 48)


# revision 39
# speedup vs baseline: 1.3972x; 1.0412x over previous
"""Trainium2 Bass kernel for nn_Attention_LR_65249143160949 (cross-attention block).

Sharding: 8 cores = 4 batches x 2 token-halves (1152 tokens each). Each core
computes k/v for its whole batch (cheap MQA single head, duplicated within the
pair) and q/attention/output for its own tokens. The host permutes tokens so
each core's own rows come first -> identical SPMD program, no collectives.

On-chip layout: features on partitions, tokens on the free axis. LayerNorm is
folded into the projections (pre-scaled weights + rank-1 -colsum*mu term).
Attention runs in sim^T layout (keys on partitions, query tokens free): kT is
rs-scaled so softmax is a plain exp; the denominator comes free as a ones
column (col 64) of the 128-col-padded v stationary (row 64 of the out psum).

Engine balance (the v1 kernel was ACT+PE serialized at ~460us):
- exp is split ACT (exact, bf16 out) / DVE (Schraudolph bit-trick: i16 =
  round(sim*184.665 + 16250.4) bitcast bf16, ~3% max err on weights).
- all fp32 broadcast/stat matmuls use float32r (1 cyc/row vs 4) or bf16.
- the 24 per-head reciprocals are batched: den rows DMA-gathered to one
  [8, 512] tile, ONE reciprocal per chunk, then per-head selector matmuls
  (K=8 one-hot-row-of-ones, f32r) broadcast 1/den to 64 partitions.
- GPSIMD (cannot touch PSUM) takes SBUF-only elementwise work: LN1 x^2,
  LN2 bf16 casts + squares, the residual add.
- v/wkv stationaries are 128-col padded/fused so FWL weight loads stay fast.
- x arrives bf16 from the host in token chunks (LN1 starts ~3us in); the
  fp32 x needed only for the residual is DMA'd last.

Walrus quirks handled: one sync-wait per TPB instruction (_split_multi_waits),
no custom DVE ops, engine ops must start at partition 0/32/64/96, GPSIMD has
no PSUM access, f32r tiles must be produced by a rounding op (DVE copy).
"""

import sys

import numpy as np

if "/opt/trn_rl_repo" not in sys.path:
    sys.path.insert(0, "/opt/trn_rl_repo")

C = 512          # channels
N = 2304         # tokens per batch (48*48)
NH = 1152        # tokens per core
HEADS = 8
DH = 64
CTXL = 77
CTXD = 768
JT = 19          # j tiles of 128: 18 img + 1 (ctx 0:77 | null 77 | pad)
JP = JT * 128
CHUNKS = [(0, 512), (512, 512), (1024, 128)]  # (start, len) token chunks
NCH = len(CHUNKS)
KT = 4           # C / 128
EPS = 1e-5

FEXP_S = 184.6649186888274   # 128 / ln(2)
FEXP_C = 16250.4             # 127*128 - 5.6 (minimax-tuned, round-to-nearest)

PROFILE = False
PROFILE_DIR = None

_cached = {}


USE_DVE_EXP = True
USE_GPS_TAIL = True
USE_F32R = True
USE_DENT_DMA = True
ALAG = 2         # attn.v lags exp by this many j-tiles (512-token chunks)


def _exp_engine(cc, hg, jt):
    """Engine for the softmax exp of (chunk cc, head-pair hg, j-tile jt).
    512-token chunks: every 4th j-tile on DVE (bit-trick exp). Returns
    'act' or 'dve'."""
    if not USE_DVE_EXP:
        return "act"
    return "dve" if (jt % 4) == 3 else "act"


def _exp_engine_128(hg, grp):
    """Engine for the packed 4-j-tile exp groups of the 128-token chunk."""
    if not USE_DVE_EXP:
        return "act"
    return "dve" if grp in (1, 3) else "act"


def _split_multi_waits(nc):
    """Walrus codegen supports one sync-wait per TPB instruction (the EVENTS
    struct has a single wait slot). Tile attaches several. Split the extras
    onto same-engine NoOps inserted just before each instruction."""
    import concourse.mybir as mybir

    n = 0
    for fn in nc.m.functions:
        for bb in fn.blocks:
            insts = bb.instructions
            i = 0
            while i < len(insts):
                ins = insts[i]
                si = getattr(ins, "sync_info", None)
                if si is not None and si.on_wait and len(si.on_wait) > 1:
                    waits = list(si.on_wait)
                    for w in waits[:-1]:
                        n += 1
                        nop = mybir.InstNoOp(name=f"WSPLIT-{n}", engine=ins.engine)
                        nop.sync_info = mybir.SyncInfo(on_wait=[w], on_update=[])
                        insts.insert(i, nop)
                        i += 1
                    ins.sync_info = mybir.SyncInfo(
                        on_wait=[waits[-1]], on_update=si.on_update)
                i += 1
    return n


def _build_bass():
    import concourse.bass as bass
    import concourse.mybir as mybir
    import concourse.tile as tile
    from concourse.masks import make_identity
    from contextlib import ExitStack

    F32 = mybir.dt.float32
    F32R = mybir.dt.float32r if USE_F32R else mybir.dt.float32
    BF = mybir.dt.bfloat16
    I16 = mybir.dt.int16
    AF = mybir.ActivationFunctionType
    ALU = mybir.AluOpType

    nc = bass.Bass()
    xbf = nc.declare_dram_parameter("xbf", [C, N], BF, isOutput=False)
    x_own = nc.declare_dram_parameter("x_own", [C, NH], F32, isOutput=False)
    ctxt = nc.declare_dram_parameter("ctxt", [CTXL, CTXD], F32, isOutput=False)
    wq = nc.declare_dram_parameter("wq", [C, C], BF, isOutput=False)
    negcq = nc.declare_dram_parameter("negcq", [1, C], BF, isOutput=False)
    wkvc = nc.declare_dram_parameter("wkvc", [C, 128], BF, isOutput=False)
    ncskv = nc.declare_dram_parameter("ncskv", [1, 128], BF, isOutput=False)
    wctx = nc.declare_dram_parameter("wctx", [CTXD, 2 * DH], F32, isOutput=False)
    bctxk = nc.declare_dram_parameter("bctxk", [DH, 1], F32, isOutput=False)
    bctxv = nc.declare_dram_parameter("bctxv", [DH, 1], F32, isOutput=False)
    nullkt = nc.declare_dram_parameter("nullkt", [DH, 1], F32, isOutput=False)
    nullv = nc.declare_dram_parameter("nullv", [DH, 1], F32, isOutput=False)
    wout = nc.declare_dram_parameter("wout", [DH, HEADS * C], BF, isOutput=False)
    selin = nc.declare_dram_parameter("selin", [8, 8 * DH], F32, isOutput=False)
    outgr = nc.declare_dram_parameter("outgr", [1, C], F32, isOutput=False)
    y = nc.declare_dram_parameter("y", [C, NH], F32, isOutput=True)

    with tile.TileContext(nc) as tc, ExitStack() as ctx:
        pconst = ctx.enter_context(tc.tile_pool(name="const", bufs=1))
        pbig = ctx.enter_context(tc.tile_pool(name="big", bufs=1))

        ident = pconst.tile([128, 128], F32)
        make_identity(nc, ident[:])
        ident_bf = pconst.tile([128, 128], BF)
        make_identity(nc, ident_bf[:])
        ones_col = pconst.tile([128, 1], F32)
        nc.vector.memset(ones_col[:], 1.0)
        ones_col_bf = pconst.tile([128, 1], BF)
        nc.vector.memset(ones_col_bf[:], 1.0)
        ones_f = pconst.tile([1, 128], F32)
        nc.vector.memset(ones_f[:], 1.0)
        ones_r = pconst.tile([1, 128], F32R)
        nc.vector.tensor_copy(ones_r[:], ones_f[:])
        eps_col = pconst.tile([128, 1], F32)
        nc.vector.memset(eps_col[:], EPS)
        ones_blk = pconst.tile([128, 64], F32)
        nc.vector.memset(ones_blk[:], 1.0)
        sel_f = pconst.tile([8, 8 * DH], F32)
        sel = pconst.tile([8, 8 * DH], F32R)
        outgr_f = pconst.tile([1, C], F32)
        outgr_r = pconst.tile([1, C], F32R)

        x_sb = pbig.tile([128, KT * NH], F32)        # kt-major; OWN half only
        x_bf = pbig.tile([128, KT * N], BF)
        qT = pbig.tile([128, (HEADS // 2) * NH], BF)  # head-pair blocks
        kT2 = pbig.tile([128, JP], BF)               # rs-scaled keys, both halves
        v_sb = pbig.tile([128, JT * 128], BF)        # per j-tile [v 0:64|ones 64|pad]
        projBF = pbig.tile([128, KT * 512], BF)      # bf16 proj (per chunk)
        stats = pbig.tile([128, 40], F32)            # col jt: rs_j (v scaling)
        wout_sb = pbig.tile([64, HEADS * C], BF)
        # per-token stat rows on partition 0: mu 0:N | rs N:2N
        # (LN2 reuses per cc: mu2 at cc*CH, rs2 at N+cc*CH, ex2 at 2N+cc*CH)
        rows = pbig.tile([1, 2 * N + NH], F32)
        rows_bf = pbig.tile([1, N], BF)
        rows_r = pbig.tile([1, N], F32R)             # f32r copy of rs row
        R_RS, R_SC = N, 2 * N

        nc.sync.dma_start(sel_f[:], selin[:, :])
        nc.sync.dma_start(outgr_f[:], outgr[:, :])
        nc.vector.tensor_copy(sel[:], sel_f[:])
        nc.vector.tensor_copy(outgr_r[:], outgr_f[:])
        nc.sync.dma_start(wout_sb[:], wout[:, :])

        with tc.tile_pool(name="load", bufs=1) as pload, \
             tc.tile_pool(name="x2p", bufs=2) as px2, \
             tc.tile_pool(name="pss", bufs=2, space="PSUM") as pss:
            # psum tags: b1 [<=64,384]x2, bS [128,<=512]x4, bT [128,128]x2
            wq_sb = pload.tile([128, KT * C], BF)
            wkv_sb = pload.tile([128, KT * 128], BF)
            wctx_sb = pload.tile([128, CTXD], F32)
            negcq_sb = pload.tile([1, C], BF)
            ncskv_sb = pload.tile([1, 128], BF)
            bctxk_sb = pload.tile([DH, 1], F32)
            bctxv_sb = pload.tile([DH, 1], F32)
            vT = pload.tile([64, N], BF)
            ck_sb = pload.tile([64, CTXL], F32)
            cv_sb = pload.tile([64, CTXL + 1], F32)
            nullk_st = pload.tile([DH, 1], F32)
            nullv_st = pload.tile([DH, 1], F32)
            ctx_sb = pload.tile([CTXL, CTXD], F32)
            ctxnT = pload.tile([128, 6 * CTXL], F32)
            ex2 = pload.tile([1, N], F32)
            kk = pload.tile([128, 512], F32)

            # DMA order = need order: ctx path, x_bf chunks, weights, x f32.
            nc.sync.dma_start(ctx_sb[:], ctxt[:, :])
            nc.sync.dma_start(wctx_sb[:].rearrange("p (k n) -> p k n", k=6),
                              wctx[:].rearrange("(k p) n -> p k n", p=128))
            nc.sync.dma_start(bctxk_sb[:], bctxk[:, :])
            nc.sync.dma_start(bctxv_sb[:], bctxv[:, :])
            nc.sync.dma_start(nullk_st[:], nullkt[:, :])
            nc.sync.dma_start(nullv_st[:], nullv[:, :])
            xbf_v = x_bf[:].rearrange("p (k n) -> p k n", k=KT)
            xbf_d = xbf[:].rearrange("(k p) n -> p k n", p=128)
            for ch in range(6):
                a, b = ch * 384, (ch + 1) * 384
                nc.sync.dma_start(xbf_v[:, :, a:b], xbf_d[:, :, a:b])
            nc.sync.dma_start(wkv_sb[:].rearrange("p (k n) -> p k n", k=KT),
                              wkvc[:].rearrange("(k p) n -> p k n", p=128))
            nc.sync.dma_start(ncskv_sb[:], ncskv[:, :])
            nc.sync.dma_start(wq_sb[:].rearrange("p (k n) -> p k n", k=KT),
                              wq[:].rearrange("(k p) n -> p k n", p=128))
            nc.sync.dma_start(negcq_sb[:], negcq[:, :])
            x_v = x_sb[:].rearrange("p (k n) -> p k n", k=KT)
            nc.sync.dma_start(x_v[:, :, 0:NH],
                              x_own[:].rearrange("(k p) n -> p k n", p=128))

            # ---- LN1 stats (row form): mu, then rs = exp(-0.5 ln(var+eps)) ----
            for ch in range(6):
                sl = slice(ch * 384, (ch + 1) * 384)
                ps_r1 = pss.tile([64, 384], F32, tag="b1")
                for kt in range(KT):
                    nc.tensor.matmul(
                        ps_r1[0:1, :], ones_col_bf[:],
                        x_bf[:, kt * N + ch * 384 : kt * N + (ch + 1) * 384],
                        start=(kt == 0), stop=(kt == KT - 1))
                nc.scalar.mul(rows[0:1, sl], ps_r1[0:1, :], 1.0 / C)
                nc.vector.tensor_copy(rows_bf[0:1, sl], rows[0:1, sl])
            for ch in range(6):
                x2 = px2.tile([128, KT * 384], BF, tag="x2")
                ps_r2 = pss.tile([64, 384], F32, tag="b1")
                for kt in range(KT):
                    xs = x_bf[:, kt * N + ch * 384 : kt * N + (ch + 1) * 384]
                    nc.vector.tensor_mul(x2[:, kt * 384 : (kt + 1) * 384], xs, xs)
                    nc.tensor.matmul(
                        ps_r2[0:1, :], ones_col_bf[:],
                        x2[:, kt * 384 : (kt + 1) * 384],
                        start=(kt == 0), stop=(kt == KT - 1))
                nc.scalar.mul(ex2[0:1, ch * 384 : (ch + 1) * 384],
                              ps_r2[0:1, :], 1.0 / C)
            for ch in range(6):
                a, b = R_RS + ch * 384, R_RS + (ch + 1) * 384
                mu = rows[0:1, ch * 384 : (ch + 1) * 384]
                nc.vector.tensor_mul(rows[0:1, a:b], mu, mu)
                nc.vector.tensor_sub(rows[0:1, a:b],
                                     ex2[0:1, ch * 384 : (ch + 1) * 384],
                                     rows[0:1, a:b])
                nc.scalar.activation(rows[0:1, a:b], rows[0:1, a:b], AF.Ln,
                                     bias=eps_col[0:1, :])
                nc.scalar.activation(rows[0:1, a:b], rows[0:1, a:b], AF.Exp,
                                     scale=-0.5)
            nc.vector.tensor_copy(rows_r[0:1, :], rows[0:1, R_RS : R_RS + N])
            # ---- context: LN (layout A, bn_stats) + k/v projection ----
            cstat = pload.tile([CTXL, 3, 6], F32)
            for sg in range(3):
                nc.vector.bn_stats(cstat[:, sg, :],
                                   ctx_sb[:, sg * 256 : (sg + 1) * 256])
            cmv = pload.tile([CTXL, 2], F32)
            nc.vector.bn_aggr(cmv[:], cstat[:])
            nc.scalar.activation(cmv[:, 1:2], cmv[:, 1:2], AF.Ln,
                                 bias=eps_col[0:CTXL, :])
            nc.scalar.activation(cmv[:, 1:2], cmv[:, 1:2], AF.Exp, scale=-0.5)
            nc.vector.tensor_scalar(
                out=ctx_sb[:], in0=ctx_sb[:],
                scalar1=cmv[:, 0:1], scalar2=cmv[:, 1:2],
                op0=ALU.subtract, op1=ALU.mult)
            for kt in range(6):
                ps_ct = pss.tile([128, 128], F32, tag="bT")
                nc.tensor.transpose(ps_ct[:, 0:CTXL],
                                    ctx_sb[:, kt * 128 : (kt + 1) * 128],
                                    ident[:CTXL, :CTXL])
                nc.vector.tensor_copy(ctxnT[:, kt * CTXL : (kt + 1) * CTXL],
                                      ps_ct[:, 0:CTXL])
            ps_ck = pss.tile([64, 384], F32, tag="b1")
            ps_cv = pss.tile([64, 384], F32, tag="b1")
            for kt in range(6):
                nc.tensor.matmul(ps_ck[:, 0:CTXL],
                                 wctx_sb[:, kt * 128 : kt * 128 + DH],
                                 ctxnT[:, kt * CTXL : (kt + 1) * CTXL],
                                 start=(kt == 0), stop=(kt == 5))
                nc.tensor.matmul(ps_cv[:, 0:CTXL],
                                 wctx_sb[:, kt * 128 + DH : (kt + 1) * 128],
                                 ctxnT[:, kt * CTXL : (kt + 1) * CTXL],
                                 start=(kt == 0), stop=(kt == 5))
            nc.vector.tensor_scalar_add(ck_sb[:], ps_ck[:, 0:CTXL], bctxk_sb[:])
            nc.vector.tensor_scalar_add(cv_sb[:, 0:CTXL], ps_cv[:, 0:CTXL],
                                        bctxv_sb[:])
            nc.vector.tensor_copy(cv_sb[:, CTXL : CTXL + 1], nullv_st[:])

            # rs as per-partition columns (v scaling)
            for jt in range(18):
                ps_c = pss.tile([128, 128], F32, tag="bT")
                nc.tensor.matmul(ps_c[:, 0:1],
                                 rows[0:1, R_RS + jt * 128 : R_RS + (jt + 1) * 128],
                                 ones_col[0:1, :])
                nc.vector.tensor_copy(stats[:, jt : jt + 1], ps_c[:, 0:1])

            # ---- j-tile 18: [ctx 0:77 | null 77 | pad 78:128] ----
            nc.gpsimd.memset(v_sb[:], 0.0)
            nc.gpsimd.memset(kT2[0:64, 18 * 128 : JP], 0.0)
            nc.vector.tensor_copy(kT2[0:64, 18 * 128 : 18 * 128 + CTXL], ck_sb[:])
            nc.vector.tensor_copy(kT2[0:64, 18 * 128 + CTXL : 18 * 128 + CTXL + 1],
                                  nullk_st[:])
            VB = 18 * 128
            ps_cvt = pss.tile([128, 128], F32, tag="bT")
            nc.tensor.transpose(ps_cvt[0 : CTXL + 1, 0:64], cv_sb[:],
                                ident[:64, :64])
            nc.vector.tensor_copy(v_sb[0 : CTXL + 1, VB : VB + DH],
                                  ps_cvt[0 : CTXL + 1, 0:64])
            nc.vector.memset(v_sb[0 : CTXL + 1, VB + DH : VB + DH + 1], 1.0)

            # ---- kv projection (all tokens; LN folded; k into kT2 top half) ----
            KV_CHUNKS = [(0, 512), (512, 512), (1024, 512), (1536, 512), (2048, 256)]
            for kva, kvl in KV_CHUNKS:
                sl = slice(kva, kva + kvl)
                ps_kv = pss.tile([128, 512], F32, tag="bS")
                for kt in range(KT):
                    xs = x_bf[:, kt * N + kva : kt * N + kva + kvl]
                    nc.tensor.matmul(ps_kv[:, 0:kvl],
                                     wkv_sb[:, kt * 128 : (kt + 1) * 128],
                                     xs, start=(kt == 0), stop=False)
                nc.tensor.matmul(ps_kv[:, 0:kvl], ncskv_sb[:], rows_bf[0:1, sl],
                                 start=False, stop=True)
                ps_bc = pss.tile([128, 512], F32, tag="bS")
                nc.tensor.matmul(ps_bc[:, 0:kvl], ones_r[0:1, 0:128],
                                 rows_r[0:1, sl])
                nc.vector.tensor_copy(kk[64:128, 0:kvl], ps_kv[64:128, 0:kvl])
                nc.vector.tensor_mul(kT2[64:128, sl], kk[64:128, 0:kvl],
                                     ps_bc[64:128, 0:kvl])
                nc.vector.tensor_copy(vT[:, sl], ps_kv[0:64, 0:kvl])

            # ---- v tiles: transpose + rs scale + ones col ----
            for jt in range(18):
                ps_vt = pss.tile([128, 128], BF, tag="bT")
                nc.tensor.transpose(ps_vt[:, 0:64], vT[:, jt * 128 : (jt + 1) * 128],
                                    ident_bf[:64, :64])
                vb = jt * 128
                nc.vector.tensor_scalar_mul(v_sb[:, vb : vb + DH], ps_vt[:, 0:64],
                                            stats[:, jt : jt + 1])
                nc.vector.memset(v_sb[:, vb + DH : vb + DH + 1], 1.0)

            # ---- duplicate kT to partitions 0:64 (sbuf->sbuf DMA) ----
            nc.sync.dma_start(kT2[0:64, 0 : 18 * 128], kT2[64:128, 0 : 18 * 128])
            nc.sync.dma_start(kT2[64:128, 18 * 128 : JP],
                              kT2[0:64, 18 * 128 : JP])

            # ---- q projection (head pairs; LN + 1/sqrt(dh) folded) ----
            for a0, ln in CHUNKS:
                sl = slice(a0, a0 + ln)
                ps_rs = pss.tile([128, 512], F32, tag="bS")
                nc.tensor.matmul(ps_rs[:, 0:ln], ones_r[0:1, :],
                                 rows_r[0:1, sl])
                rs_b = px2.tile([128, 512], F32, tag="rsb")
                nc.vector.tensor_copy(rs_b[:, 0:ln], ps_rs[:, 0:ln])
                for hg in range(HEADS // 2):
                    ps_q = pss.tile([128, 512], F32, tag="bS")
                    for kt in range(KT):
                        nc.tensor.matmul(
                            ps_q[:, 0:ln],
                            wq_sb[:, kt * C + hg * 128 : kt * C + (hg + 1) * 128],
                            x_bf[:, kt * N + a0 : kt * N + a0 + ln],
                            start=(kt == 0), stop=False)
                    nc.tensor.matmul(ps_q[:, 0:ln],
                                     negcq_sb[0:1, hg * 128 : (hg + 1) * 128],
                                     rows_bf[0:1, sl], start=False, stop=True)
                    nc.vector.tensor_mul(
                        qT[:, hg * NH + a0 : hg * NH + a0 + ln],
                        ps_q[:, 0:ln], rs_b[:, 0:ln])

        # ========= attention + output + LN2 + residual, per chunk =========
        # Per (chunk, head-pair): sim pair (row-tiled concurrent) -> exp on
        # ACT or DVE (bit-trick) -> attn.v pair lagging one j-tile. The tail
        # (recip batch, norm, out-proj, LN2, y) is deferred one stage so it
        # never head-of-line-blocks the PE queue.
        with tc.tile_pool(name="attb", bufs=4) as patb, \
             tc.tile_pool(name="atti", bufs=4) as pati, \
             tc.tile_pool(name="outp", bufs=9) as pout, \
             tc.tile_pool(name="pocp", bufs=12) as ppoc, \
             tc.tile_pool(name="denp", bufs=2) as pden, \
             tc.tile_pool(name="recp", bufs=2) as prec, \
             tc.tile_pool(name="rowp", bufs=1) as prow, \
             tc.tile_pool(name="p2p", bufs=1) as pp2, \
             tc.tile_pool(name="yp", bufs=2) as pyt, \
             tc.tile_pool(name="psatt", bufs=2, space="PSUM") as psA, \
             tc.tile_pool(name="psacc", bufs=4, space="PSUM") as psB:

            def at_tile(engine):
                if engine == "act":
                    return patb.tile([128, 1024], BF, tag="atb", name="atb")
                return pati.tile([128, 1024], I16, tag="ati", name="ati")

            def do_exp(engine, at, ps_s, lo, hi):
                if engine == "act":
                    nc.scalar.activation(at[:, lo:hi], ps_s[:, lo:hi], AF.Exp)
                else:
                    nc.vector.tensor_scalar(
                        out=at[:, lo:hi], in0=ps_s[:, lo:hi],
                        scalar1=FEXP_S, scalar2=FEXP_C,
                        op0=ALU.mult, op1=ALU.add)

            def at_slice(at_eng, lo, hi):
                at, eng = at_eng
                ap = at[:, lo:hi]
                return ap if eng == "act" else ap.bitcast(BF)

            def run_hg_512(cc, hg, a0, ln, dent, pocs):
                po0 = psB.tile([128, 512], F32, tag="po")
                po1 = psB.tile([128, 512], F32, tag="po")
                po = [po0, po1]
                q0 = qT[0:64, hg * NH + a0 : hg * NH + a0 + ln]
                q1 = qT[64:128, hg * NH + a0 : hg * NH + a0 + ln]
                ats = [None] * JT
                for jt in range(JT):
                    ps_s = psA.tile([128, 1024], F32, tag="sim")
                    nc.tensor.matmul(ps_s[:, 0:ln],
                                     kT2[0:64, jt * 128 : (jt + 1) * 128],
                                     q0, start=True, stop=True)
                    nc.tensor.matmul(ps_s[:, 512 : 512 + ln],
                                     kT2[64:128, jt * 128 : (jt + 1) * 128],
                                     q1, start=True, stop=True)
                    eng = _exp_engine(cc, hg, jt)
                    at = at_tile(eng)
                    do_exp(eng, at, ps_s, 0, 1024)
                    ats[jt] = (at, eng)
                    if jt >= ALAG:
                        j0 = jt - ALAG
                        vs = v_sb[:, j0 * 128 : (j0 + 1) * 128]
                        nc.tensor.matmul(po[0][:, 0:ln], vs,
                                         at_slice(ats[j0], 0, ln),
                                         start=(j0 == 0), stop=False)
                        nc.tensor.matmul(po[1][:, 0:ln], vs,
                                         at_slice(ats[j0], 512, 512 + ln),
                                         start=(j0 == 0), stop=False)
                        ats[j0] = None
                for j0 in range(JT - ALAG, JT):
                    vs = v_sb[:, j0 * 128 : (j0 + 1) * 128]
                    nc.tensor.matmul(po[0][:, 0:ln], vs, at_slice(ats[j0], 0, ln),
                                     start=(j0 == 0), stop=(j0 == JT - 1))
                    nc.tensor.matmul(po[1][:, 0:ln], vs,
                                     at_slice(ats[j0], 512, 512 + ln),
                                     start=(j0 == 0), stop=(j0 == JT - 1))
                _extract(hg, ln, po, dent, pocs)

            def run_hg_128(cc, hg, a0, ln, dent, pocs):
                # 19 j-tiles packed 4-per-psum-tile; slot g: q0 at col 128g
                # (bank A), q1 at 512+128g (bank B) so the row-tiled
                # concurrent sim pair never co-writes one psum bank.
                po0 = psB.tile([128, 512], F32, tag="po")
                po1 = psB.tile([128, 512], F32, tag="po")
                po = [po0, po1]
                q0 = qT[0:64, hg * NH + a0 : hg * NH + a0 + ln]
                q1 = qT[64:128, hg * NH + a0 : hg * NH + a0 + ln]
                GRP = [(0, 4), (4, 4), (8, 4), (12, 4), (16, 3)]
                ats = [None] * len(GRP)
                for gi, (jt0, ng) in enumerate(GRP):
                    ps_s = psA.tile([128, 1024], F32, tag="sim")
                    for g in range(ng):
                        jt = jt0 + g
                        nc.tensor.matmul(
                            ps_s[:, 128 * g : 128 * g + ln],
                            kT2[0:64, jt * 128 : (jt + 1) * 128],
                            q0, start=True, stop=True)
                        nc.tensor.matmul(
                            ps_s[:, 512 + 128 * g : 512 + 128 * g + ln],
                            kT2[64:128, jt * 128 : (jt + 1) * 128],
                            q1, start=True, stop=True)
                    eng = _exp_engine_128(hg, gi)
                    at = at_tile(eng)
                    if ng == 4:
                        do_exp(eng, at, ps_s, 0, 1024)
                    else:
                        do_exp(eng, at, ps_s, 0, 128 * ng)
                        do_exp(eng, at, ps_s, 512, 512 + 128 * ng)
                    ats[gi] = (at, eng)
                    if gi > 0:
                        _attnv_128(po, ats[gi - 1], GRP[gi - 1], ln,
                                   start=(gi == 1), stop=False)
                        ats[gi - 1] = None
                _attnv_128(po, ats[-1], GRP[-1], ln, start=False, stop=True)
                _extract(hg, ln, po, dent, pocs)

            def _attnv_128(po, at_eng, grp, ln, start, stop):
                jt0, ng = grp
                for g in range(ng):
                    jt = jt0 + g
                    vs = v_sb[:, jt * 128 : (jt + 1) * 128]
                    last = stop and (g == ng - 1)
                    nc.tensor.matmul(po[0][:, 0:ln], vs,
                                     at_slice(at_eng, 128 * g, 128 * g + ln),
                                     start=(start and g == 0), stop=last)
                    nc.tensor.matmul(po[1][:, 0:ln], vs,
                                     at_slice(at_eng, 512 + 128 * g,
                                              512 + 128 * g + ln),
                                     start=(start and g == 0), stop=last)

            def _extract(hg, ln, po, dent, pocs):
                for i in range(2):
                    poc = ppoc.tile([65, 512], BF, tag="poc")
                    nc.vector.tensor_copy(poc[:, 0:ln], po[i][0:65, 0:ln])
                    h = 2 * hg + i
                    if USE_DENT_DMA:
                        nc.sync.dma_start(dent[h : h + 1, 0:ln],
                                          poc[64:65, 0:ln])
                        pocs.append(poc)
                    else:
                        rec = ppoc.tile([65, 512], F32, tag="rech")
                        nc.vector.reciprocal(rec[64:65, 0:ln],
                                             po[i][64:65, 0:ln])
                        pocs.append((poc, rec))

            def tail_phases(cc, a0, ln, dent, pocs):
                ots = []
                rec8r = prec.tile([8, 512], F32R, tag="rec")

                def norm_head(h):
                    ps_rb = psB.tile([128, 512], F32, tag="po")
                    if USE_DENT_DMA:
                        nc.tensor.matmul(ps_rb[0:64, 0:ln],
                                         sel[:, h * DH : (h + 1) * DH],
                                         rec8r[:, 0:ln])
                        poc = pocs[h]
                    else:
                        poc, rec = pocs[h]
                        nc.tensor.matmul(ps_rb[0:64, 0:ln],
                                         ones_blk[64:65, 0:64],
                                         rec[64:65, 0:ln])
                    ot = pout.tile([64, 512], BF, tag="ot")
                    nc.vector.tensor_mul(ot[:, 0:ln], poc[:64, 0:ln],
                                         ps_rb[0:64, 0:ln])
                    ots.append(ot)

                def ph_norm_a():
                    if USE_DENT_DMA:
                        dentf = prec.tile([8, 512], F32, tag="dentf")
                        nc.vector.tensor_copy(dentf[:, 0:ln], dent[:, 0:ln])
                        rec8 = prec.tile([8, 512], F32, tag="rec8")
                        nc.vector.reciprocal(rec8[:, 0:ln], dentf[:, 0:ln])
                        nc.vector.tensor_copy(rec8r[:, 0:ln], rec8[:, 0:ln])
                    for h in range(4):
                        norm_head(h)

                def ph_norm_b():
                    for h in range(4, 8):
                        norm_head(h)

                def ph_proj():
                    for ct in range(KT):
                        ps_p = psB.tile([128, 512], F32, tag="po")
                        for h in range(HEADS):
                            nc.tensor.matmul(
                                ps_p[:, 0:ln],
                                wout_sb[:, h * C + ct * 128 : h * C + (ct + 1) * 128],
                                ots[h][:, 0:ln],
                                start=(h == 0), stop=(h == HEADS - 1))
                        nc.vector.tensor_copy(
                            projBF[:, ct * 512 : ct * 512 + ln],
                            ps_p[:, 0:ln])

                def ph_ln2y():
                    ra, rb2 = R_RS + a0, R_RS + a0 + ln
                    sca, scb = R_SC + a0, R_SC + a0 + ln
                    ps_m2 = psB.tile([128, 512], F32, tag="po")
                    for ct in range(KT):
                        nc.tensor.matmul(ps_m2[0:1, 0:ln], ones_col_bf[:],
                                         projBF[:, ct * 512 : ct * 512 + ln],
                                         start=(ct == 0), stop=(ct == KT - 1))
                    nc.scalar.mul(rows[0:1, a0 : a0 + ln], ps_m2[0:1, 0:ln], 1.0 / C)
                    p2 = pp2.tile([128, KT * 512], BF, tag="p2")
                    ps_q2 = psB.tile([128, 512], F32, tag="po")
                    for ct in range(KT):
                        pslc = projBF[:, ct * 512 : ct * 512 + ln]
                        eng2 = nc.gpsimd if USE_GPS_TAIL else nc.vector
                        eng2.tensor_mul(p2[:, ct * 512 : ct * 512 + ln],
                                        pslc, pslc)
                        nc.tensor.matmul(ps_q2[0:1, 0:ln], ones_col_bf[:],
                                         p2[:, ct * 512 : ct * 512 + ln],
                                         start=(ct == 0), stop=(ct == KT - 1))
                    nc.scalar.mul(rows[0:1, sca:scb], ps_q2[0:1, 0:ln], 1.0 / C)
                    nc.vector.tensor_mul(rows[0:1, ra:rb2], rows[0:1, a0 : a0 + ln],
                                         rows[0:1, a0 : a0 + ln])
                    nc.vector.tensor_sub(rows[0:1, ra:rb2], rows[0:1, sca:scb],
                                         rows[0:1, ra:rb2])
                    nc.scalar.activation(rows[0:1, ra:rb2], rows[0:1, ra:rb2],
                                         AF.Ln, bias=eps_col[0:1, :])
                    nc.scalar.activation(rows[0:1, ra:rb2], rows[0:1, ra:rb2],
                                         AF.Exp, scale=-0.5)
                    # mrs = mu * rs ; f32r rows for the g*rs / g*mu*rs bcasts
                    rs2r = prow.tile([1, 2 * 512], F32R, tag="r2")
                    nc.vector.tensor_mul(rows[0:1, sca:scb], rows[0:1, a0 : a0 + ln],
                                         rows[0:1, ra:rb2])
                    nc.vector.tensor_copy(rs2r[0:1, 0:ln], rows[0:1, ra:rb2])
                    nc.vector.tensor_copy(rs2r[0:1, 512 : 512 + ln],
                                          rows[0:1, sca:scb])
                    ps_gb = psB.tile([128, 512], F32, tag="po")
                    ps_gm = psB.tile([128, 512], F32, tag="po")
                    for ct in range(KT):
                        gsl = outgr_r[0:1, ct * 128 : (ct + 1) * 128]
                        nc.tensor.matmul(ps_gb[:, 0:ln], gsl, rs2r[0:1, 0:ln])
                        nc.tensor.matmul(ps_gm[:, 0:ln], gsl,
                                         rs2r[0:1, 512 : 512 + ln])
                        yt = pyt.tile([128, 512], F32, tag="yt")
                        pslice = projBF[:, ct * 512 : ct * 512 + ln]
                        nc.vector.tensor_mul(yt[:, 0:ln], pslice, ps_gb[:, 0:ln])
                        nc.vector.tensor_sub(yt[:, 0:ln], yt[:, 0:ln],
                                             ps_gm[:, 0:ln])
                        eng3 = nc.gpsimd if USE_GPS_TAIL else nc.vector
                        eng3.tensor_add(
                            yt[:, 0:ln], yt[:, 0:ln],
                            x_sb[:, ct * NH + a0 : ct * NH + a0 + ln])
                        nc.sync.dma_start(
                            y[ct * 128 : (ct + 1) * 128, a0 : a0 + ln],
                            yt[:, 0:ln])

                return [ph_norm_a, ph_norm_b, ph_proj, ph_ln2y]

            phases = []
            for cc, (a0, ln) in enumerate(CHUNKS):
                pocs = []
                dent = pden.tile([8, 512], BF, tag="dent")
                for hg in range(HEADS // 2):
                    if ln == 512:
                        run_hg_512(cc, hg, a0, ln, dent, pocs)
                    else:
                        run_hg_128(cc, hg, a0, ln, dent, pocs)
                    if phases:
                        phases.pop(0)()
                phases = tail_phases(cc, a0, ln, dent, pocs)
            for ph in phases:
                ph()
    _split_multi_waits(nc)
    return nc


def _prep_inputs(x, context, norm_gamma, null_kv, Wq, Wkv, ctx_ln_g, ctx_ln_b,
                 Wctx, bctx, Wout, out_ln_g):
    import ml_dtypes
    bf = ml_dtypes.bfloat16
    f = np.float32
    x = np.asarray(x, f).reshape(4, C, N)
    context = np.asarray(context, f)
    g = np.asarray(norm_gamma, f)
    scale = 1.0 / np.sqrt(DH)
    wq_h = (g[:, None] * np.asarray(Wq, f)) * scale
    negcq_h = -wq_h.sum(0, dtype=np.float64).astype(f)[None, :]
    wkv_h = g[:, None] * np.asarray(Wkv, f)
    # combined stationary: [v | k] so k lands on psum rows 64:128
    wkvc_h = np.concatenate([wkv_h[:, DH:], wkv_h[:, :DH]], axis=1)
    ncs = -wkv_h.sum(0, dtype=np.float64).astype(f)
    ncskv_h = np.concatenate([ncs[DH:], ncs[:DH]])[None, :]
    wctx_h = np.asarray(ctx_ln_g, f)[:, None] * np.asarray(Wctx, f)
    bctx_h = (np.asarray(bctx, f) + np.asarray(ctx_ln_b, f) @ np.asarray(Wctx, f))
    null = np.asarray(null_kv, f)
    wout_b = np.concatenate(
        [np.asarray(Wout, f)[h * DH:(h + 1) * DH, :] for h in range(HEADS)], axis=1)
    sel_h = np.kron(np.eye(8, dtype=f), np.ones((1, DH), f))

    shared = {
        "wq": np.ascontiguousarray(wq_h).astype(bf),
        "negcq": negcq_h.astype(bf),
        "wkvc": np.ascontiguousarray(wkvc_h).astype(bf),
        "ncskv": np.ascontiguousarray(ncskv_h).astype(bf),
        "wctx": np.ascontiguousarray(wctx_h),
        "bctxk": np.ascontiguousarray(bctx_h[:DH, None]),
        "bctxv": np.ascontiguousarray(bctx_h[DH:, None]),
        "nullkt": np.ascontiguousarray(null[0][:, None]),
        "nullv": np.ascontiguousarray(null[1][:, None]),
        "wout": np.ascontiguousarray(wout_b).astype(bf),
        "selin": sel_h,
        "outgr": np.ascontiguousarray(np.asarray(out_ln_g, f)[None, :]),
    }
    in_maps = []
    for core in range(8):
        b, half = core // 2, core % 2
        m = dict(shared)
        xo = x[b][:, half * NH : (half + 1) * NH]
        xt = x[b][:, (1 - half) * NH : (2 - half) * NH]
        m["x_own"] = np.ascontiguousarray(xo)
        m["xbf"] = np.ascontiguousarray(
            np.concatenate([xo, xt], axis=1)).astype(bf)
        m["ctxt"] = np.ascontiguousarray(context[b])
        in_maps.append(m)
    return in_maps


_LDW_OPT = [False]


def _patch_ldw_opt():
    import concourse.bass_utils as bu
    if getattr(bu, "_ldwopt_patched", False):
        return
    orig = bu.run_command

    def run2(cmd, **kw):
        if _LDW_OPT[0]:
            cmd = [c.replace("--enable-ldw-opt=false", "--enable-ldw-opt=true")
                   for c in cmd]
        return orig(cmd, **kw)

    bu.run_command = run2
    bu._ldwopt_patched = True


def kernel(**inputs):
    from concourse.bass_utils import run_bass_kernel_spmd
    _patch_ldw_opt()

    if "nc" not in _cached:
        _cached["nc"] = _build_bass()
    nc = _cached["nc"]
    in_maps = _prep_inputs(**inputs)
    kw = {}
    if PROFILE:
        import importlib.util

        if "antenv.axon_hooks" not in sys.modules:
            spec = importlib.util.spec_from_file_location(
                "antenv.axon_hooks", "/opt/trn_rl_repo/antenv/axon_hooks.py")
            m = importlib.util.module_from_spec(spec)
            spec.loader.exec_module(m)
            sys.modules["antenv.axon_hooks"] = m
            import antenv

            antenv.axon_hooks = m
        kw = dict(trace=True, tmpdir=PROFILE_DIR)
    res = run_bass_kernel_spmd(nc, in_maps, list(range(8)), **kw)
    _cached["last"] = res
    out = np.empty((4, C, N), np.float32)
    for core in range(8):
        b, half = core // 2, core % 2
        out[b][:, half * NH : (half + 1) * NH] = res.results[core]["y"]
    return out.reshape(4, C, 48, 48)


# revision 40
# speedup vs baseline: 1.3979x; 1.0005x over previous
"""Trainium2 Bass kernel for nn_Attention_LR_65249143160949 (cross-attention block).

Sharding: 8 cores = 4 batches x 2 token-halves (1152 tokens each). Each core
computes k/v for its whole batch (cheap MQA single head, duplicated within the
pair) and q/attention/output for its own tokens. The host permutes tokens so
each core's own rows come first -> identical SPMD program, no collectives.

On-chip layout: features on partitions, tokens on the free axis. LayerNorm is
folded into the projections (pre-scaled weights + rank-1 -colsum*mu term).
Attention runs in sim^T layout (keys on partitions, query tokens free): kT is
rs-scaled so softmax is a plain exp; the denominator comes free as a ones
column (col 64) of the 128-col-padded v stationary (row 64 of the out psum).

Engine balance (the v1 kernel was ACT+PE serialized at ~460us):
- exp is split ACT (exact, bf16 out) / DVE (Schraudolph bit-trick: i16 =
  round(sim*184.665 + 16250.4) bitcast bf16, ~3% max err on weights).
- all fp32 broadcast/stat matmuls use float32r (1 cyc/row vs 4) or bf16.
- the 24 per-head reciprocals are batched: den rows DMA-gathered to one
  [8, 512] tile, ONE reciprocal per chunk, then per-head selector matmuls
  (K=8 one-hot-row-of-ones, f32r) broadcast 1/den to 64 partitions.
- GPSIMD (cannot touch PSUM) takes SBUF-only elementwise work: LN1 x^2,
  LN2 bf16 casts + squares, the residual add.
- v/wkv stationaries are 128-col padded/fused so FWL weight loads stay fast.
- x arrives bf16 from the host in token chunks (LN1 starts ~3us in); the
  fp32 x needed only for the residual is DMA'd last.

Walrus quirks handled: one sync-wait per TPB instruction (_split_multi_waits),
no custom DVE ops, engine ops must start at partition 0/32/64/96, GPSIMD has
no PSUM access, f32r tiles must be produced by a rounding op (DVE copy).
"""

import sys

import numpy as np

if "/opt/trn_rl_repo" not in sys.path:
    sys.path.insert(0, "/opt/trn_rl_repo")

C = 512          # channels
N = 2304         # tokens per batch (48*48)
NH = 1152        # tokens per core
HEADS = 8
DH = 64
CTXL = 77
CTXD = 768
JT = 19          # j tiles of 128: 18 img + 1 (ctx 0:77 | null 77 | pad)
JP = JT * 128
CHUNKS = [(0, 512), (512, 512), (1024, 128)]  # (start, len) token chunks
NCH = len(CHUNKS)
KT = 4           # C / 128
EPS = 1e-5

FEXP_S = 184.6649186888274   # 128 / ln(2)
FEXP_C = 16250.4             # 127*128 - 5.6 (minimax-tuned, round-to-nearest)

PROFILE = False
PROFILE_DIR = None

_cached = {}


USE_DVE_EXP = True
USE_GPS_TAIL = True
USE_F32R = True
USE_DENT_DMA = True
ALAG = 2         # attn.v lags exp by this many j-tiles (512-token chunks)


def _exp_engine(cc, hg, jt):
    """Engine for the softmax exp of (chunk cc, head-pair hg, j-tile jt).
    512-token chunks: every 4th j-tile on DVE (bit-trick exp). Returns
    'act' or 'dve'."""
    if not USE_DVE_EXP:
        return "act"
    return "dve" if (jt % 4) == 3 else "act"


def _exp_engine_128(hg, grp):
    """Engine for the packed 4-j-tile exp groups of the 128-token chunk."""
    if not USE_DVE_EXP:
        return "act"
    return "dve" if grp in (1, 3) else "act"


def _split_multi_waits(nc):
    """Walrus codegen supports one sync-wait per TPB instruction (the EVENTS
    struct has a single wait slot). Tile attaches several. Split the extras
    onto same-engine NoOps inserted just before each instruction."""
    import concourse.mybir as mybir

    n = 0
    for fn in nc.m.functions:
        for bb in fn.blocks:
            insts = bb.instructions
            i = 0
            while i < len(insts):
                ins = insts[i]
                si = getattr(ins, "sync_info", None)
                if si is not None and si.on_wait and len(si.on_wait) > 1:
                    waits = list(si.on_wait)
                    for w in waits[:-1]:
                        n += 1
                        nop = mybir.InstNoOp(name=f"WSPLIT-{n}", engine=ins.engine)
                        nop.sync_info = mybir.SyncInfo(on_wait=[w], on_update=[])
                        insts.insert(i, nop)
                        i += 1
                    ins.sync_info = mybir.SyncInfo(
                        on_wait=[waits[-1]], on_update=si.on_update)
                i += 1
    return n


def _build_bass():
    import concourse.bass as bass
    import concourse.mybir as mybir
    import concourse.tile as tile
    from concourse.masks import make_identity
    from contextlib import ExitStack

    F32 = mybir.dt.float32
    F32R = mybir.dt.float32r if USE_F32R else mybir.dt.float32
    BF = mybir.dt.bfloat16
    I16 = mybir.dt.int16
    AF = mybir.ActivationFunctionType
    ALU = mybir.AluOpType

    nc = bass.Bass()
    xbf = nc.declare_dram_parameter("xbf", [C, N], BF, isOutput=False)
    x_own = nc.declare_dram_parameter("x_own", [C, NH], F32, isOutput=False)
    ctxt = nc.declare_dram_parameter("ctxt", [CTXL, CTXD], F32, isOutput=False)
    wq = nc.declare_dram_parameter("wq", [C, C], BF, isOutput=False)
    negcq = nc.declare_dram_parameter("negcq", [1, C], BF, isOutput=False)
    wkvc = nc.declare_dram_parameter("wkvc", [C, 128], BF, isOutput=False)
    ncskv = nc.declare_dram_parameter("ncskv", [1, 128], BF, isOutput=False)
    wctx = nc.declare_dram_parameter("wctx", [CTXD, 2 * DH], F32, isOutput=False)
    bctxk = nc.declare_dram_parameter("bctxk", [DH, 1], F32, isOutput=False)
    bctxv = nc.declare_dram_parameter("bctxv", [DH, 1], F32, isOutput=False)
    nullkt = nc.declare_dram_parameter("nullkt", [DH, 1], F32, isOutput=False)
    nullv = nc.declare_dram_parameter("nullv", [DH, 1], F32, isOutput=False)
    wout = nc.declare_dram_parameter("wout", [DH, HEADS * C], BF, isOutput=False)
    selin = nc.declare_dram_parameter("selin", [8, 8 * DH], F32, isOutput=False)
    outgr = nc.declare_dram_parameter("outgr", [1, C], F32, isOutput=False)
    y = nc.declare_dram_parameter("y", [C, NH], F32, isOutput=True)

    with tile.TileContext(nc) as tc, ExitStack() as ctx:
        pconst = ctx.enter_context(tc.tile_pool(name="const", bufs=1))
        pbig = ctx.enter_context(tc.tile_pool(name="big", bufs=1))

        ident = pconst.tile([128, 128], F32)
        make_identity(nc, ident[:])
        ident_bf = pconst.tile([128, 128], BF)
        make_identity(nc, ident_bf[:])
        ones_col = pconst.tile([128, 1], F32)
        nc.vector.memset(ones_col[:], 1.0)
        ones_col_bf = pconst.tile([128, 1], BF)
        nc.vector.memset(ones_col_bf[:], 1.0)
        ones_f = pconst.tile([1, 128], F32)
        nc.vector.memset(ones_f[:], 1.0)
        ones_r = pconst.tile([1, 128], F32R)
        nc.vector.tensor_copy(ones_r[:], ones_f[:])
        eps_col = pconst.tile([128, 1], F32)
        nc.vector.memset(eps_col[:], EPS)
        ones_blk = pconst.tile([128, 64], F32)
        nc.vector.memset(ones_blk[:], 1.0)
        sel_f = pconst.tile([8, 8 * DH], F32)
        sel = pconst.tile([8, 8 * DH], F32R)
        outgr_f = pconst.tile([1, C], F32)
        outgr_r = pconst.tile([1, C], F32R)

        x_sb = pbig.tile([128, KT * NH], F32)        # kt-major; OWN half only
        x_bf = pbig.tile([128, KT * N], BF)
        qT = pbig.tile([128, (HEADS // 2) * NH], BF)  # head-pair blocks
        kT2 = pbig.tile([128, JP], BF)               # rs-scaled keys, both halves
        v_sb = pbig.tile([128, JT * 128], BF)        # per j-tile [v 0:64|ones 64|pad]
        projBF = pbig.tile([128, KT * 512], BF)      # bf16 proj (per chunk)
        stats = pbig.tile([128, 40], F32)            # col jt: rs_j (v scaling)
        wout_sb = pbig.tile([64, HEADS * C], BF)
        # per-token stat rows on partition 0: mu 0:N | rs N:2N
        # (LN2 reuses per cc: mu2 at cc*CH, rs2 at N+cc*CH, ex2 at 2N+cc*CH)
        rows = pbig.tile([1, 2 * N + NH], F32)
        rows_bf = pbig.tile([1, N], BF)
        rows_r = pbig.tile([1, N], F32R)             # f32r copy of rs row
        R_RS, R_SC = N, 2 * N

        nc.sync.dma_start(sel_f[:], selin[:, :])
        nc.sync.dma_start(outgr_f[:], outgr[:, :])
        nc.vector.tensor_copy(sel[:], sel_f[:])
        nc.vector.tensor_copy(outgr_r[:], outgr_f[:])
        nc.sync.dma_start(wout_sb[:], wout[:, :])

        with tc.tile_pool(name="load", bufs=1) as pload, \
             tc.tile_pool(name="x2p", bufs=2) as px2, \
             tc.tile_pool(name="pss", bufs=2, space="PSUM") as pss:
            # psum tags: b1 [<=64,384]x2, bS [128,<=512]x4, bT [128,128]x2
            wq_sb = pload.tile([128, KT * C], BF)
            wkv_sb = pload.tile([128, KT * 128], BF)
            wctx_sb = pload.tile([128, CTXD], F32)
            negcq_sb = pload.tile([1, C], BF)
            ncskv_sb = pload.tile([1, 128], BF)
            bctxk_sb = pload.tile([DH, 1], F32)
            bctxv_sb = pload.tile([DH, 1], F32)
            vT = pload.tile([64, N], BF)
            ck_sb = pload.tile([64, CTXL], F32)
            cv_sb = pload.tile([64, CTXL + 1], F32)
            nullk_st = pload.tile([DH, 1], F32)
            nullv_st = pload.tile([DH, 1], F32)
            ctx_sb = pload.tile([CTXL, CTXD], F32)
            ctxnT = pload.tile([128, 6 * CTXL], F32)
            ex2 = pload.tile([1, N], F32)
            kk = pload.tile([128, 512], F32)

            # Two HWDGE queues: x_bf chunks go on the ACT queue so LN1 can
            # start while the SP queue streams ctx/weights.
            xbf_v = x_bf[:].rearrange("p (k n) -> p k n", k=KT)
            xbf_d = xbf[:].rearrange("(k p) n -> p k n", p=128)
            for ch in range(6):
                a, b = ch * 384, (ch + 1) * 384
                nc.scalar.dma_start(xbf_v[:, :, a:b], xbf_d[:, :, a:b])
            nc.sync.dma_start(ctx_sb[:], ctxt[:, :])
            nc.sync.dma_start(wctx_sb[:].rearrange("p (k n) -> p k n", k=6),
                              wctx[:].rearrange("(k p) n -> p k n", p=128))
            nc.sync.dma_start(bctxk_sb[:], bctxk[:, :])
            nc.sync.dma_start(bctxv_sb[:], bctxv[:, :])
            nc.sync.dma_start(nullk_st[:], nullkt[:, :])
            nc.sync.dma_start(nullv_st[:], nullv[:, :])
            nc.sync.dma_start(wkv_sb[:].rearrange("p (k n) -> p k n", k=KT),
                              wkvc[:].rearrange("(k p) n -> p k n", p=128))
            nc.sync.dma_start(ncskv_sb[:], ncskv[:, :])
            nc.scalar.dma_start(wq_sb[:].rearrange("p (k n) -> p k n", k=KT),
                                wq[:].rearrange("(k p) n -> p k n", p=128))
            nc.sync.dma_start(negcq_sb[:], negcq[:, :])
            x_v = x_sb[:].rearrange("p (k n) -> p k n", k=KT)
            nc.sync.dma_start(x_v[:, :, 0:NH],
                              x_own[:].rearrange("(k p) n -> p k n", p=128))

            # ---- LN1 stats (row form): mu, then rs = exp(-0.5 ln(var+eps)) ----
            for ch in range(6):
                sl = slice(ch * 384, (ch + 1) * 384)
                ps_r1 = pss.tile([64, 384], F32, tag="b1")
                for kt in range(KT):
                    nc.tensor.matmul(
                        ps_r1[0:1, :], ones_col_bf[:],
                        x_bf[:, kt * N + ch * 384 : kt * N + (ch + 1) * 384],
                        start=(kt == 0), stop=(kt == KT - 1))
                nc.scalar.mul(rows[0:1, sl], ps_r1[0:1, :], 1.0 / C)
                nc.vector.tensor_copy(rows_bf[0:1, sl], rows[0:1, sl])
            for ch in range(6):
                x2 = px2.tile([128, KT * 384], BF, tag="x2")
                ps_r2 = pss.tile([64, 384], F32, tag="b1")
                for kt in range(KT):
                    xs = x_bf[:, kt * N + ch * 384 : kt * N + (ch + 1) * 384]
                    nc.vector.tensor_mul(x2[:, kt * 384 : (kt + 1) * 384], xs, xs)
                    nc.tensor.matmul(
                        ps_r2[0:1, :], ones_col_bf[:],
                        x2[:, kt * 384 : (kt + 1) * 384],
                        start=(kt == 0), stop=(kt == KT - 1))
                nc.scalar.mul(ex2[0:1, ch * 384 : (ch + 1) * 384],
                              ps_r2[0:1, :], 1.0 / C)
            for ch in range(6):
                a, b = R_RS + ch * 384, R_RS + (ch + 1) * 384
                mu = rows[0:1, ch * 384 : (ch + 1) * 384]
                nc.vector.tensor_mul(rows[0:1, a:b], mu, mu)
                nc.vector.tensor_sub(rows[0:1, a:b],
                                     ex2[0:1, ch * 384 : (ch + 1) * 384],
                                     rows[0:1, a:b])
                nc.scalar.activation(rows[0:1, a:b], rows[0:1, a:b], AF.Ln,
                                     bias=eps_col[0:1, :])
                nc.scalar.activation(rows[0:1, a:b], rows[0:1, a:b], AF.Exp,
                                     scale=-0.5)
            nc.vector.tensor_copy(rows_r[0:1, :], rows[0:1, R_RS : R_RS + N])
            # ---- context: LN (layout A, bn_stats) + k/v projection ----
            cstat = pload.tile([CTXL, 3, 6], F32)
            for sg in range(3):
                nc.vector.bn_stats(cstat[:, sg, :],
                                   ctx_sb[:, sg * 256 : (sg + 1) * 256])
            cmv = pload.tile([CTXL, 2], F32)
            nc.vector.bn_aggr(cmv[:], cstat[:])
            nc.scalar.activation(cmv[:, 1:2], cmv[:, 1:2], AF.Ln,
                                 bias=eps_col[0:CTXL, :])
            nc.scalar.activation(cmv[:, 1:2], cmv[:, 1:2], AF.Exp, scale=-0.5)
            nc.vector.tensor_scalar(
                out=ctx_sb[:], in0=ctx_sb[:],
                scalar1=cmv[:, 0:1], scalar2=cmv[:, 1:2],
                op0=ALU.subtract, op1=ALU.mult)
            for kt in range(6):
                ps_ct = pss.tile([128, 128], F32, tag="bT")
                nc.tensor.transpose(ps_ct[:, 0:CTXL],
                                    ctx_sb[:, kt * 128 : (kt + 1) * 128],
                                    ident[:CTXL, :CTXL])
                nc.vector.tensor_copy(ctxnT[:, kt * CTXL : (kt + 1) * CTXL],
                                      ps_ct[:, 0:CTXL])
            ps_ck = pss.tile([64, 384], F32, tag="b1")
            ps_cv = pss.tile([64, 384], F32, tag="b1")
            for kt in range(6):
                nc.tensor.matmul(ps_ck[:, 0:CTXL],
                                 wctx_sb[:, kt * 128 : kt * 128 + DH],
                                 ctxnT[:, kt * CTXL : (kt + 1) * CTXL],
                                 start=(kt == 0), stop=(kt == 5))
                nc.tensor.matmul(ps_cv[:, 0:CTXL],
                                 wctx_sb[:, kt * 128 + DH : (kt + 1) * 128],
                                 ctxnT[:, kt * CTXL : (kt + 1) * CTXL],
                                 start=(kt == 0), stop=(kt == 5))
            nc.vector.tensor_scalar_add(ck_sb[:], ps_ck[:, 0:CTXL], bctxk_sb[:])
            nc.vector.tensor_scalar_add(cv_sb[:, 0:CTXL], ps_cv[:, 0:CTXL],
                                        bctxv_sb[:])
            nc.vector.tensor_copy(cv_sb[:, CTXL : CTXL + 1], nullv_st[:])

            # rs as per-partition columns (v scaling)
            for jt in range(18):
                ps_c = pss.tile([128, 128], F32, tag="bT")
                nc.tensor.matmul(ps_c[:, 0:1],
                                 rows[0:1, R_RS + jt * 128 : R_RS + (jt + 1) * 128],
                                 ones_col[0:1, :])
                nc.vector.tensor_copy(stats[:, jt : jt + 1], ps_c[:, 0:1])

            # ---- j-tile 18: [ctx 0:77 | null 77 | pad 78:128] ----
            nc.gpsimd.memset(v_sb[:], 0.0)
            nc.gpsimd.memset(kT2[0:64, 18 * 128 : JP], 0.0)
            nc.vector.tensor_copy(kT2[0:64, 18 * 128 : 18 * 128 + CTXL], ck_sb[:])
            nc.vector.tensor_copy(kT2[0:64, 18 * 128 + CTXL : 18 * 128 + CTXL + 1],
                                  nullk_st[:])
            VB = 18 * 128
            ps_cvt = pss.tile([128, 128], F32, tag="bT")
            nc.tensor.transpose(ps_cvt[0 : CTXL + 1, 0:64], cv_sb[:],
                                ident[:64, :64])
            nc.vector.tensor_copy(v_sb[0 : CTXL + 1, VB : VB + DH],
                                  ps_cvt[0 : CTXL + 1, 0:64])
            nc.vector.memset(v_sb[0 : CTXL + 1, VB + DH : VB + DH + 1], 1.0)

            # ---- kv projection (all tokens; LN folded; k into kT2 top half) ----
            KV_CHUNKS = [(0, 512), (512, 512), (1024, 512), (1536, 512), (2048, 256)]
            for kva, kvl in KV_CHUNKS:
                sl = slice(kva, kva + kvl)
                ps_kv = pss.tile([128, 512], F32, tag="bS")
                for kt in range(KT):
                    xs = x_bf[:, kt * N + kva : kt * N + kva + kvl]
                    nc.tensor.matmul(ps_kv[:, 0:kvl],
                                     wkv_sb[:, kt * 128 : (kt + 1) * 128],
                                     xs, start=(kt == 0), stop=False)
                nc.tensor.matmul(ps_kv[:, 0:kvl], ncskv_sb[:], rows_bf[0:1, sl],
                                 start=False, stop=True)
                ps_bc = pss.tile([128, 512], F32, tag="bS")
                nc.tensor.matmul(ps_bc[:, 0:kvl], ones_r[0:1, 0:128],
                                 rows_r[0:1, sl])
                nc.vector.tensor_copy(kk[64:128, 0:kvl], ps_kv[64:128, 0:kvl])
                nc.vector.tensor_mul(kT2[64:128, sl], kk[64:128, 0:kvl],
                                     ps_bc[64:128, 0:kvl])
                nc.vector.tensor_copy(vT[:, sl], ps_kv[0:64, 0:kvl])

            # ---- v tiles: transpose + rs scale + ones col ----
            for jt in range(18):
                ps_vt = pss.tile([128, 128], BF, tag="bT")
                nc.tensor.transpose(ps_vt[:, 0:64], vT[:, jt * 128 : (jt + 1) * 128],
                                    ident_bf[:64, :64])
                vb = jt * 128
                nc.vector.tensor_scalar_mul(v_sb[:, vb : vb + DH], ps_vt[:, 0:64],
                                            stats[:, jt : jt + 1])
                nc.vector.memset(v_sb[:, vb + DH : vb + DH + 1], 1.0)

            # ---- duplicate kT to partitions 0:64 (sbuf->sbuf DMA) ----
            nc.sync.dma_start(kT2[0:64, 0 : 18 * 128], kT2[64:128, 0 : 18 * 128])
            nc.sync.dma_start(kT2[64:128, 18 * 128 : JP],
                              kT2[0:64, 18 * 128 : JP])

            # ---- q projection (head pairs; LN + 1/sqrt(dh) folded) ----
            for a0, ln in CHUNKS:
                sl = slice(a0, a0 + ln)
                ps_rs = pss.tile([128, 512], F32, tag="bS")
                nc.tensor.matmul(ps_rs[:, 0:ln], ones_r[0:1, :],
                                 rows_r[0:1, sl])
                rs_b = px2.tile([128, 512], F32, tag="rsb")
                nc.vector.tensor_copy(rs_b[:, 0:ln], ps_rs[:, 0:ln])
                for hg in range(HEADS // 2):
                    ps_q = pss.tile([128, 512], F32, tag="bS")
                    for kt in range(KT):
                        nc.tensor.matmul(
                            ps_q[:, 0:ln],
                            wq_sb[:, kt * C + hg * 128 : kt * C + (hg + 1) * 128],
                            x_bf[:, kt * N + a0 : kt * N + a0 + ln],
                            start=(kt == 0), stop=False)
                    nc.tensor.matmul(ps_q[:, 0:ln],
                                     negcq_sb[0:1, hg * 128 : (hg + 1) * 128],
                                     rows_bf[0:1, sl], start=False, stop=True)
                    nc.vector.tensor_mul(
                        qT[:, hg * NH + a0 : hg * NH + a0 + ln],
                        ps_q[:, 0:ln], rs_b[:, 0:ln])

        # ========= attention + output + LN2 + residual, per chunk =========
        # Per (chunk, head-pair): sim pair (row-tiled concurrent) -> exp on
        # ACT or DVE (bit-trick) -> attn.v pair lagging one j-tile. The tail
        # (recip batch, norm, out-proj, LN2, y) is deferred one stage so it
        # never head-of-line-blocks the PE queue.
        with tc.tile_pool(name="attb", bufs=4) as patb, \
             tc.tile_pool(name="atti", bufs=4) as pati, \
             tc.tile_pool(name="outp", bufs=9) as pout, \
             tc.tile_pool(name="pocp", bufs=12) as ppoc, \
             tc.tile_pool(name="denp", bufs=2) as pden, \
             tc.tile_pool(name="recp", bufs=2) as prec, \
             tc.tile_pool(name="rowp", bufs=1) as prow, \
             tc.tile_pool(name="p2p", bufs=1) as pp2, \
             tc.tile_pool(name="yp", bufs=2) as pyt, \
             tc.tile_pool(name="psatt", bufs=2, space="PSUM") as psA, \
             tc.tile_pool(name="psacc", bufs=4, space="PSUM") as psB:

            def at_tile(engine):
                if engine == "act":
                    return patb.tile([128, 1024], BF, tag="atb", name="atb")
                return pati.tile([128, 1024], I16, tag="ati", name="ati")

            def do_exp(engine, at, ps_s, lo, hi):
                if engine == "act":
                    nc.scalar.activation(at[:, lo:hi], ps_s[:, lo:hi], AF.Exp)
                else:
                    nc.vector.tensor_scalar(
                        out=at[:, lo:hi], in0=ps_s[:, lo:hi],
                        scalar1=FEXP_S, scalar2=FEXP_C,
                        op0=ALU.mult, op1=ALU.add)

            def at_slice(at_eng, lo, hi):
                at, eng = at_eng
                ap = at[:, lo:hi]
                return ap if eng == "act" else ap.bitcast(BF)

            def run_hg_512(cc, hg, a0, ln, dent, pocs):
                po0 = psB.tile([128, 512], F32, tag="po")
                po1 = psB.tile([128, 512], F32, tag="po")
                po = [po0, po1]
                q0 = qT[0:64, hg * NH + a0 : hg * NH + a0 + ln]
                q1 = qT[64:128, hg * NH + a0 : hg * NH + a0 + ln]
                ats = [None] * JT
                for jt in range(JT):
                    ps_s = psA.tile([128, 1024], F32, tag="sim")
                    nc.tensor.matmul(ps_s[:, 0:ln],
                                     kT2[0:64, jt * 128 : (jt + 1) * 128],
                                     q0, start=True, stop=True)
                    nc.tensor.matmul(ps_s[:, 512 : 512 + ln],
                                     kT2[64:128, jt * 128 : (jt + 1) * 128],
                                     q1, start=True, stop=True)
                    eng = _exp_engine(cc, hg, jt)
                    at = at_tile(eng)
                    do_exp(eng, at, ps_s, 0, 1024)
                    ats[jt] = (at, eng)
                    if jt >= ALAG:
                        j0 = jt - ALAG
                        vs = v_sb[:, j0 * 128 : (j0 + 1) * 128]
                        nc.tensor.matmul(po[0][:, 0:ln], vs,
                                         at_slice(ats[j0], 0, ln),
                                         start=(j0 == 0), stop=False)
                        nc.tensor.matmul(po[1][:, 0:ln], vs,
                                         at_slice(ats[j0], 512, 512 + ln),
                                         start=(j0 == 0), stop=False)
                        ats[j0] = None
                for j0 in range(JT - ALAG, JT):
                    vs = v_sb[:, j0 * 128 : (j0 + 1) * 128]
                    nc.tensor.matmul(po[0][:, 0:ln], vs, at_slice(ats[j0], 0, ln),
                                     start=(j0 == 0), stop=(j0 == JT - 1))
                    nc.tensor.matmul(po[1][:, 0:ln], vs,
                                     at_slice(ats[j0], 512, 512 + ln),
                                     start=(j0 == 0), stop=(j0 == JT - 1))
                _extract(hg, ln, po, dent, pocs)

            def run_hg_128(cc, hg, a0, ln, dent, pocs):
                # 19 j-tiles packed 4-per-psum-tile; slot g: q0 at col 128g
                # (bank A), q1 at 512+128g (bank B) so the row-tiled
                # concurrent sim pair never co-writes one psum bank.
                po0 = psB.tile([128, 512], F32, tag="po")
                po1 = psB.tile([128, 512], F32, tag="po")
                po = [po0, po1]
                q0 = qT[0:64, hg * NH + a0 : hg * NH + a0 + ln]
                q1 = qT[64:128, hg * NH + a0 : hg * NH + a0 + ln]
                GRP = [(0, 4), (4, 4), (8, 4), (12, 4), (16, 3)]
                ats = [None] * len(GRP)
                for gi, (jt0, ng) in enumerate(GRP):
                    ps_s = psA.tile([128, 1024], F32, tag="sim")
                    for g in range(ng):
                        jt = jt0 + g
                        nc.tensor.matmul(
                            ps_s[:, 128 * g : 128 * g + ln],
                            kT2[0:64, jt * 128 : (jt + 1) * 128],
                            q0, start=True, stop=True)
                        nc.tensor.matmul(
                            ps_s[:, 512 + 128 * g : 512 + 128 * g + ln],
                            kT2[64:128, jt * 128 : (jt + 1) * 128],
                            q1, start=True, stop=True)
                    eng = _exp_engine_128(hg, gi)
                    at = at_tile(eng)
                    if ng == 4:
                        do_exp(eng, at, ps_s, 0, 1024)
                    else:
                        do_exp(eng, at, ps_s, 0, 128 * ng)
                        do_exp(eng, at, ps_s, 512, 512 + 128 * ng)
                    ats[gi] = (at, eng)
                    if gi > 0:
                        _attnv_128(po, ats[gi - 1], GRP[gi - 1], ln,
                                   start=(gi == 1), stop=False)
                        ats[gi - 1] = None
                _attnv_128(po, ats[-1], GRP[-1], ln, start=False, stop=True)
                _extract(hg, ln, po, dent, pocs)

            def _attnv_128(po, at_eng, grp, ln, start, stop):
                jt0, ng = grp
                for g in range(ng):
                    jt = jt0 + g
                    vs = v_sb[:, jt * 128 : (jt + 1) * 128]
                    last = stop and (g == ng - 1)
                    nc.tensor.matmul(po[0][:, 0:ln], vs,
                                     at_slice(at_eng, 128 * g, 128 * g + ln),
                                     start=(start and g == 0), stop=last)
                    nc.tensor.matmul(po[1][:, 0:ln], vs,
                                     at_slice(at_eng, 512 + 128 * g,
                                              512 + 128 * g + ln),
                                     start=(start and g == 0), stop=last)

            def _extract(hg, ln, po, dent, pocs):
                for i in range(2):
                    poc = ppoc.tile([65, 512], BF, tag="poc")
                    nc.vector.tensor_copy(poc[:, 0:ln], po[i][0:65, 0:ln])
                    h = 2 * hg + i
                    if USE_DENT_DMA:
                        nc.sync.dma_start(dent[h : h + 1, 0:ln],
                                          poc[64:65, 0:ln])
                        pocs.append(poc)
                    else:
                        rec = ppoc.tile([65, 512], F32, tag="rech")
                        nc.vector.reciprocal(rec[64:65, 0:ln],
                                             po[i][64:65, 0:ln])
                        pocs.append((poc, rec))

            def tail_phases(cc, a0, ln, dent, pocs):
                ots = []
                rec8r = prec.tile([8, 512], F32R, tag="rec")

                def norm_head(h):
                    ps_rb = psB.tile([128, 512], F32, tag="po")
                    if USE_DENT_DMA:
                        nc.tensor.matmul(ps_rb[0:64, 0:ln],
                                         sel[:, h * DH : (h + 1) * DH],
                                         rec8r[:, 0:ln])
                        poc = pocs[h]
                    else:
                        poc, rec = pocs[h]
                        nc.tensor.matmul(ps_rb[0:64, 0:ln],
                                         ones_blk[64:65, 0:64],
                                         rec[64:65, 0:ln])
                    ot = pout.tile([64, 512], BF, tag="ot")
                    nc.vector.tensor_mul(ot[:, 0:ln], poc[:64, 0:ln],
                                         ps_rb[0:64, 0:ln])
                    ots.append(ot)

                def ph_norm_a():
                    if USE_DENT_DMA:
                        dentf = prec.tile([8, 512], F32, tag="dentf")
                        nc.vector.tensor_copy(dentf[:, 0:ln], dent[:, 0:ln])
                        rec8 = prec.tile([8, 512], F32, tag="rec8")
                        nc.vector.reciprocal(rec8[:, 0:ln], dentf[:, 0:ln])
                        nc.vector.tensor_copy(rec8r[:, 0:ln], rec8[:, 0:ln])
                    for h in range(4):
                        norm_head(h)

                def ph_norm_b():
                    for h in range(4, 8):
                        norm_head(h)

                def ph_proj():
                    for ct in range(KT):
                        ps_p = psB.tile([128, 512], F32, tag="po")
                        for h in range(HEADS):
                            nc.tensor.matmul(
                                ps_p[:, 0:ln],
                                wout_sb[:, h * C + ct * 128 : h * C + (ct + 1) * 128],
                                ots[h][:, 0:ln],
                                start=(h == 0), stop=(h == HEADS - 1))
                        nc.vector.tensor_copy(
                            projBF[:, ct * 512 : ct * 512 + ln],
                            ps_p[:, 0:ln])

                def ph_ln2y():
                    ra, rb2 = R_RS + a0, R_RS + a0 + ln
                    sca, scb = R_SC + a0, R_SC + a0 + ln
                    ps_m2 = psB.tile([128, 512], F32, tag="po")
                    for ct in range(KT):
                        nc.tensor.matmul(ps_m2[0:1, 0:ln], ones_col_bf[:],
                                         projBF[:, ct * 512 : ct * 512 + ln],
                                         start=(ct == 0), stop=(ct == KT - 1))
                    nc.scalar.mul(rows[0:1, a0 : a0 + ln], ps_m2[0:1, 0:ln], 1.0 / C)
                    p2 = pp2.tile([128, KT * 512], BF, tag="p2")
                    ps_q2 = psB.tile([128, 512], F32, tag="po")
                    for ct in range(KT):
                        pslc = projBF[:, ct * 512 : ct * 512 + ln]
                        eng2 = nc.gpsimd if USE_GPS_TAIL else nc.vector
                        eng2.tensor_mul(p2[:, ct * 512 : ct * 512 + ln],
                                        pslc, pslc)
                        nc.tensor.matmul(ps_q2[0:1, 0:ln], ones_col_bf[:],
                                         p2[:, ct * 512 : ct * 512 + ln],
                                         start=(ct == 0), stop=(ct == KT - 1))
                    nc.scalar.mul(rows[0:1, sca:scb], ps_q2[0:1, 0:ln], 1.0 / C)
                    nc.vector.tensor_mul(rows[0:1, ra:rb2], rows[0:1, a0 : a0 + ln],
                                         rows[0:1, a0 : a0 + ln])
                    nc.vector.tensor_sub(rows[0:1, ra:rb2], rows[0:1, sca:scb],
                                         rows[0:1, ra:rb2])
                    nc.scalar.activation(rows[0:1, ra:rb2], rows[0:1, ra:rb2],
                                         AF.Ln, bias=eps_col[0:1, :])
                    nc.scalar.activation(rows[0:1, ra:rb2], rows[0:1, ra:rb2],
                                         AF.Exp, scale=-0.5)
                    # mrs = mu * rs ; f32r rows for the g*rs / g*mu*rs bcasts
                    rs2r = prow.tile([1, 2 * 512], F32R, tag="r2")
                    nc.vector.tensor_mul(rows[0:1, sca:scb], rows[0:1, a0 : a0 + ln],
                                         rows[0:1, ra:rb2])
                    nc.vector.tensor_copy(rs2r[0:1, 0:ln], rows[0:1, ra:rb2])
                    nc.vector.tensor_copy(rs2r[0:1, 512 : 512 + ln],
                                          rows[0:1, sca:scb])
                    ps_gb = psB.tile([128, 512], F32, tag="po")
                    ps_gm = psB.tile([128, 512], F32, tag="po")
                    for ct in range(KT):
                        gsl = outgr_r[0:1, ct * 128 : (ct + 1) * 128]
                        nc.tensor.matmul(ps_gb[:, 0:ln], gsl, rs2r[0:1, 0:ln])
                        nc.tensor.matmul(ps_gm[:, 0:ln], gsl,
                                         rs2r[0:1, 512 : 512 + ln])
                        yt = pyt.tile([128, 512], F32, tag="yt")
                        pslice = projBF[:, ct * 512 : ct * 512 + ln]
                        nc.vector.tensor_mul(yt[:, 0:ln], pslice, ps_gb[:, 0:ln])
                        nc.vector.tensor_sub(yt[:, 0:ln], yt[:, 0:ln],
                                             ps_gm[:, 0:ln])
                        eng3 = nc.gpsimd if USE_GPS_TAIL else nc.vector
                        eng3.tensor_add(
                            yt[:, 0:ln], yt[:, 0:ln],
                            x_sb[:, ct * NH + a0 : ct * NH + a0 + ln])
                        nc.sync.dma_start(
                            y[ct * 128 : (ct + 1) * 128, a0 : a0 + ln],
                            yt[:, 0:ln])

                return [ph_norm_a, ph_norm_b, ph_proj, ph_ln2y]

            phases = []
            for cc, (a0, ln) in enumerate(CHUNKS):
                pocs = []
                dent = pden.tile([8, 512], BF, tag="dent")
                for hg in range(HEADS // 2):
                    if ln == 512:
                        run_hg_512(cc, hg, a0, ln, dent, pocs)
                    else:
                        run_hg_128(cc, hg, a0, ln, dent, pocs)
                    if phases:
                        phases.pop(0)()
                phases = tail_phases(cc, a0, ln, dent, pocs)
            for ph in phases:
                ph()
    _split_multi_waits(nc)
    return nc


def _prep_inputs(x, context, norm_gamma, null_kv, Wq, Wkv, ctx_ln_g, ctx_ln_b,
                 Wctx, bctx, Wout, out_ln_g):
    import ml_dtypes
    bf = ml_dtypes.bfloat16
    f = np.float32
    x = np.asarray(x, f).reshape(4, C, N)
    context = np.asarray(context, f)
    g = np.asarray(norm_gamma, f)
    scale = 1.0 / np.sqrt(DH)
    wq_h = (g[:, None] * np.asarray(Wq, f)) * scale
    negcq_h = -wq_h.sum(0, dtype=np.float64).astype(f)[None, :]
    wkv_h = g[:, None] * np.asarray(Wkv, f)
    # combined stationary: [v | k] so k lands on psum rows 64:128
    wkvc_h = np.concatenate([wkv_h[:, DH:], wkv_h[:, :DH]], axis=1)
    ncs = -wkv_h.sum(0, dtype=np.float64).astype(f)
    ncskv_h = np.concatenate([ncs[DH:], ncs[:DH]])[None, :]
    wctx_h = np.asarray(ctx_ln_g, f)[:, None] * np.asarray(Wctx, f)
    bctx_h = (np.asarray(bctx, f) + np.asarray(ctx_ln_b, f) @ np.asarray(Wctx, f))
    null = np.asarray(null_kv, f)
    wout_b = np.concatenate(
        [np.asarray(Wout, f)[h * DH:(h + 1) * DH, :] for h in range(HEADS)], axis=1)
    sel_h = np.kron(np.eye(8, dtype=f), np.ones((1, DH), f))

    shared = {
        "wq": np.ascontiguousarray(wq_h).astype(bf),
        "negcq": negcq_h.astype(bf),
        "wkvc": np.ascontiguousarray(wkvc_h).astype(bf),
        "ncskv": np.ascontiguousarray(ncskv_h).astype(bf),
        "wctx": np.ascontiguousarray(wctx_h),
        "bctxk": np.ascontiguousarray(bctx_h[:DH, None]),
        "bctxv": np.ascontiguousarray(bctx_h[DH:, None]),
        "nullkt": np.ascontiguousarray(null[0][:, None]),
        "nullv": np.ascontiguousarray(null[1][:, None]),
        "wout": np.ascontiguousarray(wout_b).astype(bf),
        "selin": sel_h,
        "outgr": np.ascontiguousarray(np.asarray(out_ln_g, f)[None, :]),
    }
    in_maps = []
    for core in range(8):
        b, half = core // 2, core % 2
        m = dict(shared)
        xo = x[b][:, half * NH : (half + 1) * NH]
        xt = x[b][:, (1 - half) * NH : (2 - half) * NH]
        m["x_own"] = np.ascontiguousarray(xo)
        m["xbf"] = np.ascontiguousarray(
            np.concatenate([xo, xt], axis=1)).astype(bf)
        m["ctxt"] = np.ascontiguousarray(context[b])
        in_maps.append(m)
    return in_maps


_LDW_OPT = [False]


def _patch_ldw_opt():
    import concourse.bass_utils as bu
    if getattr(bu, "_ldwopt_patched", False):
        return
    orig = bu.run_command

    def run2(cmd, **kw):
        if _LDW_OPT[0]:
            cmd = [c.replace("--enable-ldw-opt=false", "--enable-ldw-opt=true")
                   for c in cmd]
        return orig(cmd, **kw)

    bu.run_command = run2
    bu._ldwopt_patched = True


def kernel(**inputs):
    from concourse.bass_utils import run_bass_kernel_spmd
    _patch_ldw_opt()

    if "nc" not in _cached:
        _cached["nc"] = _build_bass()
    nc = _cached["nc"]
    in_maps = _prep_inputs(**inputs)
    kw = {}
    if PROFILE:
        import importlib.util

        if "antenv.axon_hooks" not in sys.modules:
            spec = importlib.util.spec_from_file_location(
                "antenv.axon_hooks", "/opt/trn_rl_repo/antenv/axon_hooks.py")
            m = importlib.util.module_from_spec(spec)
            spec.loader.exec_module(m)
            sys.modules["antenv.axon_hooks"] = m
            import antenv

            antenv.axon_hooks = m
        kw = dict(trace=True, tmpdir=PROFILE_DIR)
    res = run_bass_kernel_spmd(nc, in_maps, list(range(8)), **kw)
    _cached["last"] = res
    out = np.empty((4, C, N), np.float32)
    for core in range(8):
        b, half = core // 2, core % 2
        out[b][:, half * NH : (half + 1) * NH] = res.results[core]["y"]
    return out.reshape(4, C, 48, 48)


# revision 50
# speedup vs baseline: 1.4213x; 1.0167x over previous
"""Trainium2 Bass kernel for nn_Attention_LR_65249143160949 (cross-attention block).

Sharding: 8 cores = 4 batches x 2 token-halves (1152 tokens each). Each core
computes k/v for its whole batch (cheap MQA single head, duplicated within the
pair) and q/attention/output for its own tokens. The host permutes tokens so
each core's own rows come first -> identical SPMD program, no collectives.

On-chip layout: features on partitions, tokens on the free axis. LayerNorm is
folded into the projections (pre-scaled weights + rank-1 -colsum*mu term).
Attention runs in sim^T layout (keys on partitions, query tokens free): kT is
rs-scaled so softmax is a plain exp; the denominator comes free as a ones
column (col 64) of the 128-col-padded v stationary (row 64 of the out psum).

Engine balance (the v1 kernel was ACT+PE serialized at ~460us):
- exp is split ACT (exact, bf16 out) / DVE (Schraudolph bit-trick: i16 =
  round(sim*184.665 + 16250.4) bitcast bf16, ~3% max err on weights).
- all fp32 broadcast/stat matmuls use float32r (1 cyc/row vs 4) or bf16.
- the 24 per-head reciprocals are batched: den rows DMA-gathered to one
  [8, 512] tile, ONE reciprocal per chunk, then per-head selector matmuls
  (K=8 one-hot-row-of-ones, f32r) broadcast 1/den to 64 partitions.
- GPSIMD (cannot touch PSUM) takes SBUF-only elementwise work: LN1 x^2,
  LN2 bf16 casts + squares, the residual add.
- v/wkv stationaries are 128-col padded/fused so FWL weight loads stay fast.
- x arrives bf16 from the host in token chunks (LN1 starts ~3us in); the
  fp32 x needed only for the residual is DMA'd last.

Walrus quirks handled: one sync-wait per TPB instruction (_split_multi_waits),
no custom DVE ops, engine ops must start at partition 0/32/64/96, GPSIMD has
no PSUM access, f32r tiles must be produced by a rounding op (DVE copy).
"""

import sys

import numpy as np

if "/opt/trn_rl_repo" not in sys.path:
    sys.path.insert(0, "/opt/trn_rl_repo")

C = 512          # channels
N = 2304         # tokens per batch (48*48)
NH = 1152        # tokens per core
HEADS = 8
DH = 64
CTXL = 77
CTXD = 768
JT = 19          # j tiles of 128: 18 img + 1 (ctx 0:77 | null 77 | pad)
JP = JT * 128
CHUNKS = [(0, 512), (512, 512), (1024, 128)]  # (start, len) token chunks
NCH = len(CHUNKS)
KT = 4           # C / 128
EPS = 1e-5

FEXP_S = 184.6649186888274   # 128 / ln(2)
FEXP_C = 16250.4             # 127*128 - 5.6 (minimax-tuned, round-to-nearest)

PROFILE = False
PROFILE_DIR = None

_cached = {}


USE_DVE_EXP = True
USE_GPS_TAIL = True
USE_F32R = True
USE_DENT_DMA = True
ALAG = 2         # attn.v lags exp by this many j-tiles (512-token chunks)


def _exp_engine(cc, hg, jt):
    """Engine for the softmax exp of (chunk cc, head-pair hg, j-tile jt).
    512-token chunks: every 4th j-tile on DVE (bit-trick exp). Returns
    'act' or 'dve'."""
    if not USE_DVE_EXP:
        return "act"
    return "dve" if (jt % 4) == 3 else "act"


def _exp_engine_128(hg, grp):
    """Engine for the packed 4-j-tile exp groups of the 128-token chunk."""
    if not USE_DVE_EXP:
        return "act"
    return "dve" if grp in (1, 3) else "act"


def _split_multi_waits(nc):
    """Walrus codegen supports one sync-wait per TPB instruction (the EVENTS
    struct has a single wait slot). Tile attaches several. Split the extras
    onto same-engine NoOps inserted just before each instruction."""
    import concourse.mybir as mybir

    n = 0
    for fn in nc.m.functions:
        for bb in fn.blocks:
            insts = bb.instructions
            i = 0
            while i < len(insts):
                ins = insts[i]
                si = getattr(ins, "sync_info", None)
                if si is not None and si.on_wait and len(si.on_wait) > 1:
                    waits = list(si.on_wait)
                    for w in waits[:-1]:
                        n += 1
                        nop = mybir.InstNoOp(name=f"WSPLIT-{n}", engine=ins.engine)
                        nop.sync_info = mybir.SyncInfo(on_wait=[w], on_update=[])
                        insts.insert(i, nop)
                        i += 1
                    ins.sync_info = mybir.SyncInfo(
                        on_wait=[waits[-1]], on_update=si.on_update)
                i += 1
    return n


def _build_bass():
    import concourse.bass as bass
    import concourse.mybir as mybir
    import concourse.tile as tile
    from concourse.masks import make_identity
    from contextlib import ExitStack

    F32 = mybir.dt.float32
    F32R = mybir.dt.float32r if USE_F32R else mybir.dt.float32
    BF = mybir.dt.bfloat16
    I16 = mybir.dt.int16
    AF = mybir.ActivationFunctionType
    ALU = mybir.AluOpType

    nc = bass.Bass()
    xbf = nc.declare_dram_parameter("xbf", [C, N], BF, isOutput=False)
    x_own = nc.declare_dram_parameter("x_own", [C, NH], F32, isOutput=False)
    ctxt = nc.declare_dram_parameter("ctxt", [CTXL, CTXD], F32, isOutput=False)
    wq = nc.declare_dram_parameter("wq", [C, C], BF, isOutput=False)
    negcq = nc.declare_dram_parameter("negcq", [1, C], BF, isOutput=False)
    wkvc = nc.declare_dram_parameter("wkvc", [C, 128], BF, isOutput=False)
    ncskv = nc.declare_dram_parameter("ncskv", [1, 128], BF, isOutput=False)
    wctx = nc.declare_dram_parameter("wctx", [CTXD, 2 * DH], F32, isOutput=False)
    bctxk = nc.declare_dram_parameter("bctxk", [DH, 1], F32, isOutput=False)
    bctxv = nc.declare_dram_parameter("bctxv", [DH, 1], F32, isOutput=False)
    nullkt = nc.declare_dram_parameter("nullkt", [DH, 1], F32, isOutput=False)
    nullv = nc.declare_dram_parameter("nullv", [DH, 1], F32, isOutput=False)
    wout = nc.declare_dram_parameter("wout", [128, (HEADS // 2) * C], BF,
                                     isOutput=False)
    selin = nc.declare_dram_parameter("selin", [8, 8 * DH], F32, isOutput=False)
    outgr = nc.declare_dram_parameter("outgr", [1, C], F32, isOutput=False)
    y = nc.declare_dram_parameter("y", [C, NH], F32, isOutput=True)

    with tile.TileContext(nc) as tc, ExitStack() as ctx:
        pconst = ctx.enter_context(tc.tile_pool(name="const", bufs=1))
        pbig = ctx.enter_context(tc.tile_pool(name="big", bufs=1))

        ident = pconst.tile([128, 128], F32)
        make_identity(nc, ident[:])
        ident_bf = pconst.tile([128, 128], BF)
        make_identity(nc, ident_bf[:])
        ones_col = pconst.tile([128, 1], F32)
        nc.vector.memset(ones_col[:], 1.0)
        ones_col_bf = pconst.tile([128, 1], BF)
        nc.vector.memset(ones_col_bf[:], 1.0)
        ones_f = pconst.tile([1, 128], F32)
        nc.vector.memset(ones_f[:], 1.0)
        ones_r = pconst.tile([1, 128], F32R)
        nc.vector.tensor_copy(ones_r[:], ones_f[:])
        eps_col = pconst.tile([128, 1], F32)
        nc.vector.memset(eps_col[:], EPS)
        ones_blk = pconst.tile([128, 64], F32)
        nc.vector.memset(ones_blk[:], 1.0)
        sel_f = pconst.tile([8, 8 * DH], F32)
        sel = pconst.tile([8, 8 * DH], F32R)
        outgr_f = pconst.tile([1, C], F32)
        outgr_r = pconst.tile([1, C], F32R)

        x_sb = pbig.tile([128, KT * NH], F32)        # kt-major; OWN half only
        x_bf = pbig.tile([128, KT * N], BF)
        qT = pbig.tile([128, (HEADS // 2) * NH], BF)  # head-pair blocks
        kT2 = pbig.tile([128, JP], BF)               # rs-scaled keys, both halves
        v_sb = pbig.tile([128, JT * 128], BF)        # per j-tile [v 0:64|ones 64|pad]
        v_sb2 = pbig.tile([128, JT * 128], BF)       # odd-head: [0|ones@32|0|v 64:128]
        projBF = pbig.tile([128, KT * 512], BF)      # bf16 proj (per chunk)
        stats = pbig.tile([128, 40], F32)            # col jt: rs_j (v scaling)
        wout_sb = pbig.tile([128, (HEADS // 2) * C], BF)  # head-pair stacked
        # per-token stat rows on partition 0: mu 0:N | rs N:2N
        # (LN2 reuses per cc: mu2 at cc*CH, rs2 at N+cc*CH, ex2 at 2N+cc*CH)
        rows = pbig.tile([1, 2 * N + NH], F32)
        rows_bf = pbig.tile([1, N], BF)
        rows_r = pbig.tile([1, N], F32R)             # f32r copy of rs row
        R_RS, R_SC = N, 2 * N

        nc.sync.dma_start(sel_f[:], selin[:, :])
        nc.sync.dma_start(outgr_f[:], outgr[:, :])
        nc.vector.tensor_copy(sel[:], sel_f[:])
        nc.vector.tensor_copy(outgr_r[:], outgr_f[:])
        nc.sync.dma_start(wout_sb[:], wout[:, :])

        with tc.tile_pool(name="load", bufs=1) as pload, \
             tc.tile_pool(name="x2p", bufs=2) as px2, \
             tc.tile_pool(name="pss", bufs=2, space="PSUM") as pss:
            # psum tags: b1 [<=64,384]x2, bS [128,<=512]x4, bT [128,128]x2
            wq_sb = pload.tile([128, KT * C], BF)
            wkv_sb = pload.tile([128, KT * 128], BF)
            wctx_sb = pload.tile([128, CTXD], F32)
            negcq_sb = pload.tile([1, C], BF)
            ncskv_sb = pload.tile([1, 128], BF)
            bctxk_sb = pload.tile([DH, 1], F32)
            bctxv_sb = pload.tile([DH, 1], F32)
            vT = pload.tile([64, N], BF)
            ck_sb = pload.tile([64, CTXL], F32)
            cv_sb = pload.tile([64, CTXL + 1], F32)
            nullk_st = pload.tile([DH, 1], F32)
            nullv_st = pload.tile([DH, 1], F32)
            ctx_sb = pload.tile([CTXL, CTXD], F32)
            ctxnT = pload.tile([128, 6 * CTXL], F32)
            ex2 = pload.tile([1, N], F32)
            kk = pload.tile([128, 512], F32)

            # PE warmup: ~4us of back-to-back junk matmuls while the DMAs
            # stream in, so the tensor engine reaches its fast pstate
            # before LN1's first real matmul.
            ps_wu = pss.tile([128, 128], F32, tag="bT")
            for _ in range(24):
                nc.tensor.matmul(ps_wu[:, :], ident_bf[:], ident_bf[:],
                                 start=True, stop=True)

            # Two HWDGE queues: x_bf chunks go on the ACT queue so LN1 can
            # start while the SP queue streams ctx/weights.
            xbf_v = x_bf[:].rearrange("p (k n) -> p k n", k=KT)
            xbf_d = xbf[:].rearrange("(k p) n -> p k n", p=128)
            for ch in range(6):
                a, b = ch * 384, (ch + 1) * 384
                nc.scalar.dma_start(xbf_v[:, :, a:b], xbf_d[:, :, a:b])
            nc.sync.dma_start(ctx_sb[:], ctxt[:, :])
            nc.sync.dma_start(wctx_sb[:].rearrange("p (k n) -> p k n", k=6),
                              wctx[:].rearrange("(k p) n -> p k n", p=128))
            nc.sync.dma_start(bctxk_sb[:], bctxk[:, :])
            nc.sync.dma_start(bctxv_sb[:], bctxv[:, :])
            nc.sync.dma_start(nullk_st[:], nullkt[:, :])
            nc.sync.dma_start(nullv_st[:], nullv[:, :])
            nc.sync.dma_start(wkv_sb[:].rearrange("p (k n) -> p k n", k=KT),
                              wkvc[:].rearrange("(k p) n -> p k n", p=128))
            nc.sync.dma_start(ncskv_sb[:], ncskv[:, :])
            nc.scalar.dma_start(wq_sb[:].rearrange("p (k n) -> p k n", k=KT),
                                wq[:].rearrange("(k p) n -> p k n", p=128))
            nc.sync.dma_start(negcq_sb[:], negcq[:, :])
            x_v = x_sb[:].rearrange("p (k n) -> p k n", k=KT)
            nc.sync.dma_start(x_v[:, :, 0:NH],
                              x_own[:].rearrange("(k p) n -> p k n", p=128))

            # ---- LN1 stats (row form): mu, then rs = exp(-0.5 ln(var+eps)) ----
            for ch in range(6):
                sl = slice(ch * 384, (ch + 1) * 384)
                ps_r1 = pss.tile([64, 384], F32, tag="b1")
                for kt in range(KT):
                    nc.tensor.matmul(
                        ps_r1[0:1, :], ones_col_bf[:],
                        x_bf[:, kt * N + ch * 384 : kt * N + (ch + 1) * 384],
                        start=(kt == 0), stop=(kt == KT - 1))
                nc.scalar.mul(rows[0:1, sl], ps_r1[0:1, :], 1.0 / C)
                nc.vector.tensor_copy(rows_bf[0:1, sl], rows[0:1, sl])
            for ch in range(6):
                x2 = px2.tile([128, KT * 384], BF, tag="x2")
                ps_r2 = pss.tile([64, 384], F32, tag="b1")
                for kt in range(KT):
                    xs = x_bf[:, kt * N + ch * 384 : kt * N + (ch + 1) * 384]
                    nc.vector.tensor_mul(x2[:, kt * 384 : (kt + 1) * 384], xs, xs)
                    nc.tensor.matmul(
                        ps_r2[0:1, :], ones_col_bf[:],
                        x2[:, kt * 384 : (kt + 1) * 384],
                        start=(kt == 0), stop=(kt == KT - 1))
                nc.scalar.mul(ex2[0:1, ch * 384 : (ch + 1) * 384],
                              ps_r2[0:1, :], 1.0 / C)
            for ch in range(6):
                a, b = R_RS + ch * 384, R_RS + (ch + 1) * 384
                mu = rows[0:1, ch * 384 : (ch + 1) * 384]
                nc.vector.tensor_mul(rows[0:1, a:b], mu, mu)
                nc.vector.tensor_sub(rows[0:1, a:b],
                                     ex2[0:1, ch * 384 : (ch + 1) * 384],
                                     rows[0:1, a:b])
                nc.scalar.activation(rows[0:1, a:b], rows[0:1, a:b], AF.Ln,
                                     bias=eps_col[0:1, :])
                nc.scalar.activation(rows[0:1, a:b], rows[0:1, a:b], AF.Exp,
                                     scale=-0.5)
            nc.vector.tensor_copy(rows_r[0:1, :], rows[0:1, R_RS : R_RS + N])
            # ---- context: LN (layout A, bn_stats) + k/v projection ----
            cstat = pload.tile([CTXL, 3, 6], F32)
            for sg in range(3):
                nc.vector.bn_stats(cstat[:, sg, :],
                                   ctx_sb[:, sg * 256 : (sg + 1) * 256])
            cmv = pload.tile([CTXL, 2], F32)
            nc.vector.bn_aggr(cmv[:], cstat[:])
            nc.scalar.activation(cmv[:, 1:2], cmv[:, 1:2], AF.Ln,
                                 bias=eps_col[0:CTXL, :])
            nc.scalar.activation(cmv[:, 1:2], cmv[:, 1:2], AF.Exp, scale=-0.5)
            nc.vector.tensor_scalar(
                out=ctx_sb[:], in0=ctx_sb[:],
                scalar1=cmv[:, 0:1], scalar2=cmv[:, 1:2],
                op0=ALU.subtract, op1=ALU.mult)
            for kt in range(6):
                ps_ct = pss.tile([128, 128], F32, tag="bT")
                nc.tensor.transpose(ps_ct[:, 0:CTXL],
                                    ctx_sb[:, kt * 128 : (kt + 1) * 128],
                                    ident[:CTXL, :CTXL])
                nc.vector.tensor_copy(ctxnT[:, kt * CTXL : (kt + 1) * CTXL],
                                      ps_ct[:, 0:CTXL])
            ps_ck = pss.tile([64, 384], F32, tag="b1")
            ps_cv = pss.tile([64, 384], F32, tag="b1")
            for kt in range(6):
                nc.tensor.matmul(ps_ck[:, 0:CTXL],
                                 wctx_sb[:, kt * 128 : kt * 128 + DH],
                                 ctxnT[:, kt * CTXL : (kt + 1) * CTXL],
                                 start=(kt == 0), stop=(kt == 5))
                nc.tensor.matmul(ps_cv[:, 0:CTXL],
                                 wctx_sb[:, kt * 128 + DH : (kt + 1) * 128],
                                 ctxnT[:, kt * CTXL : (kt + 1) * CTXL],
                                 start=(kt == 0), stop=(kt == 5))
            nc.vector.tensor_scalar_add(ck_sb[:], ps_ck[:, 0:CTXL], bctxk_sb[:])
            nc.vector.tensor_scalar_add(cv_sb[:, 0:CTXL], ps_cv[:, 0:CTXL],
                                        bctxv_sb[:])
            nc.vector.tensor_copy(cv_sb[:, CTXL : CTXL + 1], nullv_st[:])

            # rs as per-partition columns (v scaling)
            for jt in range(18):
                ps_c = pss.tile([128, 128], F32, tag="bT")
                nc.tensor.matmul(ps_c[:, 0:1],
                                 rows[0:1, R_RS + jt * 128 : R_RS + (jt + 1) * 128],
                                 ones_col[0:1, :])
                nc.vector.tensor_copy(stats[:, jt : jt + 1], ps_c[:, 0:1])

            # ---- j-tile 18: [ctx 0:77 | null 77 | pad 78:128] ----
            nc.gpsimd.memset(v_sb[:], 0.0)
            nc.gpsimd.memset(v_sb2[:], 0.0)
            nc.gpsimd.memset(kT2[0:64, 18 * 128 : JP], 0.0)
            nc.vector.tensor_copy(kT2[0:64, 18 * 128 : 18 * 128 + CTXL], ck_sb[:])
            nc.vector.tensor_copy(kT2[0:64, 18 * 128 + CTXL : 18 * 128 + CTXL + 1],
                                  nullk_st[:])
            VB = 18 * 128
            ps_cvt = pss.tile([128, 128], F32, tag="bT")
            nc.tensor.transpose(ps_cvt[0 : CTXL + 1, 0:64], cv_sb[:],
                                ident[:64, :64])
            nc.vector.tensor_copy(v_sb[0 : CTXL + 1, VB : VB + DH],
                                  ps_cvt[0 : CTXL + 1, 0:64])
            nc.vector.memset(v_sb[0 : CTXL + 1, VB + DH : VB + DH + 1], 1.0)
            nc.vector.tensor_copy(v_sb2[0 : CTXL + 1, VB + 64 : VB + 128],
                                  ps_cvt[0 : CTXL + 1, 0:64])
            nc.vector.memset(v_sb2[0 : CTXL + 1, VB + 32 : VB + 33], 1.0)

            # ---- kv projection (all tokens; LN folded; k into kT2 top half) ----
            KV_CHUNKS = [(0, 512), (512, 512), (1024, 512), (1536, 512), (2048, 256)]
            for kva, kvl in KV_CHUNKS:
                sl = slice(kva, kva + kvl)
                ps_kv = pss.tile([128, 512], F32, tag="bS")
                for kt in range(KT):
                    xs = x_bf[:, kt * N + kva : kt * N + kva + kvl]
                    nc.tensor.matmul(ps_kv[:, 0:kvl],
                                     wkv_sb[:, kt * 128 : (kt + 1) * 128],
                                     xs, start=(kt == 0), stop=False)
                nc.tensor.matmul(ps_kv[:, 0:kvl], ncskv_sb[:], rows_bf[0:1, sl],
                                 start=False, stop=True)
                ps_bc = pss.tile([128, 512], F32, tag="bS")
                nc.tensor.matmul(ps_bc[:, 0:kvl], ones_r[0:1, 0:128],
                                 rows_r[0:1, sl])
                nc.vector.tensor_copy(kk[64:128, 0:kvl], ps_kv[64:128, 0:kvl])
                nc.vector.tensor_mul(kT2[64:128, sl], kk[64:128, 0:kvl],
                                     ps_bc[64:128, 0:kvl])
                nc.vector.tensor_copy(vT[:, sl], ps_kv[0:64, 0:kvl])

            # ---- v tiles: transpose + rs scale + ones col (both layouts) ----
            for jt in range(18):
                ps_vt = pss.tile([128, 128], BF, tag="bT")
                nc.tensor.transpose(ps_vt[:, 0:64], vT[:, jt * 128 : (jt + 1) * 128],
                                    ident_bf[:64, :64])
                vb = jt * 128
                nc.vector.tensor_scalar_mul(v_sb[:, vb : vb + DH], ps_vt[:, 0:64],
                                            stats[:, jt : jt + 1])
                nc.vector.memset(v_sb[:, vb + DH : vb + DH + 1], 1.0)
                nc.vector.tensor_scalar_mul(v_sb2[:, vb + 64 : vb + 128],
                                            ps_vt[:, 0:64],
                                            stats[:, jt : jt + 1])
                nc.vector.memset(v_sb2[:, vb + 32 : vb + 33], 1.0)

            # ---- duplicate kT to partitions 0:64 (sbuf->sbuf DMA) ----
            nc.sync.dma_start(kT2[0:64, 0 : 18 * 128], kT2[64:128, 0 : 18 * 128])
            nc.sync.dma_start(kT2[64:128, 18 * 128 : JP],
                              kT2[0:64, 18 * 128 : JP])

            # ---- q projection (head pairs; LN + 1/sqrt(dh) folded) ----
            for a0, ln in CHUNKS:
                sl = slice(a0, a0 + ln)
                ps_rs = pss.tile([128, 512], F32, tag="bS")
                nc.tensor.matmul(ps_rs[:, 0:ln], ones_r[0:1, :],
                                 rows_r[0:1, sl])
                rs_b = px2.tile([128, 512], F32, tag="rsb")
                nc.vector.tensor_copy(rs_b[:, 0:ln], ps_rs[:, 0:ln])
                for hg in range(HEADS // 2):
                    ps_q = pss.tile([128, 512], F32, tag="bS")
                    for kt in range(KT):
                        nc.tensor.matmul(
                            ps_q[:, 0:ln],
                            wq_sb[:, kt * C + hg * 128 : kt * C + (hg + 1) * 128],
                            x_bf[:, kt * N + a0 : kt * N + a0 + ln],
                            start=(kt == 0), stop=False)
                    nc.tensor.matmul(ps_q[:, 0:ln],
                                     negcq_sb[0:1, hg * 128 : (hg + 1) * 128],
                                     rows_bf[0:1, sl], start=False, stop=True)
                    nc.vector.tensor_mul(
                        qT[:, hg * NH + a0 : hg * NH + a0 + ln],
                        ps_q[:, 0:ln], rs_b[:, 0:ln])

        # ========= attention + output + LN2 + residual, per chunk =========
        # Per (chunk, head-pair): sim pair (row-tiled concurrent) -> exp on
        # ACT or DVE (bit-trick) -> attn.v pair lagging one j-tile. The tail
        # (recip batch, norm, out-proj, LN2, y) is deferred one stage so it
        # never head-of-line-blocks the PE queue.
        with tc.tile_pool(name="attb", bufs=4) as patb, \
             tc.tile_pool(name="atti", bufs=4) as pati, \
             tc.tile_pool(name="outp", bufs=6) as pout, \
             tc.tile_pool(name="pocp", bufs=6) as ppoc, \
             tc.tile_pool(name="denp", bufs=2) as pden, \
             tc.tile_pool(name="recp", bufs=2) as prec, \
             tc.tile_pool(name="rowp", bufs=1) as prow, \
             tc.tile_pool(name="p2p", bufs=1) as pp2, \
             tc.tile_pool(name="yp", bufs=2) as pyt, \
             tc.tile_pool(name="psatt", bufs=2, space="PSUM") as psA, \
             tc.tile_pool(name="psacc", bufs=4, space="PSUM") as psB:

            def at_tile(engine):
                if engine == "act":
                    return patb.tile([128, 1024], BF, tag="atb", name="atb")
                return pati.tile([128, 1024], I16, tag="ati", name="ati")

            def do_exp(engine, at, ps_s, lo, hi):
                if engine == "act":
                    nc.scalar.activation(at[:, lo:hi], ps_s[:, lo:hi], AF.Exp)
                else:
                    nc.vector.tensor_scalar(
                        out=at[:, lo:hi], in0=ps_s[:, lo:hi],
                        scalar1=FEXP_S, scalar2=FEXP_C,
                        op0=ALU.mult, op1=ALU.add)

            def at_slice(at_eng, lo, hi):
                at, eng = at_eng
                ap = at[:, lo:hi]
                return ap if eng == "act" else ap.bitcast(BF)

            def run_hg_512(cc, hg, a0, ln, dent, pocs):
                po0 = psB.tile([128, 512], F32, tag="po")
                po1 = psB.tile([128, 512], F32, tag="po")
                po = [po0, po1]
                q0 = qT[0:64, hg * NH + a0 : hg * NH + a0 + ln]
                q1 = qT[64:128, hg * NH + a0 : hg * NH + a0 + ln]
                ats = [None] * JT
                for jt in range(JT):
                    ps_s = psA.tile([128, 1024], F32, tag="sim")
                    nc.tensor.matmul(ps_s[:, 0:ln],
                                     kT2[0:64, jt * 128 : (jt + 1) * 128],
                                     q0, start=True, stop=True)
                    nc.tensor.matmul(ps_s[:, 512 : 512 + ln],
                                     kT2[64:128, jt * 128 : (jt + 1) * 128],
                                     q1, start=True, stop=True)
                    eng = _exp_engine(cc, hg, jt)
                    at = at_tile(eng)
                    do_exp(eng, at, ps_s, 0, 1024)
                    ats[jt] = (at, eng)
                    if jt >= ALAG:
                        j0 = jt - ALAG
                        nc.tensor.matmul(po[0][:, 0:ln],
                                         v_sb[:, j0 * 128 : (j0 + 1) * 128],
                                         at_slice(ats[j0], 0, ln),
                                         start=(j0 == 0), stop=False)
                        nc.tensor.matmul(po[1][:, 0:ln],
                                         v_sb2[:, j0 * 128 : (j0 + 1) * 128],
                                         at_slice(ats[j0], 512, 512 + ln),
                                         start=(j0 == 0), stop=False)
                        ats[j0] = None
                for j0 in range(JT - ALAG, JT):
                    nc.tensor.matmul(po[0][:, 0:ln],
                                     v_sb[:, j0 * 128 : (j0 + 1) * 128],
                                     at_slice(ats[j0], 0, ln),
                                     start=(j0 == 0), stop=(j0 == JT - 1))
                    nc.tensor.matmul(po[1][:, 0:ln],
                                     v_sb2[:, j0 * 128 : (j0 + 1) * 128],
                                     at_slice(ats[j0], 512, 512 + ln),
                                     start=(j0 == 0), stop=(j0 == JT - 1))
                _extract(hg, ln, po, dent, pocs)

            def run_hg_128(cc, hg, a0, ln, dent, pocs):
                # 19 j-tiles packed 4-per-psum-tile; slot g: q0 at col 128g
                # (bank A), q1 at 512+128g (bank B) so the row-tiled
                # concurrent sim pair never co-writes one psum bank.
                po0 = psB.tile([128, 512], F32, tag="po")
                po1 = psB.tile([128, 512], F32, tag="po")
                po = [po0, po1]
                q0 = qT[0:64, hg * NH + a0 : hg * NH + a0 + ln]
                q1 = qT[64:128, hg * NH + a0 : hg * NH + a0 + ln]
                GRP = [(0, 4), (4, 4), (8, 4), (12, 4), (16, 3)]
                ats = [None] * len(GRP)
                for gi, (jt0, ng) in enumerate(GRP):
                    ps_s = psA.tile([128, 1024], F32, tag="sim")
                    for g in range(ng):
                        jt = jt0 + g
                        nc.tensor.matmul(
                            ps_s[:, 128 * g : 128 * g + ln],
                            kT2[0:64, jt * 128 : (jt + 1) * 128],
                            q0, start=True, stop=True)
                        nc.tensor.matmul(
                            ps_s[:, 512 + 128 * g : 512 + 128 * g + ln],
                            kT2[64:128, jt * 128 : (jt + 1) * 128],
                            q1, start=True, stop=True)
                    eng = _exp_engine_128(hg, gi)
                    at = at_tile(eng)
                    if ng == 4:
                        do_exp(eng, at, ps_s, 0, 1024)
                    else:
                        do_exp(eng, at, ps_s, 0, 128 * ng)
                        do_exp(eng, at, ps_s, 512, 512 + 128 * ng)
                    ats[gi] = (at, eng)
                    if gi > 0:
                        _attnv_128(po, ats[gi - 1], GRP[gi - 1], ln,
                                   start=(gi == 1), stop=False)
                        ats[gi - 1] = None
                _attnv_128(po, ats[-1], GRP[-1], ln, start=False, stop=True)
                _extract(hg, ln, po, dent, pocs)

            def _attnv_128(po, at_eng, grp, ln, start, stop):
                jt0, ng = grp
                for g in range(ng):
                    jt = jt0 + g
                    last = stop and (g == ng - 1)
                    nc.tensor.matmul(po[0][:, 0:ln],
                                     v_sb[:, jt * 128 : (jt + 1) * 128],
                                     at_slice(at_eng, 128 * g, 128 * g + ln),
                                     start=(start and g == 0), stop=last)
                    nc.tensor.matmul(po[1][:, 0:ln],
                                     v_sb2[:, jt * 128 : (jt + 1) * 128],
                                     at_slice(at_eng, 512 + 128 * g,
                                              512 + 128 * g + ln),
                                     start=(start and g == 0), stop=last)

            def _extract(hg, ln, po, dent, pocs):
                # stacked ot: even head out -> partitions 0:64 (from po0),
                # odd head out -> 64:128 (po1; its v sat cols 64:128).
                # dens: po0 row 64 (even), po1 row 32 (odd).
                otp = ppoc.tile([128, 512], BF, tag="poc")
                nc.vector.tensor_copy(otp[0:64, 0:ln], po[0][0:64, 0:ln])
                nc.vector.tensor_copy(otp[64:128, 0:ln], po[1][64:128, 0:ln])
                stg = ppoc.tile([65, 512], BF, tag="stg")
                nc.vector.tensor_copy(stg[64:65, 0:ln], po[0][64:65, 0:ln])
                nc.vector.tensor_copy(stg[32:33, 0:ln], po[1][32:33, 0:ln])
                nc.sync.dma_start(dent[2 * hg : 2 * hg + 1, 0:ln],
                                  stg[64:65, 0:ln])
                nc.sync.dma_start(dent[2 * hg + 1 : 2 * hg + 2, 0:ln],
                                  stg[32:33, 0:ln])
                pocs.append(otp)

            def tail_phases(cc, a0, ln, dent, pocs):
                ots = []
                rec8r = prec.tile([8, 512], F32R, tag="rec")

                def norm_pair(hg):
                    ps_rb = psB.tile([128, 512], F32, tag="po")
                    nc.tensor.matmul(ps_rb[:, 0:ln],
                                     sel[:, hg * 128 : (hg + 1) * 128],
                                     rec8r[:, 0:ln])
                    ot = pout.tile([128, 512], BF, tag="ot")
                    nc.vector.tensor_mul(ot[:, 0:ln], pocs[hg][:, 0:ln],
                                         ps_rb[:, 0:ln])
                    ots.append(ot)

                def ph_norm_a():
                    dentf = prec.tile([8, 512], F32, tag="dentf")
                    nc.vector.tensor_copy(dentf[:, 0:ln], dent[:, 0:ln])
                    rec8 = prec.tile([8, 512], F32, tag="rec8")
                    nc.vector.reciprocal(rec8[:, 0:ln], dentf[:, 0:ln])
                    nc.vector.tensor_copy(rec8r[:, 0:ln], rec8[:, 0:ln])
                    norm_pair(0)
                    norm_pair(1)

                def ph_norm_b():
                    norm_pair(2)
                    norm_pair(3)

                def ph_proj():
                    for ct in range(KT):
                        ps_p = psB.tile([128, 512], F32, tag="po")
                        for hg in range(HEADS // 2):
                            nc.tensor.matmul(
                                ps_p[:, 0:ln],
                                wout_sb[:, hg * C + ct * 128 : hg * C + (ct + 1) * 128],
                                ots[hg][:, 0:ln],
                                start=(hg == 0), stop=(hg == HEADS // 2 - 1))
                        nc.vector.tensor_copy(
                            projBF[:, ct * 512 : ct * 512 + ln],
                            ps_p[:, 0:ln])

                def ph_ln2y():
                    ra, rb2 = R_RS + a0, R_RS + a0 + ln
                    sca, scb = R_SC + a0, R_SC + a0 + ln
                    ps_m2 = psB.tile([128, 512], F32, tag="po")
                    for ct in range(KT):
                        nc.tensor.matmul(ps_m2[0:1, 0:ln], ones_col_bf[:],
                                         projBF[:, ct * 512 : ct * 512 + ln],
                                         start=(ct == 0), stop=(ct == KT - 1))
                    nc.scalar.mul(rows[0:1, a0 : a0 + ln], ps_m2[0:1, 0:ln], 1.0 / C)
                    p2 = pp2.tile([128, KT * 512], BF, tag="p2")
                    ps_q2 = psB.tile([128, 512], F32, tag="po")
                    for ct in range(KT):
                        pslc = projBF[:, ct * 512 : ct * 512 + ln]
                        eng2 = nc.gpsimd if USE_GPS_TAIL else nc.vector
                        eng2.tensor_mul(p2[:, ct * 512 : ct * 512 + ln],
                                        pslc, pslc)
                        nc.tensor.matmul(ps_q2[0:1, 0:ln], ones_col_bf[:],
                                         p2[:, ct * 512 : ct * 512 + ln],
                                         start=(ct == 0), stop=(ct == KT - 1))
                    nc.scalar.mul(rows[0:1, sca:scb], ps_q2[0:1, 0:ln], 1.0 / C)
                    nc.vector.tensor_mul(rows[0:1, ra:rb2], rows[0:1, a0 : a0 + ln],
                                         rows[0:1, a0 : a0 + ln])
                    nc.vector.tensor_sub(rows[0:1, ra:rb2], rows[0:1, sca:scb],
                                         rows[0:1, ra:rb2])
                    nc.scalar.activation(rows[0:1, ra:rb2], rows[0:1, ra:rb2],
                                         AF.Ln, bias=eps_col[0:1, :])
                    nc.scalar.activation(rows[0:1, ra:rb2], rows[0:1, ra:rb2],
                                         AF.Exp, scale=-0.5)
                    # mrs = mu * rs ; f32r rows for the g*rs / g*mu*rs bcasts
                    rs2r = prow.tile([1, 2 * 512], F32R, tag="r2")
                    nc.vector.tensor_mul(rows[0:1, sca:scb], rows[0:1, a0 : a0 + ln],
                                         rows[0:1, ra:rb2])
                    nc.vector.tensor_copy(rs2r[0:1, 0:ln], rows[0:1, ra:rb2])
                    nc.vector.tensor_copy(rs2r[0:1, 512 : 512 + ln],
                                          rows[0:1, sca:scb])
                    ps_gb = psB.tile([128, 512], F32, tag="po")
                    ps_gm = psB.tile([128, 512], F32, tag="po")
                    for ct in range(KT):
                        gsl = outgr_r[0:1, ct * 128 : (ct + 1) * 128]
                        nc.tensor.matmul(ps_gb[:, 0:ln], gsl, rs2r[0:1, 0:ln])
                        nc.tensor.matmul(ps_gm[:, 0:ln], gsl,
                                         rs2r[0:1, 512 : 512 + ln])
                        yt = pyt.tile([128, 512], F32, tag="yt")
                        pslice = projBF[:, ct * 512 : ct * 512 + ln]
                        nc.vector.tensor_mul(yt[:, 0:ln], pslice, ps_gb[:, 0:ln])
                        nc.vector.tensor_sub(yt[:, 0:ln], yt[:, 0:ln],
                                             ps_gm[:, 0:ln])
                        eng3 = nc.gpsimd if USE_GPS_TAIL else nc.vector
                        eng3.tensor_add(
                            yt[:, 0:ln], yt[:, 0:ln],
                            x_sb[:, ct * NH + a0 : ct * NH + a0 + ln])
                        nc.sync.dma_start(
                            y[ct * 128 : (ct + 1) * 128, a0 : a0 + ln],
                            yt[:, 0:ln])

                return [ph_norm_a, ph_norm_b, ph_proj, ph_ln2y]

            phases = []
            for cc, (a0, ln) in enumerate(CHUNKS):
                pocs = []
                dent = pden.tile([8, 512], BF, tag="dent")
                for hg in range(HEADS // 2):
                    if ln == 512:
                        run_hg_512(cc, hg, a0, ln, dent, pocs)
                    else:
                        run_hg_128(cc, hg, a0, ln, dent, pocs)
                    if phases:
                        phases.pop(0)()
                phases = tail_phases(cc, a0, ln, dent, pocs)
            for ph in phases:
                ph()
    _split_multi_waits(nc)
    return nc


def _prep_inputs(x, context, norm_gamma, null_kv, Wq, Wkv, ctx_ln_g, ctx_ln_b,
                 Wctx, bctx, Wout, out_ln_g):
    import ml_dtypes
    bf = ml_dtypes.bfloat16
    f = np.float32
    x = np.asarray(x, f).reshape(4, C, N)
    context = np.asarray(context, f)
    g = np.asarray(norm_gamma, f)
    scale = 1.0 / np.sqrt(DH)
    wq_h = (g[:, None] * np.asarray(Wq, f)) * scale
    negcq_h = -wq_h.sum(0, dtype=np.float64).astype(f)[None, :]
    wkv_h = g[:, None] * np.asarray(Wkv, f)
    # combined stationary: [v | k] so k lands on psum rows 64:128
    wkvc_h = np.concatenate([wkv_h[:, DH:], wkv_h[:, :DH]], axis=1)
    ncs = -wkv_h.sum(0, dtype=np.float64).astype(f)
    ncskv_h = np.concatenate([ncs[DH:], ncs[:DH]])[None, :]
    wctx_h = np.asarray(ctx_ln_g, f)[:, None] * np.asarray(Wctx, f)
    bctx_h = (np.asarray(bctx, f) + np.asarray(ctx_ln_b, f) @ np.asarray(Wctx, f))
    null = np.asarray(null_kv, f)
    W_o = np.asarray(Wout, f)
    # head-pair stacked: rows 0:64 = even head dims, 64:128 = odd head dims
    wout_b = np.concatenate(
        [np.concatenate([W_o[2 * hg * DH : (2 * hg + 1) * DH, :],
                         W_o[(2 * hg + 1) * DH : (2 * hg + 2) * DH, :]], axis=0)
         for hg in range(HEADS // 2)], axis=1)
    # pair selector: block hg, cols 0:64 -> row 2hg, cols 64:128 -> row 2hg+1
    sel_h = np.zeros((8, 4 * 128), f)
    for hg in range(4):
        sel_h[2 * hg, hg * 128 : hg * 128 + 64] = 1.0
        sel_h[2 * hg + 1, hg * 128 + 64 : (hg + 1) * 128] = 1.0

    shared = {
        "wq": np.ascontiguousarray(wq_h).astype(bf),
        "negcq": negcq_h.astype(bf),
        "wkvc": np.ascontiguousarray(wkvc_h).astype(bf),
        "ncskv": np.ascontiguousarray(ncskv_h).astype(bf),
        "wctx": np.ascontiguousarray(wctx_h),
        "bctxk": np.ascontiguousarray(bctx_h[:DH, None]),
        "bctxv": np.ascontiguousarray(bctx_h[DH:, None]),
        "nullkt": np.ascontiguousarray(null[0][:, None]),
        "nullv": np.ascontiguousarray(null[1][:, None]),
        "wout": np.ascontiguousarray(wout_b).astype(bf),
        "selin": sel_h,
        "outgr": np.ascontiguousarray(np.asarray(out_ln_g, f)[None, :]),
    }
    in_maps = []
    for core in range(8):
        b, half = core // 2, core % 2
        m = dict(shared)
        xo = x[b][:, half * NH : (half + 1) * NH]
        xt = x[b][:, (1 - half) * NH : (2 - half) * NH]
        m["x_own"] = np.ascontiguousarray(xo)
        m["xbf"] = np.ascontiguousarray(
            np.concatenate([xo, xt], axis=1)).astype(bf)
        m["ctxt"] = np.ascontiguousarray(context[b])
        in_maps.append(m)
    return in_maps


_LDW_OPT = [False]


def _patch_ldw_opt():
    import concourse.bass_utils as bu
    if getattr(bu, "_ldwopt_patched", False):
        return
    orig = bu.run_command

    def run2(cmd, **kw):
        if _LDW_OPT[0]:
            cmd = [c.replace("--enable-ldw-opt=false", "--enable-ldw-opt=true")
                   for c in cmd]
        return orig(cmd, **kw)

    bu.run_command = run2
    bu._ldwopt_patched = True


def kernel(**inputs):
    from concourse.bass_utils import run_bass_kernel_spmd
    _patch_ldw_opt()

    if "nc" not in _cached:
        _cached["nc"] = _build_bass()
    nc = _cached["nc"]
    in_maps = _prep_inputs(**inputs)
    kw = {}
    if PROFILE:
        import importlib.util

        if "antenv.axon_hooks" not in sys.modules:
            spec = importlib.util.spec_from_file_location(
                "antenv.axon_hooks", "/opt/trn_rl_repo/antenv/axon_hooks.py")
            m = importlib.util.module_from_spec(spec)
            spec.loader.exec_module(m)
            sys.modules["antenv.axon_hooks"] = m
            import antenv

            antenv.axon_hooks = m
        kw = dict(trace=True, tmpdir=PROFILE_DIR)
    res = run_bass_kernel_spmd(nc, in_maps, list(range(8)), **kw)
    _cached["last"] = res
    out = np.empty((4, C, N), np.float32)
    for core in range(8):
        b, half = core // 2, core % 2
        out[b][:, half * NH : (half + 1) * NH] = res.results[core]["y"]
    return out.reshape(4, C, 48, 48)


# revision 62
# speedup vs baseline: 1.4352x; 1.0098x over previous
"""Trainium2 Bass kernel for nn_Attention_LR_65249143160949 (cross-attention block).

Sharding: 8 cores = 4 batches x 2 token-halves (1152 tokens each). Each core
computes k/v for its whole batch (cheap MQA single head, duplicated within the
pair) and q/attention/output for its own tokens. The host permutes tokens so
each core's own rows come first -> identical SPMD program, no collectives.

On-chip layout: features on partitions, tokens on the free axis. LayerNorm is
folded into the projections (pre-scaled weights + rank-1 -colsum*mu term).
Attention runs in sim^T layout (keys on partitions, query tokens free): kT is
rs-scaled so softmax is a plain exp; the denominator comes free as a ones
column (col 64) of the 128-col-padded v stationary (row 64 of the out psum).

Engine balance (the v1 kernel was ACT+PE serialized at ~460us):
- exp is split ACT (exact, bf16 out) / DVE (Schraudolph bit-trick: i16 =
  round(sim*184.665 + 16250.4) bitcast bf16, ~3% max err on weights).
- all fp32 broadcast/stat matmuls use float32r (1 cyc/row vs 4) or bf16.
- the 24 per-head reciprocals are batched: den rows DMA-gathered to one
  [8, 512] tile, ONE reciprocal per chunk, then per-head selector matmuls
  (K=8 one-hot-row-of-ones, f32r) broadcast 1/den to 64 partitions.
- GPSIMD (cannot touch PSUM) takes SBUF-only elementwise work: LN1 x^2,
  LN2 bf16 casts + squares, the residual add.
- v/wkv stationaries are 128-col padded/fused so FWL weight loads stay fast.
- x arrives bf16 from the host in token chunks (LN1 starts ~3us in); the
  fp32 x needed only for the residual is DMA'd last.

Walrus quirks handled: one sync-wait per TPB instruction (_split_multi_waits),
no custom DVE ops, engine ops must start at partition 0/32/64/96, GPSIMD has
no PSUM access, f32r tiles must be produced by a rounding op (DVE copy).
"""

import sys

import numpy as np

if "/opt/trn_rl_repo" not in sys.path:
    sys.path.insert(0, "/opt/trn_rl_repo")

C = 512          # channels
N = 2304         # tokens per batch (48*48)
NH = 1152        # tokens per core
HEADS = 8
DH = 64
CTXL = 77
CTXD = 768
JT = 19          # j tiles of 128: 18 img + 1 (ctx 0:77 | null 77 | pad)
JP = JT * 128
CHUNKS = [(0, 512), (512, 512), (1024, 128)]  # (start, len) token chunks
NCH = len(CHUNKS)
KT = 4           # C / 128
EPS = 1e-5

FEXP_S = 184.6649186888274   # 128 / ln(2)
FEXP_C = 16250.4             # 127*128 - 5.6 (minimax-tuned, round-to-nearest)

PROFILE = False
PROFILE_DIR = None

_cached = {}


USE_DVE_EXP = True
USE_GPS_TAIL = True
USE_F32R = True
USE_DENT_DMA = True
ALAG = 4         # attn.v lags exp by this many j-tiles (512-token chunks)


def _exp_engine(cc, hg, jt):
    """Engine for the softmax exp of (chunk cc, head-pair hg, j-tile jt).
    512-token chunks: every 4th j-tile on DVE (bit-trick exp). Returns
    'act' or 'dve'."""
    if not USE_DVE_EXP:
        return "act"
    return "dve" if (jt % 5) == 4 else "act"


def _exp_engine_128(hg, grp):
    """Engine for the packed 4-j-tile exp groups of the 128-token chunk."""
    if not USE_DVE_EXP:
        return "act"
    return "dve" if grp in (1, 3) else "act"


def _split_multi_waits(nc):
    """Walrus codegen supports one sync-wait per TPB instruction (the EVENTS
    struct has a single wait slot). Tile attaches several. Split the extras
    onto same-engine NoOps inserted just before each instruction."""
    import concourse.mybir as mybir

    n = 0
    for fn in nc.m.functions:
        for bb in fn.blocks:
            insts = bb.instructions
            i = 0
            while i < len(insts):
                ins = insts[i]
                si = getattr(ins, "sync_info", None)
                if si is not None and si.on_wait and len(si.on_wait) > 1:
                    waits = list(si.on_wait)
                    for w in waits[:-1]:
                        n += 1
                        nop = mybir.InstNoOp(name=f"WSPLIT-{n}", engine=ins.engine)
                        nop.sync_info = mybir.SyncInfo(on_wait=[w], on_update=[])
                        insts.insert(i, nop)
                        i += 1
                    ins.sync_info = mybir.SyncInfo(
                        on_wait=[waits[-1]], on_update=si.on_update)
                i += 1
    return n


def _build_bass():
    import concourse.bass as bass
    import concourse.mybir as mybir
    import concourse.tile as tile
    from concourse.masks import make_identity
    from contextlib import ExitStack

    F32 = mybir.dt.float32
    F32R = mybir.dt.float32r if USE_F32R else mybir.dt.float32
    BF = mybir.dt.bfloat16
    I16 = mybir.dt.int16
    AF = mybir.ActivationFunctionType
    ALU = mybir.AluOpType

    nc = bass.Bass()
    xbf = nc.declare_dram_parameter("xbf", [C, N], BF, isOutput=False)
    x_own = nc.declare_dram_parameter("x_own", [C, NH], F32, isOutput=False)
    ctxt = nc.declare_dram_parameter("ctxt", [CTXL, CTXD], F32, isOutput=False)
    wq = nc.declare_dram_parameter("wq", [C, C], BF, isOutput=False)
    negcq = nc.declare_dram_parameter("negcq", [1, C], BF, isOutput=False)
    wkvc = nc.declare_dram_parameter("wkvc", [C, 128], BF, isOutput=False)
    ncskv = nc.declare_dram_parameter("ncskv", [1, 128], BF, isOutput=False)
    wctx = nc.declare_dram_parameter("wctx", [CTXD, 2 * DH], F32, isOutput=False)
    bctxk = nc.declare_dram_parameter("bctxk", [DH, 1], F32, isOutput=False)
    bctxv = nc.declare_dram_parameter("bctxv", [DH, 1], F32, isOutput=False)
    nullkt = nc.declare_dram_parameter("nullkt", [DH, 1], F32, isOutput=False)
    nullv = nc.declare_dram_parameter("nullv", [DH, 1], F32, isOutput=False)
    wout = nc.declare_dram_parameter("wout", [128, (HEADS // 2) * C], BF,
                                     isOutput=False)
    selin = nc.declare_dram_parameter("selin", [8, 8 * DH], F32, isOutput=False)
    outgr = nc.declare_dram_parameter("outgr", [1, C], F32, isOutput=False)
    y = nc.declare_dram_parameter("y", [C, NH], F32, isOutput=True)

    with tile.TileContext(nc) as tc, ExitStack() as ctx:
        pconst = ctx.enter_context(tc.tile_pool(name="const", bufs=1))
        pbig = ctx.enter_context(tc.tile_pool(name="big", bufs=1))

        ident = pconst.tile([128, 128], F32)
        make_identity(nc, ident[:])
        ident_bf = pconst.tile([128, 128], BF)
        make_identity(nc, ident_bf[:])
        ones_col = pconst.tile([128, 1], F32)
        nc.vector.memset(ones_col[:], 1.0)
        ones_col_bf = pconst.tile([128, 1], BF)
        nc.vector.memset(ones_col_bf[:], 1.0)
        ones_f = pconst.tile([1, 128], F32)
        nc.vector.memset(ones_f[:], 1.0)
        ones_r = pconst.tile([1, 128], F32R)
        nc.vector.tensor_copy(ones_r[:], ones_f[:])
        eps_col = pconst.tile([128, 1], F32)
        nc.vector.memset(eps_col[:], EPS)
        ones_blk = pconst.tile([128, 64], F32)
        nc.vector.memset(ones_blk[:], 1.0)
        sel_f = pconst.tile([8, 8 * DH], F32)
        sel = pconst.tile([8, 8 * DH], F32R)
        outgr_f = pconst.tile([1, C], F32)
        outgr_r = pconst.tile([1, C], F32R)

        x_sb = pbig.tile([128, KT * NH], F32)        # kt-major; OWN half only
        x_bf = pbig.tile([128, 6 * KT * 384], BF)   # chunk-major: [ch][kt][384] (kept)
        qT = pbig.tile([128, (HEADS // 2) * NH], BF)  # head-pair blocks
        kT2 = pbig.tile([128, JP], BF)               # rs-scaled keys, both halves
        v_sb = pbig.tile([128, JT * 128], BF)        # per j-tile [v 0:64|ones 64|pad]
        v_sb2 = pbig.tile([128, JT * 128], BF)       # odd-head: [0|ones@32|0|v 64:128]
        projBF = pbig.tile([128, KT * 512], BF)      # bf16 proj (per chunk)
        stats = pbig.tile([128, 40], F32)            # col jt: rs_j (v scaling)
        wout_sb = pbig.tile([128, (HEADS // 2) * C], BF)  # head-pair stacked
        # per-token stat rows on partition 0: mu 0:N | rs N:2N
        # (LN2 reuses per cc: mu2 at cc*CH, rs2 at N+cc*CH, ex2 at 2N+cc*CH)
        rows = pbig.tile([1, 2 * N + NH], F32)
        rows_bf = pbig.tile([1, N], BF)
        rows_r = pbig.tile([1, N], F32R)             # f32r copy of rs row
        R_RS, R_SC = N, 2 * N

        nc.sync.dma_start(sel_f[:], selin[:, :])
        nc.sync.dma_start(outgr_f[:], outgr[:, :])
        nc.vector.tensor_copy(sel[:], sel_f[:])
        nc.vector.tensor_copy(outgr_r[:], outgr_f[:])
        nc.sync.dma_start(wout_sb[:], wout[:, :])

        with tc.tile_pool(name="load", bufs=1) as pload, \
             tc.tile_pool(name="x2p", bufs=2) as px2, \
             tc.tile_pool(name="pss", bufs=2, space="PSUM") as pss:
            # psum tags: b1 [<=64,384]x2, bS [128,<=512]x4, bT [128,128]x2
            wq_sb = pload.tile([128, KT * C], BF)
            wkv_sb = pload.tile([128, KT * 128], BF)
            wctx_sb = pload.tile([128, CTXD], F32)
            negcq_sb = pload.tile([1, C], BF)
            ncskv_sb = pload.tile([1, 128], BF)
            bctxk_sb = pload.tile([DH, 1], F32)
            bctxv_sb = pload.tile([DH, 1], F32)
            vT = pload.tile([64, N], BF)
            ck_sb = pload.tile([64, CTXL], F32)
            cv_sb = pload.tile([64, CTXL + 1], F32)
            nullk_st = pload.tile([DH, 1], F32)
            nullv_st = pload.tile([DH, 1], F32)
            ctx_sb = pload.tile([CTXL, CTXD], F32)
            ctxnT = pload.tile([128, 6 * CTXL], F32)
            ex2 = pload.tile([1, N], F32)
            kk = pload.tile([128, 512], F32)

            # PE warmup: ~4us of back-to-back junk matmuls while the DMAs
            # stream in, so the tensor engine reaches its fast pstate
            # before LN1's first real matmul.
            ps_wu = pss.tile([128, 128], F32, tag="bT")
            for _ in range(12):
                nc.tensor.matmul(ps_wu[:, :], ones_f[:], ones_f[:],
                                 start=True, stop=True)

            # x_bf chunks on the ACT queue so LN1 starts while the SP
            # queue streams ctx/weights. SBUF x_bf is chunk-major.
            xbf_d = xbf[:].rearrange("(k p) n -> p k n", p=128)
            xv4 = x_bf[:].rearrange("p (c k n) -> p c k n", c=6, k=KT)
            for ch in range(6):
                a, b = ch * 384, (ch + 1) * 384
                nc.scalar.dma_start(xv4[:, ch, :, :], xbf_d[:, :, a:b])
            nc.sync.dma_start(ctx_sb[:], ctxt[:, :])
            nc.sync.dma_start(wctx_sb[:].rearrange("p (k n) -> p k n", k=6),
                              wctx[:].rearrange("(k p) n -> p k n", p=128))
            nc.sync.dma_start(bctxk_sb[:], bctxk[:, :])
            nc.sync.dma_start(bctxv_sb[:], bctxv[:, :])
            nc.sync.dma_start(nullk_st[:], nullkt[:, :])
            nc.sync.dma_start(nullv_st[:], nullv[:, :])
            nc.sync.dma_start(wkv_sb[:].rearrange("p (k n) -> p k n", k=KT),
                              wkvc[:].rearrange("(k p) n -> p k n", p=128))
            nc.sync.dma_start(ncskv_sb[:], ncskv[:, :])
            nc.scalar.dma_start(wq_sb[:].rearrange("p (k n) -> p k n", k=KT),
                                wq[:].rearrange("(k p) n -> p k n", p=128))
            nc.sync.dma_start(negcq_sb[:], negcq[:, :])
            x_v = x_sb[:].rearrange("p (k n) -> p k n", k=KT)
            nc.sync.dma_start(x_v[:, :, 0:NH],
                              x_own[:].rearrange("(k p) n -> p k n", p=128))

            # ---- fused per-384-token-chunk pipeline:
            # LN1 stats -> kv projection -> v tiles for the 3 j-tiles,
            # software-pipelined with the x_bf chunk DMAs.
            nc.gpsimd.memset(v_sb[:], 0.0)
            nc.gpsimd.memset(v_sb2[:], 0.0)
            nc.gpsimd.memset(kT2[0:64, 18 * 128 : JP], 0.0)
            for ch in range(6):
                sl = slice(ch * 384, (ch + 1) * 384)
                ps_r1 = pss.tile([64, 384], F32, tag="b1")
                for kt in range(KT):
                    xo = (ch * KT + kt) * 384
                    nc.tensor.matmul(
                        ps_r1[0:1, :], ones_col_bf[:],
                        x_bf[:, xo : xo + 384],
                        start=(kt == 0), stop=(kt == KT - 1))
                nc.scalar.mul(rows[0:1, sl], ps_r1[0:1, :], 1.0 / C)
                nc.vector.tensor_copy(rows_bf[0:1, sl], rows[0:1, sl])
                x2 = px2.tile([128, KT * 384], BF, tag="x2")
                ps_r2 = pss.tile([64, 384], F32, tag="b1")
                for kt in range(KT):
                    xs = x_bf[:, (ch * KT + kt) * 384 : (ch * KT + kt + 1) * 384]
                    nc.vector.tensor_mul(x2[:, kt * 384 : (kt + 1) * 384], xs, xs)
                    nc.tensor.matmul(
                        ps_r2[0:1, :], ones_col_bf[:],
                        x2[:, kt * 384 : (kt + 1) * 384],
                        start=(kt == 0), stop=(kt == KT - 1))
                nc.scalar.mul(ex2[0:1, ch * 384 : (ch + 1) * 384],
                              ps_r2[0:1, :], 1.0 / C)
                a, b = R_RS + ch * 384, R_RS + (ch + 1) * 384
                mu = rows[0:1, ch * 384 : (ch + 1) * 384]
                nc.vector.tensor_mul(rows[0:1, a:b], mu, mu)
                nc.vector.tensor_sub(rows[0:1, a:b],
                                     ex2[0:1, ch * 384 : (ch + 1) * 384],
                                     rows[0:1, a:b])
                nc.scalar.activation(rows[0:1, a:b], rows[0:1, a:b], AF.Ln,
                                     bias=eps_col[0:1, :])
                nc.scalar.activation(rows[0:1, a:b], rows[0:1, a:b], AF.Exp,
                                     scale=-0.5)
                nc.vector.tensor_copy(rows_r[0:1, sl], rows[0:1, a:b])
                # kv projection for this chunk (LN folded; k -> kT2 top half)
                kvl = 384
                ps_kv = pss.tile([128, 512], F32, tag="bS")
                for kt in range(KT):
                    xs = x_bf[:, (ch * KT + kt) * 384 : (ch * KT + kt + 1) * 384]
                    nc.tensor.matmul(ps_kv[:, 0:kvl],
                                     wkv_sb[:, kt * 128 : (kt + 1) * 128],
                                     xs, start=(kt == 0), stop=False)
                nc.tensor.matmul(ps_kv[:, 0:kvl], ncskv_sb[:], rows_bf[0:1, sl],
                                 start=False, stop=True)
                ps_bc = pss.tile([128, 512], F32, tag="bS")
                nc.tensor.matmul(ps_bc[:, 0:kvl], ones_r[0:1, 0:128],
                                 rows_r[0:1, sl])
                nc.vector.tensor_copy(kk[64:128, 0:kvl], ps_kv[64:128, 0:kvl])
                nc.vector.tensor_mul(kT2[64:128, sl], kk[64:128, 0:kvl],
                                     ps_bc[64:128, 0:kvl])
                nc.vector.tensor_copy(vT[:, sl], ps_kv[0:64, 0:kvl])
                # v tiles + rs columns for the 3 j-tiles of this chunk
                for jt in range(3 * ch, 3 * ch + 3):
                    ps_c = pss.tile([128, 128], F32, tag="bT")
                    nc.tensor.matmul(
                        ps_c[:, 0:1],
                        rows[0:1, R_RS + jt * 128 : R_RS + (jt + 1) * 128],
                        ones_col[0:1, :])
                    nc.vector.tensor_copy(stats[:, jt : jt + 1], ps_c[:, 0:1])
                    ps_vt = pss.tile([128, 128], BF, tag="bT")
                    nc.tensor.transpose(ps_vt[:, 0:64],
                                        vT[:, jt * 128 : (jt + 1) * 128],
                                        ident_bf[:64, :64])
                    vb = jt * 128
                    nc.vector.tensor_scalar_mul(v_sb[:, vb : vb + DH],
                                                ps_vt[:, 0:64],
                                                stats[:, jt : jt + 1])
                    nc.vector.memset(v_sb[:, vb + DH : vb + DH + 1], 1.0)
                    nc.vector.tensor_scalar_mul(v_sb2[:, vb + 64 : vb + 128],
                                                ps_vt[:, 0:64],
                                                stats[:, jt : jt + 1])
                    nc.vector.memset(v_sb2[:, vb + 32 : vb + 33], 1.0)

            # ---- context: LN (layout A, bn_stats) + k/v projection ----
            cstat = pload.tile([CTXL, 3, 6], F32)
            for sg in range(3):
                nc.vector.bn_stats(cstat[:, sg, :],
                                   ctx_sb[:, sg * 256 : (sg + 1) * 256])
            cmv = pload.tile([CTXL, 2], F32)
            nc.vector.bn_aggr(cmv[:], cstat[:])
            nc.scalar.activation(cmv[:, 1:2], cmv[:, 1:2], AF.Ln,
                                 bias=eps_col[0:CTXL, :])
            nc.scalar.activation(cmv[:, 1:2], cmv[:, 1:2], AF.Exp, scale=-0.5)
            nc.vector.tensor_scalar(
                out=ctx_sb[:], in0=ctx_sb[:],
                scalar1=cmv[:, 0:1], scalar2=cmv[:, 1:2],
                op0=ALU.subtract, op1=ALU.mult)
            for kt in range(6):
                ps_ct = pss.tile([128, 128], F32, tag="bT")
                nc.tensor.transpose(ps_ct[:, 0:CTXL],
                                    ctx_sb[:, kt * 128 : (kt + 1) * 128],
                                    ident[:CTXL, :CTXL])
                nc.vector.tensor_copy(ctxnT[:, kt * CTXL : (kt + 1) * CTXL],
                                      ps_ct[:, 0:CTXL])
            ps_ck = pss.tile([64, 384], F32, tag="b1")
            ps_cv = pss.tile([64, 384], F32, tag="b1")
            for kt in range(6):
                nc.tensor.matmul(ps_ck[:, 0:CTXL],
                                 wctx_sb[:, kt * 128 : kt * 128 + DH],
                                 ctxnT[:, kt * CTXL : (kt + 1) * CTXL],
                                 start=(kt == 0), stop=(kt == 5))
                nc.tensor.matmul(ps_cv[:, 0:CTXL],
                                 wctx_sb[:, kt * 128 + DH : (kt + 1) * 128],
                                 ctxnT[:, kt * CTXL : (kt + 1) * CTXL],
                                 start=(kt == 0), stop=(kt == 5))
            nc.vector.tensor_scalar_add(ck_sb[:], ps_ck[:, 0:CTXL], bctxk_sb[:])
            nc.vector.tensor_scalar_add(cv_sb[:, 0:CTXL], ps_cv[:, 0:CTXL],
                                        bctxv_sb[:])
            nc.vector.tensor_copy(cv_sb[:, CTXL : CTXL + 1], nullv_st[:])

            # ---- j-tile 18: [ctx 0:77 | null 77 | pad 78:128] ----
            nc.vector.tensor_copy(kT2[0:64, 18 * 128 : 18 * 128 + CTXL], ck_sb[:])
            nc.vector.tensor_copy(kT2[0:64, 18 * 128 + CTXL : 18 * 128 + CTXL + 1],
                                  nullk_st[:])
            VB = 18 * 128
            ps_cvt = pss.tile([128, 128], F32, tag="bT")
            nc.tensor.transpose(ps_cvt[0 : CTXL + 1, 0:64], cv_sb[:],
                                ident[:64, :64])
            nc.vector.tensor_copy(v_sb[0 : CTXL + 1, VB : VB + DH],
                                  ps_cvt[0 : CTXL + 1, 0:64])
            nc.vector.memset(v_sb[0 : CTXL + 1, VB + DH : VB + DH + 1], 1.0)
            nc.vector.tensor_copy(v_sb2[0 : CTXL + 1, VB + 64 : VB + 128],
                                  ps_cvt[0 : CTXL + 1, 0:64])
            nc.vector.memset(v_sb2[0 : CTXL + 1, VB + 32 : VB + 33], 1.0)

            # ---- duplicate kT to partitions 0:64 (sbuf->sbuf DMA) ----
            nc.sync.dma_start(kT2[0:64, 0 : 18 * 128], kT2[64:128, 0 : 18 * 128])
            nc.sync.dma_start(kT2[64:128, 18 * 128 : JP],
                              kT2[0:64, 18 * 128 : JP])

            # ---- q projection (head pairs; LN + 1/sqrt(dh) folded) ----
            for a0, ln in CHUNKS:
                sl = slice(a0, a0 + ln)
                ps_rs = pss.tile([128, 512], F32, tag="bS")
                nc.tensor.matmul(ps_rs[:, 0:ln], ones_r[0:1, :],
                                 rows_r[0:1, sl])
                rs_b = px2.tile([128, 512], F32, tag="rsb")
                nc.vector.tensor_copy(rs_b[:, 0:ln], ps_rs[:, 0:ln])
                for hg in range(HEADS // 2):
                    ps_q = pss.tile([128, 512], F32, tag="bS")
                    for kt in range(KT):
                        nc.tensor.matmul(
                            ps_q[:, 0:ln],
                            wq_sb[:, kt * C + hg * 128 : kt * C + (hg + 1) * 128],
                            x_bf[:, kt * N + a0 : kt * N + a0 + ln],
                            start=(kt == 0), stop=False)
                    nc.tensor.matmul(ps_q[:, 0:ln],
                                     negcq_sb[0:1, hg * 128 : (hg + 1) * 128],
                                     rows_bf[0:1, sl], start=False, stop=True)
                    nc.vector.tensor_mul(
                        qT[:, hg * NH + a0 : hg * NH + a0 + ln],
                        ps_q[:, 0:ln], rs_b[:, 0:ln])

        # ========= attention + output + LN2 + residual, per chunk =========
        # Per (chunk, head-pair): sim pair (row-tiled concurrent) -> exp on
        # ACT or DVE (bit-trick) -> attn.v pair lagging one j-tile. The tail
        # (recip batch, norm, out-proj, LN2, y) is deferred one stage so it
        # never head-of-line-blocks the PE queue.
        with tc.tile_pool(name="attb", bufs=6) as patb, \
             tc.tile_pool(name="atti", bufs=3) as pati, \
             tc.tile_pool(name="outp", bufs=6) as pout, \
             tc.tile_pool(name="pocp", bufs=6) as ppoc, \
             tc.tile_pool(name="denp", bufs=2) as pden, \
             tc.tile_pool(name="recp", bufs=2) as prec, \
             tc.tile_pool(name="rowp", bufs=1) as prow, \
             tc.tile_pool(name="p2p", bufs=1) as pp2, \
             tc.tile_pool(name="yp", bufs=2) as pyt, \
             tc.tile_pool(name="psatt", bufs=2, space="PSUM") as psA, \
             tc.tile_pool(name="psacc", bufs=4, space="PSUM") as psB:

            def at_tile(engine):
                if engine == "act":
                    return patb.tile([128, 1024], BF, tag="atb", name="atb")
                return pati.tile([128, 1024], I16, tag="ati", name="ati")

            def do_exp(engine, at, ps_s, lo, hi):
                if engine == "act":
                    nc.scalar.activation(at[:, lo:hi], ps_s[:, lo:hi], AF.Exp)
                else:
                    nc.vector.tensor_scalar(
                        out=at[:, lo:hi], in0=ps_s[:, lo:hi],
                        scalar1=FEXP_S, scalar2=FEXP_C,
                        op0=ALU.mult, op1=ALU.add)

            def at_slice(at_eng, lo, hi):
                at, eng = at_eng
                ap = at[:, lo:hi]
                return ap if eng == "act" else ap.bitcast(BF)

            def run_hg_512(cc, hg, a0, ln, dent, pocs, workq):
                po0 = psB.tile([128, 512], F32, tag="po")
                po1 = psB.tile([128, 512], F32, tag="po")
                po = [po0, po1]
                q0 = qT[0:64, hg * NH + a0 : hg * NH + a0 + ln]
                q1 = qT[64:128, hg * NH + a0 : hg * NH + a0 + ln]
                ats = [None] * JT
                for jt in range(JT):
                    ps_s = psA.tile([128, 1024], F32, tag="sim")
                    nc.tensor.matmul(ps_s[:, 0:ln],
                                     kT2[0:64, jt * 128 : (jt + 1) * 128],
                                     q0, start=True, stop=True)
                    nc.tensor.matmul(ps_s[:, 512 : 512 + ln],
                                     kT2[64:128, jt * 128 : (jt + 1) * 128],
                                     q1, start=True, stop=True)
                    eng = _exp_engine(cc, hg, jt)
                    at = at_tile(eng)
                    do_exp(eng, at, ps_s, 0, 1024)
                    ats[jt] = (at, eng)
                    if jt >= ALAG:
                        j0 = jt - ALAG
                        nc.tensor.matmul(po[0][:, 0:ln],
                                         v_sb[:, j0 * 128 : (j0 + 1) * 128],
                                         at_slice(ats[j0], 0, ln),
                                         start=(j0 == 0), stop=False)
                        nc.tensor.matmul(po[1][:, 0:ln],
                                         v_sb2[:, j0 * 128 : (j0 + 1) * 128],
                                         at_slice(ats[j0], 512, 512 + ln),
                                         start=(j0 == 0), stop=False)
                        ats[j0] = None
                for j0 in range(JT - ALAG, JT):
                    nc.tensor.matmul(po[0][:, 0:ln],
                                     v_sb[:, j0 * 128 : (j0 + 1) * 128],
                                     at_slice(ats[j0], 0, ln),
                                     start=(j0 == 0), stop=(j0 == JT - 1))
                    nc.tensor.matmul(po[1][:, 0:ln],
                                     v_sb2[:, j0 * 128 : (j0 + 1) * 128],
                                     at_slice(ats[j0], 512, 512 + ln),
                                     start=(j0 == 0), stop=(j0 == JT - 1))
                _extract(hg, ln, po, dent, pocs)

            def run_hg_128(cc, hg, a0, ln, dent, pocs, workq):
                # 19 j-tiles packed 4-per-psum-tile; slot g: q0 at col 128g
                # (bank A), q1 at 512+128g (bank B) so the row-tiled
                # concurrent sim pair never co-writes one psum bank.
                po0 = psB.tile([128, 512], F32, tag="po")
                po1 = psB.tile([128, 512], F32, tag="po")
                po = [po0, po1]
                q0 = qT[0:64, hg * NH + a0 : hg * NH + a0 + ln]
                q1 = qT[64:128, hg * NH + a0 : hg * NH + a0 + ln]
                GRP = [(0, 4), (4, 4), (8, 4), (12, 4), (16, 3)]
                ats = [None] * len(GRP)
                for gi, (jt0, ng) in enumerate(GRP):
                    ps_s = psA.tile([128, 1024], F32, tag="sim")
                    for g in range(ng):
                        jt = jt0 + g
                        nc.tensor.matmul(
                            ps_s[:, 128 * g : 128 * g + ln],
                            kT2[0:64, jt * 128 : (jt + 1) * 128],
                            q0, start=True, stop=True)
                        nc.tensor.matmul(
                            ps_s[:, 512 + 128 * g : 512 + 128 * g + ln],
                            kT2[64:128, jt * 128 : (jt + 1) * 128],
                            q1, start=True, stop=True)
                    eng = _exp_engine_128(hg, gi)
                    at = at_tile(eng)
                    if ng == 4:
                        do_exp(eng, at, ps_s, 0, 1024)
                    else:
                        do_exp(eng, at, ps_s, 0, 128 * ng)
                        do_exp(eng, at, ps_s, 512, 512 + 128 * ng)
                    ats[gi] = (at, eng)
                    if gi > 0:
                        _attnv_128(po, ats[gi - 1], GRP[gi - 1], ln,
                                   start=(gi == 1), stop=False)
                        ats[gi - 1] = None
                _attnv_128(po, ats[-1], GRP[-1], ln, start=False, stop=True)
                _extract(hg, ln, po, dent, pocs)

            def _attnv_128(po, at_eng, grp, ln, start, stop):
                jt0, ng = grp
                for g in range(ng):
                    jt = jt0 + g
                    last = stop and (g == ng - 1)
                    nc.tensor.matmul(po[0][:, 0:ln],
                                     v_sb[:, jt * 128 : (jt + 1) * 128],
                                     at_slice(at_eng, 128 * g, 128 * g + ln),
                                     start=(start and g == 0), stop=last)
                    nc.tensor.matmul(po[1][:, 0:ln],
                                     v_sb2[:, jt * 128 : (jt + 1) * 128],
                                     at_slice(at_eng, 512 + 128 * g,
                                              512 + 128 * g + ln),
                                     start=(start and g == 0), stop=last)

            def _extract(hg, ln, po, dent, pocs):
                # stacked ot: even head out -> partitions 0:64 (from po0),
                # odd head out -> 64:128 (po1; its v sat cols 64:128).
                # dens: po0 row 64 (even), po1 row 32 (odd).
                otp = ppoc.tile([128, 512], BF, tag="poc")
                nc.vector.tensor_copy(otp[0:64, 0:ln], po[0][0:64, 0:ln])
                nc.vector.tensor_copy(otp[64:128, 0:ln], po[1][64:128, 0:ln])
                stg = ppoc.tile([65, 512], BF, tag="stg")
                nc.vector.tensor_copy(stg[64:65, 0:ln], po[0][64:65, 0:ln])
                nc.vector.tensor_copy(stg[32:33, 0:ln], po[1][32:33, 0:ln])
                nc.sync.dma_start(dent[2 * hg : 2 * hg + 1, 0:ln],
                                  stg[64:65, 0:ln])
                nc.sync.dma_start(dent[2 * hg + 1 : 2 * hg + 2, 0:ln],
                                  stg[32:33, 0:ln])
                pocs.append(otp)

            def tail_phases(cc, a0, ln, dent, pocs):
                ots = []
                rec8r = prec.tile([8, 512], F32R, tag="rec")
                rs2r = prow.tile([1, 2 * 512], F32R, tag="r2")

                def th_recip():
                    dentf = prec.tile([8, 512], F32, tag="dentf")
                    nc.vector.tensor_copy(dentf[:, 0:ln], dent[:, 0:ln])
                    rec8 = prec.tile([8, 512], F32, tag="rec8")
                    nc.vector.reciprocal(rec8[:, 0:ln], dentf[:, 0:ln])
                    nc.vector.tensor_copy(rec8r[:, 0:ln], rec8[:, 0:ln])

                def norm_pair(hg):
                    def th():
                        ps_rb = psB.tile([128, 512], F32, tag="po")
                        nc.tensor.matmul(ps_rb[:, 0:ln],
                                         sel[:, hg * 128 : (hg + 1) * 128],
                                         rec8r[:, 0:ln])
                        ot = pout.tile([128, 512], BF, tag="ot", name="ot")
                        nc.vector.tensor_mul(ot[:, 0:ln], pocs[hg][:, 0:ln],
                                             ps_rb[:, 0:ln])
                        ots.append(ot)
                    return th

                def proj_ct(ct):
                    def th():
                        ps_p = psB.tile([128, 512], F32, tag="po")
                        for hg in range(HEADS // 2):
                            nc.tensor.matmul(
                                ps_p[:, 0:ln],
                                wout_sb[:, hg * C + ct * 128 : hg * C + (ct + 1) * 128],
                                ots[hg][:, 0:ln],
                                start=(hg == 0), stop=(hg == HEADS // 2 - 1))
                        nc.vector.tensor_copy(
                            projBF[:, ct * 512 : ct * 512 + ln],
                            ps_p[:, 0:ln])
                    return th

                def th_mean():
                    ps_m2 = psB.tile([128, 512], F32, tag="po")
                    for ct in range(KT):
                        nc.tensor.matmul(ps_m2[0:1, 0:ln], ones_col_bf[:],
                                         projBF[:, ct * 512 : ct * 512 + ln],
                                         start=(ct == 0), stop=(ct == KT - 1))
                    nc.scalar.mul(rows[0:1, a0 : a0 + ln], ps_m2[0:1, 0:ln],
                                  1.0 / C)

                def th_var():
                    sca, scb = R_SC + a0, R_SC + a0 + ln
                    p2 = pp2.tile([128, KT * 512], BF, tag="p2")
                    ps_q2 = psB.tile([128, 512], F32, tag="po")
                    for ct in range(KT):
                        pslc = projBF[:, ct * 512 : ct * 512 + ln]
                        eng2 = nc.gpsimd if USE_GPS_TAIL else nc.vector
                        eng2.tensor_mul(p2[:, ct * 512 : ct * 512 + ln],
                                        pslc, pslc)
                        nc.tensor.matmul(ps_q2[0:1, 0:ln], ones_col_bf[:],
                                         p2[:, ct * 512 : ct * 512 + ln],
                                         start=(ct == 0), stop=(ct == KT - 1))
                    nc.scalar.mul(rows[0:1, sca:scb], ps_q2[0:1, 0:ln], 1.0 / C)

                def th_rows():
                    ra, rb2 = R_RS + a0, R_RS + a0 + ln
                    sca, scb = R_SC + a0, R_SC + a0 + ln
                    nc.vector.tensor_mul(rows[0:1, ra:rb2],
                                         rows[0:1, a0 : a0 + ln],
                                         rows[0:1, a0 : a0 + ln])
                    nc.vector.tensor_sub(rows[0:1, ra:rb2], rows[0:1, sca:scb],
                                         rows[0:1, ra:rb2])
                    nc.scalar.activation(rows[0:1, ra:rb2], rows[0:1, ra:rb2],
                                         AF.Ln, bias=eps_col[0:1, :])
                    nc.scalar.activation(rows[0:1, ra:rb2], rows[0:1, ra:rb2],
                                         AF.Exp, scale=-0.5)
                    nc.vector.tensor_mul(rows[0:1, sca:scb],
                                         rows[0:1, a0 : a0 + ln],
                                         rows[0:1, ra:rb2])
                    nc.vector.tensor_copy(rs2r[0:1, 0:ln], rows[0:1, ra:rb2])
                    nc.vector.tensor_copy(rs2r[0:1, 512 : 512 + ln],
                                          rows[0:1, sca:scb])

                def y_ct(ct):
                    def th():
                        ps_gb = psB.tile([128, 512], F32, tag="po")
                        gsl = outgr_r[0:1, ct * 128 : (ct + 1) * 128]
                        nc.tensor.matmul(ps_gb[:, 0:ln], gsl, rs2r[0:1, 0:ln])
                        yt = pyt.tile([128, 512], F32, tag="yt", name="yt")
                        pslice = projBF[:, ct * 512 : ct * 512 + ln]
                        nc.vector.tensor_mul(yt[:, 0:ln], pslice, ps_gb[:, 0:ln])
                        ps_gm = psB.tile([128, 512], F32, tag="po")
                        nc.tensor.matmul(ps_gm[:, 0:ln], gsl,
                                         rs2r[0:1, 512 : 512 + ln])
                        nc.vector.tensor_sub(yt[:, 0:ln], yt[:, 0:ln],
                                             ps_gm[:, 0:ln])
                        eng3 = nc.gpsimd if USE_GPS_TAIL else nc.vector
                        eng3.tensor_add(
                            yt[:, 0:ln], yt[:, 0:ln],
                            x_sb[:, ct * NH + a0 : ct * NH + a0 + ln])
                        nc.sync.dma_start(
                            y[ct * 128 : (ct + 1) * 128, a0 : a0 + ln],
                            yt[:, 0:ln])
                    return th

                return ([th_recip] + [norm_pair(hg) for hg in range(4)]
                        + [proj_ct(ct) for ct in range(KT)]
                        + [th_mean, th_var, th_rows]
                        + [y_ct(ct) for ct in range(KT)])

            workq = []
            for cc, (a0, ln) in enumerate(CHUNKS):
                pocs = []
                dent = pden.tile([8, 512], BF, tag="dent", name="dent")
                GSZ = [3, 2, 4, 7]
                for hg in range(HEADS // 2):
                    if ln == 512:
                        run_hg_512(cc, hg, a0, ln, dent, pocs, workq)
                    else:
                        run_hg_128(cc, hg, a0, ln, dent, pocs, workq)
                    for _ in range(GSZ[hg]):
                        if workq:
                            workq.pop(0)()
                workq.extend(tail_phases(cc, a0, ln, dent, pocs))
            for th in workq:
                th()
    _split_multi_waits(nc)
    return nc


def _prep_inputs(x, context, norm_gamma, null_kv, Wq, Wkv, ctx_ln_g, ctx_ln_b,
                 Wctx, bctx, Wout, out_ln_g):
    import ml_dtypes
    bf = ml_dtypes.bfloat16
    f = np.float32
    x = np.asarray(x, f).reshape(4, C, N)
    context = np.asarray(context, f)
    g = np.asarray(norm_gamma, f)
    scale = 1.0 / np.sqrt(DH)
    wq_h = (g[:, None] * np.asarray(Wq, f)) * scale
    negcq_h = -wq_h.sum(0, dtype=np.float64).astype(f)[None, :]
    wkv_h = g[:, None] * np.asarray(Wkv, f)
    # combined stationary: [v | k] so k lands on psum rows 64:128
    wkvc_h = np.concatenate([wkv_h[:, DH:], wkv_h[:, :DH]], axis=1)
    ncs = -wkv_h.sum(0, dtype=np.float64).astype(f)
    ncskv_h = np.concatenate([ncs[DH:], ncs[:DH]])[None, :]
    wctx_h = np.asarray(ctx_ln_g, f)[:, None] * np.asarray(Wctx, f)
    bctx_h = (np.asarray(bctx, f) + np.asarray(ctx_ln_b, f) @ np.asarray(Wctx, f))
    null = np.asarray(null_kv, f)
    W_o = np.asarray(Wout, f)
    # head-pair stacked: rows 0:64 = even head dims, 64:128 = odd head dims
    wout_b = np.concatenate(
        [np.concatenate([W_o[2 * hg * DH : (2 * hg + 1) * DH, :],
                         W_o[(2 * hg + 1) * DH : (2 * hg + 2) * DH, :]], axis=0)
         for hg in range(HEADS // 2)], axis=1)
    # pair selector: block hg, cols 0:64 -> row 2hg, cols 64:128 -> row 2hg+1
    sel_h = np.zeros((8, 4 * 128), f)
    for hg in range(4):
        sel_h[2 * hg, hg * 128 : hg * 128 + 64] = 1.0
        sel_h[2 * hg + 1, hg * 128 + 64 : (hg + 1) * 128] = 1.0

    def mirror(a):
        """[K*128, n] row-major -> SBUF mirror [128, K*n]."""
        K = a.shape[0] // 128
        return np.ascontiguousarray(
            a.reshape(K, 128, -1).transpose(1, 0, 2).reshape(128, -1))

    shared = {
        "wq": np.ascontiguousarray(wq_h).astype(bf),
        "negcq": negcq_h.astype(bf),
        "wkvc": np.ascontiguousarray(wkvc_h).astype(bf),
        "ncskv": np.ascontiguousarray(ncskv_h).astype(bf),
        "wctx": np.ascontiguousarray(wctx_h),
        "bctxk": np.ascontiguousarray(bctx_h[:DH, None]),
        "bctxv": np.ascontiguousarray(bctx_h[DH:, None]),
        "nullkt": np.ascontiguousarray(null[0][:, None]),
        "nullv": np.ascontiguousarray(null[1][:, None]),
        "wout": np.ascontiguousarray(wout_b).astype(bf),
        "selin": sel_h,
        "outgr": np.ascontiguousarray(np.asarray(out_ln_g, f)[None, :]),
    }
    in_maps = []
    for core in range(8):
        b, half = core // 2, core % 2
        m = dict(shared)
        xo = x[b][:, half * NH : (half + 1) * NH]
        xt = x[b][:, (1 - half) * NH : (2 - half) * NH]
        m["x_own"] = np.ascontiguousarray(xo)
        m["xbf"] = np.ascontiguousarray(
            np.concatenate([xo, xt], axis=1)).astype(bf)
        m["ctxt"] = np.ascontiguousarray(context[b])
        in_maps.append(m)
    return in_maps


_LDW_OPT = [False]


def _patch_ldw_opt():
    import concourse.bass_utils as bu
    if getattr(bu, "_ldwopt_patched", False):
        return
    orig = bu.run_command

    def run2(cmd, **kw):
        if _LDW_OPT[0]:
            cmd = [c.replace("--enable-ldw-opt=false", "--enable-ldw-opt=true")
                   for c in cmd]
        return orig(cmd, **kw)

    bu.run_command = run2
    bu._ldwopt_patched = True


def kernel(**inputs):
    from concourse.bass_utils import run_bass_kernel_spmd
    _patch_ldw_opt()

    if "nc" not in _cached:
        _cached["nc"] = _build_bass()
    nc = _cached["nc"]
    in_maps = _prep_inputs(**inputs)
    kw = {}
    if PROFILE:
        import importlib.util

        if "antenv.axon_hooks" not in sys.modules:
            spec = importlib.util.spec_from_file_location(
                "antenv.axon_hooks", "/opt/trn_rl_repo/antenv/axon_hooks.py")
            m = importlib.util.module_from_spec(spec)
            spec.loader.exec_module(m)
            sys.modules["antenv.axon_hooks"] = m
            import antenv

            antenv.axon_hooks = m
        kw = dict(trace=True, tmpdir=PROFILE_DIR)
    res = run_bass_kernel_spmd(nc, in_maps, list(range(8)), **kw)
    _cached["last"] = res
    out = np.empty((4, C, N), np.float32)
    for core in range(8):
        b, half = core // 2, core % 2
        out[b][:, half * NH : (half + 1) * NH] = res.results[core]["y"]
    return out.reshape(4, C, 48, 48)
